# revision 1
# baseline (speedup 1.0000x reference)
"""Self-contained Trainium2 Bass kernel for the int4-quantized 4-layer Llama decode problem.

Strategy: tensor-parallel over 8 NeuronCores (attention heads + FFN hidden dim),
sequence-parallel residual (each core keeps a feature-major fp32 residual shard
[D, T/8] in SBUF), AllGather before QKV/MLP, ReduceScatter after o-proj/down-proj.
Only the last position of each sequence goes through layer-4 Q/attention/MLP and
the LM head.
"""
import sys

sys.path.insert(0, '/opt/trn_rl_repo')

import numpy as np
import ml_dtypes
from contextlib import ExitStack

import concourse.bass as bass
import concourse.tile as tile
from concourse import bacc, mybir
from concourse import bass_utils

# model dims (hardcoded per problem spec)
L, D, H, HD, KVH, DFF, V, B, S = 4, 2048, 32, 64, 8, 8192, 32000, 4, 1024
NC = 8
T = B * S              # 4096 tokens
TB = T // NC           # 512 tokens per core shard
QH = H // NC           # 4 local q heads
DQ = QH * HD           # 256 local q dims
DFFS = DFF // NC       # 1024 local ffn dims
VS = V // NC           # 4000 local vocab
KC = D // 128          # 16 feature chunks
MFF = DFFS // 128      # 8
ROPE_THETA = 500000.0
NEG = np.float32(-1e9)
EPS = 1e-5

f32 = mybir.dt.float32
bf16 = mybir.dt.bfloat16
i32 = mybir.dt.int32

AX = mybir.AxisListType.X
MUL = mybir.AluOpType.mult
AF = mybir.ActivationFunctionType

_CACHE = {}
TLSIM = False  # single-core cost-model sim mode (collectives -> DMA copies)


# ----------------------------------------------------------------------------
# bass program
# ----------------------------------------------------------------------------

def _declare_inputs(nc):
    I = {}
    I['x0'] = nc.dram_tensor('x0', [D, TB], f32, kind='ExternalInput').ap()
    for l in range(L):
        I[f'wqkv{l}'] = nc.dram_tensor(f'wqkv{l}', [D, 384], i32, kind='ExternalInput').ap()
        I[f'sqkv{l}'] = nc.dram_tensor(f'sqkv{l}', [128, 3], f32, kind='ExternalInput').ap()
        I[f'wo{l}'] = nc.dram_tensor(f'wo{l}', [DQ, D], i32, kind='ExternalInput').ap()
        I[f'so{l}'] = nc.dram_tensor(f'so{l}', [128, KC], f32, kind='ExternalInput').ap()
        I[f'wg{l}'] = nc.dram_tensor(f'wg{l}', [D, DFFS], i32, kind='ExternalInput').ap()
        I[f'sg{l}'] = nc.dram_tensor(f'sg{l}', [128, MFF], f32, kind='ExternalInput').ap()
        I[f'wu{l}'] = nc.dram_tensor(f'wu{l}', [D, DFFS], i32, kind='ExternalInput').ap()
        I[f'wd{l}'] = nc.dram_tensor(f'wd{l}', [DFFS, D], i32, kind='ExternalInput').ap()
        I[f'sud{l}'] = nc.dram_tensor(f'sud{l}', [128, MFF], f32, kind='ExternalInput').ap()
        I[f'sd{l}'] = nc.dram_tensor(f'sd{l}', [128, KC], f32, kind='ExternalInput').ap()
        I[f'ga{l}'] = nc.dram_tensor(f'ga{l}', [128, KC], f32, kind='ExternalInput').ap()
        I[f'gm{l}'] = nc.dram_tensor(f'gm{l}', [128, KC], f32, kind='ExternalInput').ap()
    I['gf'] = nc.dram_tensor('gf', [128, KC], f32, kind='ExternalInput').ap()
    I['cosq'] = nc.dram_tensor('cosq', [128, T], bf16, kind='ExternalInput').ap()
    I['sinq'] = nc.dram_tensor('sinq', [128, T], bf16, kind='ExternalInput').ap()
    I['cosq4'] = nc.dram_tensor('cosq4', [128, 4], f32, kind='ExternalInput').ap()
    I['sinq4'] = nc.dram_tensor('sinq4', [128, 4], f32, kind='ExternalInput').ap()
    I['trimask'] = nc.dram_tensor('trimask', [128, 128], f32, kind='ExternalInput').ap()
    I['identbf'] = nc.dram_tensor('identbf', [128, 128], bf16, kind='ExternalInput').ap()
    I['ones_k'] = nc.dram_tensor('ones_k', [128, 1], f32, kind='ExternalInput').ap()
    I['ones_m'] = nc.dram_tensor('ones_m', [1, 128], f32, kind='ExternalInput').ap()
    I['onebf'] = nc.dram_tensor('onebf', [1, 1], bf16, kind='ExternalInput').ap()
    I['embT'] = nc.dram_tensor('embT', [D, VS], f32, kind='ExternalInput').ap()
    return I


def _fm_norm(nc, tc, ctx, src, gamma_ap, width, out_tiles, tag):
    """Feature-major rmsnorm: src = list of KC sbuf [128,width] f32 tiles.
    Writes out_tiles (KC tiles, caller-allocated, any dtype)."""
    with tc.tile_pool(name=f'np_{tag}', bufs=2) as sp, \
         tc.tile_pool(name=f'npp_{tag}', bufs=2, space='PSUM') as pp:
        C = ctx['const']
        ssum = pp.tile([1, width], f32, name=f'nsum_{tag}')
        for k in range(KC):
            xsq = sp.tile([128, width], f32, name=f'nxsq_{tag}', bufs=3)
            nc.vector.tensor_mul(xsq[:], src[k][:], src[k][:])
            nc.tensor.matmul(ssum[:], C['ones_k'][:], xsq[:],
                             start=(k == 0), stop=(k == KC - 1))
        sq = sp.tile([1, width], f32, name=f'nsq_{tag}')
        nc.scalar.activation(sq[:], ssum[:], AF.Sqrt, bias=ctx['eps'][0:1, :],
                             scale=1.0 / D)
        rstd = sp.tile([1, width], f32, name=f'nrstd_{tag}')
        nc.vector.reciprocal(rstd[:], sq[:])
        bcp = pp.tile([128, width], f32, name=f'nbc_{tag}')
        nc.tensor.matmul(bcp[:], C['ones_m'][:], rstd[:], start=True, stop=True)
        rbc = sp.tile([128, width], f32, name=f'nrbc_{tag}')
        nc.scalar.copy(rbc[:], bcp[:])
        for k in range(KC):
            nc.vector.scalar_tensor_tensor(
                out=out_tiles[k][:], in0=src[k][:], scalar=gamma_ap[:, k:k + 1],
                in1=rbc[:], op0=MUL, op1=MUL)


def _dequant_weights(nc, tc, pool, stage_pool, dram_ap, ncols, nchunks, name,
                     scale_ap=None, stage_tag='ist'):
    """Load int32 [nchunks*128, ncols] lhsT weights, cast to bf16 tiles."""
    tiles = []
    cc = min(ncols, 1024)
    for k in range(nchunks):
        wt = pool.tile([128, ncols], bf16, name=f'{name}{k}')
        for c0 in range(0, ncols, cc):
            ist = stage_pool.tile([128, cc], i32, name=stage_tag, bufs=3)
            nc.sync.dma_start(out=ist[:],
                              in_=dram_ap[k * 128:(k + 1) * 128, c0:c0 + cc])
            if scale_ap is not None:
                nc.vector.tensor_scalar_mul(wt[:, c0:c0 + cc], ist[:],
                                            scale_ap[:, k:k + 1])
            else:
                nc.vector.tensor_copy(wt[:, c0:c0 + cc], ist[:])
        tiles.append(wt)
    return tiles


def _build():
    nc = bacc.Bacc('TRN2', target_bir_lowering=False, debug=False,
                   num_devices=(1 if TLSIM else NC))
    I = _declare_inputs(nc)
    logits_out = nc.dram_tensor('logits', [4, VS], f32, kind='ExternalOutput').ap()

    with tile.TileContext(nc) as tc, ExitStack() as top:
        const_p = top.enter_context(tc.tile_pool(name='constp', bufs=1))
        resid_p = top.enter_context(tc.tile_pool(name='residp', bufs=1))
        dram_p = top.enter_context(tc.tile_pool(name='dramp', bufs=1, space='DRAM'))

        C = {}
        for cn, shape, dt in [('cosq', [128, T], bf16), ('sinq', [128, T], bf16),
                              ('cosq4', [128, 4], f32), ('sinq4', [128, 4], f32),
                              ('trimask', [128, 128], f32), ('identbf', [128, 128], bf16),
                              ('ones_k', [128, 1], f32), ('ones_m', [1, 128], f32),
                              ('onebf', [1, 1], bf16), ('gf', [128, KC], f32)]:
            t = const_p.tile(shape, dt, name=f'c_{cn}')
            nc.sync.dma_start(out=t[:], in_=I[cn][:])
            C[cn] = t
        for l in range(L):
            for cn in ('sqkv', 'so', 'sg', 'sud', 'sd', 'ga', 'gm'):
                shp = [128, {'sqkv': 3, 'so': KC, 'sg': MFF, 'sud': MFF,
                             'sd': KC, 'ga': KC, 'gm': KC}[cn]]
                t = const_p.tile(shp, f32, name=f'c_{cn}{l}')
                nc.sync.dma_start(out=t[:], in_=I[f'{cn}{l}'][:])
                C[f'{cn}{l}'] = t
        epst = const_p.tile([128, 1], f32, name='c_eps')
        nc.vector.memset(epst[:], EPS)
        ctx = {'const': C, 'eps': epst}

        # persistent residual shard [D, TB] fp32
        xs = []
        for k in range(KC):
            t = resid_p.tile([128, TB], f32, name=f'xsh{k}')
            nc.sync.dma_start(out=t[:], in_=I['x0'][k * 128:(k + 1) * 128, :])
            xs.append(t)

        # DRAM bounce buffers for collectives
        rs_in = dram_p.tile([NC * D, TB], bf16, name='rs_in')
        rs_out = dram_p.tile([D, TB], bf16, name='rs_out')
        RG = [list(range(NC))]

        def allgather_norm(src_tiles, gamma_ap, tag):
            """norm src -> bf16 -> ag_in -> AllGather; returns ag_out tile."""
            ag_in = dram_p.tile([D, TB], bf16, name=f'ag_in_{tag}')
            ag_out = dram_p.tile([NC * D, TB], bf16, name=f'ag_out_{tag}',
                                 addr_space=('Local' if TLSIM else 'Shared'))
            with tc.tile_pool(name=f'agp_{tag}', bufs=2) as sp:
                outs = [sp.tile([128, TB], bf16, name=f'xn_{tag}', bufs=3)
                        for _ in range(KC)]
                _fm_norm(nc, tc, ctx, src_tiles, gamma_ap, TB, outs, tag)
                for k in range(KC):
                    nc.sync.dma_start(out=ag_in[k * 128:(k + 1) * 128, :], in_=outs[k][:])
            if TLSIM:
                for r in range(NC):
                    nc.sync.dma_start(out=ag_out[r * D:(r + 1) * D, :], in_=ag_in[:])
            else:
                nc.gpsimd.collective_compute(
                    'AllGather', mybir.AluOpType.bypass, replica_groups=RG,
                    ins=[ag_in.opt()], outs=[ag_out.opt()])
            return ag_out

        def reduce_scatter_add(tag):
            """ReduceScatter rs_in -> rs_out; add into xs."""
            if TLSIM:
                nc.sync.dma_start(out=rs_out[:], in_=rs_in[0:D, :])
            else:
                nc.gpsimd.collective_compute(
                    'ReduceScatter', mybir.AluOpType.add, replica_groups=RG,
                    ins=[rs_in.opt()], outs=[rs_out.opt()])
            with tc.tile_pool(name=f'rsp_{tag}', bufs=3) as sp:
                for k in range(KC):
                    rt = sp.tile([128, TB], bf16, name=f'rs_{tag}')
                    nc.sync.dma_start(out=rt[:], in_=rs_out[k * 128:(k + 1) * 128, :])
                    nc.vector.tensor_add(xs[k][:], xs[k][:], rt[:])

        def load_xn_tiles(sp, xn_buf, tb, tag, bufs=3):
            tiles = []
            for k in range(KC):
                t = sp.tile([128, 512], bf16, name=f'xnl_{tag}', bufs=bufs)
                nc.sync.dma_start(
                    out=t[:], in_=xn_buf[tb * D + k * 128: tb * D + (k + 1) * 128, :])
                tiles.append(t)
            return tiles

        # ------------------------------------------------------------------
        # per-layer phases
        # ------------------------------------------------------------------

        def qkv_attention(l, xn_buf, last_layer):
            """Full attention block for layer l. For last_layer, q/attention are
            computed only for the 4 last-position tokens (returns nothing; writes
            o-proj partials to rs_in, or ar4 path for last layer)."""
            sqkv = C[f'sqkv{l}']
            with ExitStack() as ph:
                wsp = ph.enter_context(tc.tile_pool(name=f'wq_{l}', bufs=1))
                stg = ph.enter_context(tc.tile_pool(name=f'stq_{l}', bufs=1))
                wq_t = _dequant_weights(nc, tc, wsp, stg, I[f'wqkv{l}'], 384, KC, f'wqkv{l}')

                atp = ph.enter_context(tc.tile_pool(name=f'at_{l}', bufs=1))
                qsb = None
                if not last_layer:
                    qsb = [atp.tile([128, T], bf16, name=f'qsb{l}_{m}') for m in range(2)]
                ksb = atp.tile([128, T], bf16, name=f'ksb{l}')
                vt = {}
                for b in range(B):
                    for kb in range(8):
                        vt[(b, kb)] = atp.tile([128, 64], bf16, name=f'vt{l}_{b}_{kb}')

                with tc.tile_pool(name=f'qk_{l}', bufs=2) as sp, \
                     tc.tile_pool(name=f'qkp_{l}', bufs=2, space='PSUM') as pp:
                    for tb in range(NC):
                        xn = load_xn_tiles(sp, xn_buf, tb, f'q{l}', bufs=KC + 2)
                        col = tb * 512
                        mlist = [2] if last_layer else [0, 1, 2]
                        for m in mlist:
                            ps = pp.tile([128, 512], f32, name=f'qkvps{l}', bufs=3)
                            for k in range(KC):
                                nc.tensor.matmul(
                                    ps[:], wq_t[k][:, m * 128:(m + 1) * 128], xn[k][:],
                                    start=(k == 0), stop=(k == KC - 1))
                            if m < 2:
                                qf = sp.tile([128, 512], f32, name=f'qf{l}')
                                nc.vector.tensor_scalar_mul(qf[:], ps[:], sqkv[:, m:m + 1])
                                qs = sp.tile([128, 512], f32, name=f'qs{l}')
                                for g, src_g in ((0, 1), (1, 0), (2, 3), (3, 2)):
                                    eng = nc.scalar if g % 2 == 0 else nc.vector
                                    (eng.copy if g % 2 == 0 else eng.tensor_copy)(
                                        qs[g * 32:(g + 1) * 32, :],
                                        qf[src_g * 32:(src_g + 1) * 32, :])
                                m1 = sp.tile([128, 512], f32, name=f'm1{l}')
                                nc.vector.tensor_mul(m1[:], qf[:], C['cosq'][:, col:col + 512])
                                m2 = sp.tile([128, 512], f32, name=f'm2{l}')
                                nc.vector.tensor_mul(m2[:], qs[:], C['sinq'][:, col:col + 512])
                                nc.vector.tensor_add(qsb[m][:, col:col + 512], m1[:], m2[:])
                            else:
                                kf = sp.tile([64, 512], f32, name=f'kf{l}')
                                nc.vector.tensor_scalar_mul(kf[:], ps[0:64, :], sqkv[0:64, 2:3])
                                ks = sp.tile([64, 512], f32, name=f'ks{l}')
                                nc.scalar.copy(ks[0:32, :], kf[32:64, :])
                                nc.scalar.copy(ks[32:64, :], kf[0:32, :])
                                m1k = sp.tile([64, 512], f32, name=f'm1k{l}')
                                nc.vector.tensor_mul(m1k[:], kf[:], C['cosq'][0:64, col:col + 512])
                                m2k = sp.tile([64, 512], f32, name=f'm2k{l}')
                                nc.vector.tensor_mul(m2k[:], ks[:], C['sinq'][0:64, col:col + 512])
                                nc.vector.tensor_add(ksb[0:64, col:col + 512], m1k[:], m2k[:])
                                nc.vector.tensor_add(ksb[64:128, col:col + 512], m1k[:], m2k[:])
                                vf = sp.tile([64, 512], bf16, name=f'vf{l}')
                                nc.vector.tensor_scalar_mul(vf[:], ps[64:128, :], sqkv[64:128, 2:3])
                                b = tb // 2
                                for j in range(4):
                                    kb = (tb % 2) * 4 + j
                                    vps = pp.tile([128, 64], f32, name=f'vtp{l}', bufs=2)
                                    nc.tensor.matmul(vps[:], vf[:, j * 128:(j + 1) * 128],
                                                     C['identbf'][0:64, 0:64],
                                                     start=True, stop=True)
                                    nc.scalar.copy(vt[(b, kb)][:], vps[:])

                if last_layer:
                    return ksb, vt, wq_t, ph.pop_all()

                # ---- attention core (layers 0..2) ----
                attnf = [atp.tile([128, T], bf16, name=f'af{l}_{m}') for m in range(2)]
                with tc.tile_pool(name=f'sc_{l}', bufs=2) as sp, \
                     tc.tile_pool(name=f'scp_{l}', bufs=2, space='PSUM') as pp:
                    for b in range(B):
                        for h in range(QH):
                            qrows = ((h % 2) * 64, (h % 2) * 64 + 64)
                            qt_tile = qsb[h // 2]
                            psb = []
                            dgs = []
                            for qt in range(8):
                                W = (qt + 1) * 128
                                sps = pp.tile([128, 1024], f32, name=f'sps{l}', bufs=2)
                                for c0 in range(0, W, 512):
                                    cw = min(512, W - c0)
                                    nc.tensor.matmul(
                                        sps[:, c0:c0 + cw],
                                        qt_tile[qrows[0]:qrows[1],
                                                b * 1024 + qt * 128: b * 1024 + qt * 128 + 128],
                                        ksb[qrows[0]:qrows[1],
                                            b * 1024 + c0: b * 1024 + c0 + cw],
                                        start=True, stop=True)
                                nc.vector.tensor_add(sps[:, qt * 128:W],
                                                     sps[:, qt * 128:W], C['trimask'][:])
                                nm = sp.tile([128, 1], f32, name=f'nm{l}', bufs=3)
                                nc.vector.tensor_reduce(out=nm[:], in_=sps[:, 0:W],
                                                        axis=AX, op=mybir.AluOpType.max,
                                                        negate=True)
                                pt = sp.tile([128, 1024], bf16, name=f'pexp{l}_{qt}')
                                den = sp.tile([128, 1], f32, name=f'den{l}', bufs=3)
                                nc.scalar.activation(pt[:, 0:W], sps[:, 0:W], AF.Exp,
                                                     bias=nm[:], scale=1.0,
                                                     accum_out=den[:])
                                rden = sp.tile([128, 1], f32, name=f'rden{l}', bufs=3)
                                nc.vector.reciprocal(rden[:], den[:])
                                dg = sp.tile([128, 128], bf16, name=f'dg{l}_{qt}')
                                nc.vector.tensor_scalar_mul(dg[:], C['identbf'][:], rden[:])
                                psb.append(pt)
                                dgs.append(dg)
                            for Hh in range(2):
                                pv = pp.tile([64, 512], f32, name=f'pvps{l}', bufs=2)
                                for kb in range(4 * Hh + 4):
                                    qt0 = max(kb, 4 * Hh)
                                    ptp = pp.tile([128, 512], f32, name=f'ptp{l}', bufs=2)
                                    for qt in range(qt0, 4 * Hh + 4):
                                        nc.tensor.matmul(
                                            ptp[:, (qt - 4 * Hh) * 128:(qt - 4 * Hh + 1) * 128],
                                            psb[qt][:, kb * 128:(kb + 1) * 128],
                                            dgs[qt][:], start=True, stop=True)
                                    cs = (qt0 - 4 * Hh) * 128
                                    pts = sp.tile([128, 512], bf16, name=f'pts{l}', bufs=3)
                                    eng = nc.vector if kb % 2 == 0 else nc.scalar
                                    (eng.tensor_copy if kb % 2 == 0 else eng.copy)(
                                        pts[:, cs:512], ptp[:, cs:512])
                                    nc.tensor.matmul(pv[:, cs:512], vt[(b, kb)][:],
                                                     pts[:, cs:512],
                                                     start=(kb == 0), stop=(kb == 4 * Hh + 3))
                                nc.scalar.copy(
                                    attnf[h // 2][(h % 2) * 64:(h % 2) * 64 + 64,
                                                  b * 1024 + Hh * 512: b * 1024 + Hh * 512 + 512],
                                    pv[:])

                # ---- o-proj ----
                so = C[f'so{l}']
                with tc.tile_pool(name=f'wo_{l}', bufs=1) as wsp2, \
                     tc.tile_pool(name=f'sto_{l}', bufs=1) as stg2, \
                     tc.tile_pool(name=f'op_{l}', bufs=2) as sp, \
                     tc.tile_pool(name=f'opp_{l}', bufs=3, space='PSUM') as pp:
                    wo_t = _dequant_weights(nc, tc, wsp2, stg2, I[f'wo{l}'], D, 2, f'wo{l}')
                    for tb in range(NC):
                        for m in range(KC):
                            ops = pp.tile([128, 512], f32, name=f'ops{l}', bufs=3)
                            for kc in range(2):
                                nc.tensor.matmul(
                                    ops[:], wo_t[kc][:, m * 128:(m + 1) * 128],
                                    attnf[kc][:, tb * 512:(tb + 1) * 512],
                                    start=(kc == 0), stop=(kc == 1))
                            ob = sp.tile([128, 512], bf16, name=f'ob{l}', bufs=3)
                            if m % 2 == 0:
                                nc.vector.tensor_scalar_mul(ob[:], ops[:], so[:, m:m + 1])
                            else:
                                nc.scalar.activation(ob[:], ops[:], AF.Copy,
                                                     scale=so[:, m:m + 1])
                            nc.sync.dma_start(
                                out=rs_in[tb * D + m * 128: tb * D + (m + 1) * 128, :],
                                in_=ob[:])
            return None

        def mlp(l, xn_buf):
            """MLP block for layers 0..2 (full T tokens)."""
            sg, sud, sd = C[f'sg{l}'], C[f'sud{l}'], C[f'sd{l}']
            with ExitStack() as ph:
                wsp = ph.enter_context(tc.tile_pool(name=f'wm_{l}', bufs=1))
                stg = ph.enter_context(tc.tile_pool(name=f'stm_{l}', bufs=1))
                wg_t = _dequant_weights(nc, tc, wsp, stg, I[f'wg{l}'], DFFS, KC, f'wg{l}')
                wu_t = _dequant_weights(nc, tc, wsp, stg, I[f'wu{l}'], DFFS, KC, f'wu{l}')
                wd_t = _dequant_weights(nc, tc, wsp, stg, I[f'wd{l}'], D, MFF, f'wd{l}',
                                        scale_ap=sud)
                with tc.tile_pool(name=f'ml_{l}', bufs=2) as sp, \
                     tc.tile_pool(name=f'mlp_{l}', bufs=2, space='PSUM') as pp:
                    for tb in range(NC):
                        xn = load_xn_tiles(sp, xn_buf, tb, f'm{l}', bufs=KC + 2)
                        hm = []
                        for mf in range(MFF):
                            gps = pp.tile([128, 512], f32, name=f'gps{l}', bufs=2)
                            for k in range(KC):
                                nc.tensor.matmul(gps[:], wg_t[k][:, mf * 128:(mf + 1) * 128],
                                                 xn[k][:], start=(k == 0), stop=(k == KC - 1))
                            ups = pp.tile([128, 512], f32, name=f'ups{l}', bufs=2)
                            for k in range(KC):
                                nc.tensor.matmul(ups[:], wu_t[k][:, mf * 128:(mf + 1) * 128],
                                                 xn[k][:], start=(k == 0), stop=(k == KC - 1))
                            gsb = sp.tile([128, 512], bf16, name=f'gsb{l}', bufs=2)
                            nc.scalar.activation(gsb[:], gps[:], AF.Silu,
                                                 scale=sg[:, mf:mf + 1])
                            ht = sp.tile([128, 512], bf16, name=f'hm{l}_{mf}', bufs=1)
                            nc.vector.tensor_mul(ht[:], gsb[:], ups[:])
                            hm.append(ht)
                        for mo in range(KC):
                            dps = pp.tile([128, 512], f32, name=f'dps{l}', bufs=3)
                            for k in range(MFF):
                                nc.tensor.matmul(dps[:], wd_t[k][:, mo * 128:(mo + 1) * 128],
                                                 hm[k][:], start=(k == 0), stop=(k == MFF - 1))
                            db = sp.tile([128, 512], bf16, name=f'db{l}', bufs=2)
                            if mo % 2 == 0:
                                nc.vector.tensor_scalar_mul(db[:], dps[:], sd[:, mo:mo + 1])
                            else:
                                nc.scalar.activation(db[:], dps[:], AF.Copy,
                                                     scale=sd[:, mo:mo + 1])
                            nc.sync.dma_start(
                                out=rs_in[tb * D + mo * 128: tb * D + (mo + 1) * 128, :],
                                in_=db[:])

        # ------------------------------------------------------------------
        # layers 0..2
        # ------------------------------------------------------------------
        for l in range(L - 1):
            agb = allgather_norm(xs, C[f'ga{l}'][:], f'a{l}')
            qkv_attention(l, agb, last_layer=False)
            reduce_scatter_add(f'o{l}')
            agb = allgather_norm(xs, C[f'gm{l}'][:], f'm{l}')
            mlp(l, agb)
            reduce_scatter_add(f'd{l}')

        # ------------------------------------------------------------------
        # layer 3 (last): only last-position tokens through q/attn/mlp/head
        # ------------------------------------------------------------------
        l = L - 1
        lx_in = dram_p.tile([D, 1], f32, name='lx_in')
        lx_out = dram_p.tile([NC * D, 1], f32, name='lx_out',
                             addr_space=('Local' if TLSIM else 'Shared'))
        ar_in = dram_p.tile([D, 4], f32, name='ar_in')
        ar_out = dram_p.tile([D, 4], f32, name='ar_out',
                             addr_space=('Local' if TLSIM else 'Shared'))
        ar2_in = dram_p.tile([D, 4], f32, name='ar2_in')
        ar2_out = dram_p.tile([D, 4], f32, name='ar2_out',
                             addr_space=('Local' if TLSIM else 'Shared'))

        for k in range(KC):
            nc.sync.dma_start(out=lx_in[k * 128:(k + 1) * 128, :], in_=xs[k][:, 511:512])
        if TLSIM:
            for r in range(NC):
                nc.sync.dma_start(out=lx_out[r * D:(r + 1) * D, :], in_=lx_in[:])
        else:
            nc.gpsimd.collective_compute('AllGather', mybir.AluOpType.bypass,
                                         replica_groups=RG, ins=[lx_in.opt()],
                                         outs=[lx_out.opt()])
        l4p = top.enter_context(tc.tile_pool(name='l4p', bufs=1))
        lastx = []
        for k in range(KC):
            t = l4p.tile([128, 4], f32, name=f'lastx{k}')
            src = bass.AP(tensor=lx_out.tensor, offset=lx_out[:].offset + D + k * 128,
                          ap=[[1, 128], [2 * D, 4]])
            nc.sync.dma_start(out=t[:], in_=src)
            lastx.append(t)

        # norm for q (on last-position tokens)
        qn4 = [l4p.tile([128, 4], bf16, name=f'qn4_{k}') for k in range(KC)]
        _fm_norm(nc, tc, ctx, lastx, C[f'ga{l}'][:], 4, qn4, 'q4')

        # full norm + AG for k/v
        agb = allgather_norm(xs, C[f'ga{l}'][:], f'a{l}')
        ksb, vt, wq_t, wctx = qkv_attention(l, agb, last_layer=True)

        sqkv = C[f'sqkv{l}']
        q4h = [l4p.tile([64, 4], bf16, name=f'q4h_{h}') for h in range(QH)]
        at4 = [l4p.tile([64, 4], bf16, name=f'at4_{h}') for h in range(QH)]
        with tc.tile_pool(name='l4qs', bufs=2) as sp, \
             tc.tile_pool(name='l4qp', bufs=2, space='PSUM') as pp:
            for m in range(2):
                ps = pp.tile([128, 4], f32, name='q4ps', bufs=2)
                for k in range(KC):
                    nc.tensor.matmul(ps[:], wq_t[k][:, m * 128:(m + 1) * 128], qn4[k][:],
                                     start=(k == 0), stop=(k == KC - 1))
                qf = sp.tile([128, 4], f32, name='q4f')
                nc.vector.tensor_scalar_mul(qf[:], ps[:], sqkv[:, m:m + 1])
                qs = sp.tile([128, 4], f32, name='q4s')
                for g, src_g in ((0, 1), (1, 0), (2, 3), (3, 2)):
                    nc.vector.tensor_copy(qs[g * 32:(g + 1) * 32, :],
                                          qf[src_g * 32:(src_g + 1) * 32, :])
                m1 = sp.tile([128, 4], f32, name='q4m1')
                nc.vector.tensor_mul(m1[:], qf[:], C['cosq4'][:])
                m2 = sp.tile([128, 4], f32, name='q4m2')
                nc.vector.tensor_mul(m2[:], qs[:], C['sinq4'][:])
                for sub in range(2):
                    nc.vector.tensor_add(q4h[m * 2 + sub][:],
                                         m1[sub * 64:(sub + 1) * 64, :],
                                         m2[sub * 64:(sub + 1) * 64, :])

        # attention for 4 last tokens
        with tc.tile_pool(name='l4as', bufs=2) as sp, \
             tc.tile_pool(name='l4ap', bufs=1, space='PSUM') as pp:
            for b in range(B):
                for h in range(QH):
                    s4 = pp.tile([1, 1024], f32, name='s4ps', bufs=2)
                    for c0 in range(0, 1024, 512):
                        nc.tensor.matmul(s4[:, c0:c0 + 512],
                                         q4h[h][:, b:b + 1],
                                         ksb[0:64, b * 1024 + c0: b * 1024 + c0 + 512],
                                         start=True, stop=True)
                    nm = sp.tile([1, 1], f32, name='nm4', bufs=3)
                    nc.vector.tensor_reduce(out=nm[:], in_=s4[:], axis=AX,
                                            op=mybir.AluOpType.max, negate=True)
                    p4 = sp.tile([1, 1024], bf16, name='p4', bufs=2)
                    den = sp.tile([1, 1], f32, name='den4', bufs=3)
                    nc.scalar.activation(p4[:], s4[:], AF.Exp, bias=nm[:], scale=1.0,
                                         accum_out=den[:])
                    rden = sp.tile([1, 1], f32, name='rden4', bufs=3)
                    nc.vector.reciprocal(rden[:], den[:])
                    rbcp = pp.tile([128, 1], f32, name='rbcp', bufs=1)
                    nc.tensor.matmul(rbcp[:], C['ones_m'][:], rden[:], start=True, stop=True)
                    rbc = sp.tile([128, 1], f32, name='rbc4', bufs=3)
                    nc.scalar.copy(rbc[:], rbcp[:])
                    pt4p = pp.tile([128, 8], f32, name='pt4p', bufs=1)
                    for kb in range(8):
                        nc.tensor.matmul(pt4p[:, kb:kb + 1], p4[:, kb * 128:(kb + 1) * 128],
                                         C['onebf'][:], start=True, stop=True)
                    pt4 = sp.tile([128, 8], bf16, name='pt4', bufs=2)
                    nc.vector.tensor_scalar_mul(pt4[:], pt4p[:], rbc[:])
                    pv4 = pp.tile([64, 1], f32, name='pv4', bufs=2)
                    for kb in range(8):
                        nc.tensor.matmul(pv4[:], vt[(b, kb)][:], pt4[:, kb:kb + 1],
                                         start=(kb == 0), stop=(kb == 7))
                    nc.scalar.copy(at4[h][:, b:b + 1], pv4[:])
        wctx.close()

        # o-proj for 4 tokens
        so = C[f'so{l}']
        with tc.tile_pool(name='wo3', bufs=1) as wsp2, \
             tc.tile_pool(name='sto3', bufs=1) as stg2, \
             tc.tile_pool(name='l4os', bufs=2) as sp, \
             tc.tile_pool(name='l4op', bufs=2, space='PSUM') as pp:
            wo4 = []
            for h in range(QH):
                ist = stg2.tile([64, D], i32, name='ist_wo4', bufs=2)
                nc.sync.dma_start(out=ist[:], in_=I[f'wo{l}'][h * 64:(h + 1) * 64, :])
                wt = wsp2.tile([64, D], bf16, name=f'wo4_{h}')
                nc.vector.tensor_copy(wt[:], ist[:])
                wo4.append(wt)
            for m in range(KC):
                ops = pp.tile([128, 4], f32, name='o4ps', bufs=2)
                for h in range(QH):
                    nc.tensor.matmul(ops[:], wo4[h][:, m * 128:(m + 1) * 128],
                                     at4[h][:], start=(h == 0), stop=(h == QH - 1))
                ob = sp.tile([128, 4], f32, name='o4b', bufs=3)
                nc.vector.tensor_scalar_mul(ob[:], ops[:], so[:, m:m + 1])
                nc.sync.dma_start(out=ar_in[m * 128:(m + 1) * 128, :], in_=ob[:])

        if TLSIM:
            nc.sync.dma_start(out=ar_out[:], in_=ar_in[:])
        else:
            nc.gpsimd.collective_compute('AllReduce', mybir.AluOpType.add,
                                         replica_groups=RG, ins=[ar_in.opt()],
                                         outs=[ar_out.opt()])

        # residual add (4 tokens)
        x4 = []
        with tc.tile_pool(name='l4r', bufs=3) as sp:
            for k in range(KC):
                rt = sp.tile([128, 4], f32, name='ar4l')
                nc.sync.dma_start(out=rt[:], in_=ar_out[k * 128:(k + 1) * 128, :])
                t = l4p.tile([128, 4], f32, name=f'x4_{k}')
                nc.vector.tensor_add(t[:], lastx[k][:], rt[:])
                x4.append(t)

        # norm2 + tiny MLP
        xn4 = [l4p.tile([128, 4], bf16, name=f'xn4_{k}') for k in range(KC)]
        _fm_norm(nc, tc, ctx, x4, C[f'gm{l}'][:], 4, xn4, 'm4')
        sg, sud, sd = C[f'sg{l}'], C[f'sud{l}'], C[f'sd{l}']
        with ExitStack() as ph:
            wsp = ph.enter_context(tc.tile_pool(name='wm3', bufs=1))
            stg = ph.enter_context(tc.tile_pool(name='stm3', bufs=1))
            wg_t = _dequant_weights(nc, tc, wsp, stg, I[f'wg{l}'], DFFS, KC, f'wg{l}')
            wu_t = _dequant_weights(nc, tc, wsp, stg, I[f'wu{l}'], DFFS, KC, f'wu{l}')
            wd_t = _dequant_weights(nc, tc, wsp, stg, I[f'wd{l}'], D, MFF, f'wd{l}',
                                    scale_ap=sud)
            with tc.tile_pool(name='m4s', bufs=2) as sp, \
                 tc.tile_pool(name='m4p', bufs=2, space='PSUM') as pp:
                hm = []
                for mf in range(MFF):
                    gps = pp.tile([128, 4], f32, name='g4ps', bufs=2)
                    for k in range(KC):
                        nc.tensor.matmul(gps[:], wg_t[k][:, mf * 128:(mf + 1) * 128],
                                         xn4[k][:], start=(k == 0), stop=(k == KC - 1))
                    ups = pp.tile([128, 4], f32, name='u4ps', bufs=2)
                    for k in range(KC):
                        nc.tensor.matmul(ups[:], wu_t[k][:, mf * 128:(mf + 1) * 128],
                                         xn4[k][:], start=(k == 0), stop=(k == KC - 1))
                    gsb = sp.tile([128, 4], bf16, name='g4sb', bufs=3)
                    nc.scalar.activation(gsb[:], gps[:], AF.Silu, scale=sg[:, mf:mf + 1])
                    ht = sp.tile([128, 4], bf16, name=f'h4_{mf}')
                    nc.vector.tensor_mul(ht[:], gsb[:], ups[:])
                    hm.append(ht)
                for mo in range(KC):
                    dps = pp.tile([128, 4], f32, name='d4ps', bufs=2)
                    for k in range(MFF):
                        nc.tensor.matmul(dps[:], wd_t[k][:, mo * 128:(mo + 1) * 128],
                                         hm[k][:], start=(k == 0), stop=(k == MFF - 1))
                    db = sp.tile([128, 4], f32, name='d4b', bufs=3)
                    nc.vector.tensor_scalar_mul(db[:], dps[:], sd[:, mo:mo + 1])
                    nc.sync.dma_start(out=ar2_in[mo * 128:(mo + 1) * 128, :], in_=db[:])

        if TLSIM:
            nc.sync.dma_start(out=ar2_out[:], in_=ar2_in[:])
        else:
            nc.gpsimd.collective_compute('AllReduce', mybir.AluOpType.add,
                                         replica_groups=RG, ins=[ar2_in.opt()],
                                         outs=[ar2_out.opt()])

        # final residual + final norm + LM head
        with tc.tile_pool(name='fhs', bufs=2) as sp, \
             tc.tile_pool(name='fhp', bufs=2, space='PSUM') as pp:
            xf = []
            for k in range(KC):
                rt = sp.tile([128, 4], f32, name='ar4l2', bufs=3)
                nc.sync.dma_start(out=rt[:], in_=ar2_out[k * 128:(k + 1) * 128, :])
                t = l4p.tile([128, 4], f32, name=f'xf_{k}')
                nc.vector.tensor_add(t[:], x4[k][:], rt[:])
                xf.append(t)
            xfn = [l4p.tile([128, 4], f32, name=f'xfn_{k}') for k in range(KC)]
            _fm_norm(nc, tc, ctx, xf, C['gf'][:], 4, xfn, 'f4')
            nch = (VS + 511) // 512
            for n in range(nch):
                cw = min(512, VS - n * 512)
                hps = pp.tile([4, 512], f32, name='hps', bufs=2)
                for k in range(KC):
                    et = sp.tile([128, 512], f32, name='embt', bufs=4)
                    nc.sync.dma_start(
                        out=et[:, 0:cw],
                        in_=I['embT'][k * 128:(k + 1) * 128, n * 512:n * 512 + cw])
                    nc.tensor.matmul(hps[:, 0:cw], xfn[k][:], et[:, 0:cw],
                                     start=(k == 0), stop=(k == KC - 1))
                lsb = sp.tile([4, 512], f32, name='lsb', bufs=3)
                nc.scalar.copy(lsb[:, 0:cw], hps[:, 0:cw])
                nc.sync.dma_start(out=logits_out[:, n * 512:n * 512 + cw],
                                  in_=lsb[:, 0:cw])

    nc.compile()
    return nc


# ----------------------------------------------------------------------------
# host-side prep
# ----------------------------------------------------------------------------

def _prep_in_maps(token_ids, embed, gamma_attn, gamma_mlp, gamma_final,
                  wq, sq, wk, sk, wv, sv, wo, so, wg, sg, wu, su, wd, sd):
    half = HD // 2
    inv = ROPE_THETA ** (-np.arange(half, dtype=np.float32) * 2.0 / HD)
    ang = np.arange(S, dtype=np.float32)[:, None] * inv          # [S, 32]
    cos1 = np.cos(ang).T.astype(np.float32)                      # [32, S]
    sin1 = np.sin(ang).T.astype(np.float32)
    cos64 = np.concatenate([cos1, cos1], 0)                      # [64, S]
    sin64s = np.concatenate([-sin1, sin1], 0)
    cosq = np.tile(np.concatenate([cos64, cos64], 0), (1, B))    # [128, T]
    sinq = np.tile(np.concatenate([sin64s, sin64s], 0), (1, B))
    cosq4 = np.repeat(cosq[:, S - 1:S], 4, axis=1).copy()
    sinq4 = np.repeat(sinq[:, S - 1:S], 4, axis=1).copy()

    ii, jj = np.meshgrid(np.arange(128), np.arange(128), indexing='ij')
    trimask = np.where(jj <= ii, 0.0, NEG).astype(np.float32)

    tok = np.asarray(token_ids).reshape(T)
    x0full = np.ascontiguousarray(embed[tok].T.astype(np.float32))  # [D, T]
    embT = np.ascontiguousarray(embed.T.astype(np.float32))          # [D, V]

    def percol(a):
        return np.ascontiguousarray(a.reshape(-1, 128).T.astype(np.float32))

    common = {
        'cosq': np.ascontiguousarray(cosq.astype(ml_dtypes.bfloat16)),
        'sinq': np.ascontiguousarray(sinq.astype(ml_dtypes.bfloat16)),
        'cosq4': cosq4, 'sinq4': sinq4, 'trimask': trimask,
        'identbf': np.eye(128, dtype=ml_dtypes.bfloat16),
        'ones_k': np.ones((128, 1), np.float32),
        'ones_m': np.ones((1, 128), np.float32),
        'onebf': np.ones((1, 1), ml_dtypes.bfloat16),
        'gf': percol(gamma_final),
    }
    in_maps = []
    for c in range(NC):
        m = dict(common)
        m['x0'] = np.ascontiguousarray(x0full[:, c * TB:(c + 1) * TB])
        m['embT'] = np.ascontiguousarray(embT[:, c * VS:(c + 1) * VS])
        for l in range(L):
            qsl = slice(c * DQ, (c + 1) * DQ)
            ksl = slice(c * HD, (c + 1) * HD)
            fsl = slice(c * DFFS, (c + 1) * DFFS)
            m[f'wqkv{l}'] = np.ascontiguousarray(np.concatenate(
                [wq[l][qsl].T, wk[l][ksl].T, wv[l][ksl].T], axis=1).astype(np.int32))
            sq_l = sq[l][qsl] * np.float32(1.0 / np.sqrt(HD))
            m[f'sqkv{l}'] = np.ascontiguousarray(np.stack(
                [sq_l[0:128], sq_l[128:256],
                 np.concatenate([sk[l][ksl], sv[l][ksl]])], axis=1).astype(np.float32))
            m[f'wo{l}'] = np.ascontiguousarray(wo[l][:, qsl].T.astype(np.int32))
            m[f'so{l}'] = percol(so[l])
            m[f'wg{l}'] = np.ascontiguousarray(wg[l][fsl].T.astype(np.int32))
            m[f'sg{l}'] = percol(sg[l][fsl])
            m[f'wu{l}'] = np.ascontiguousarray(wu[l][fsl].T.astype(np.int32))
            m[f'wd{l}'] = np.ascontiguousarray(wd[l][:, fsl].T.astype(np.int32))
            m[f'sud{l}'] = percol(su[l][fsl])
            m[f'sd{l}'] = percol(sd[l])
            m[f'ga{l}'] = percol(gamma_attn[l])
            m[f'gm{l}'] = percol(gamma_mlp[l])
        in_maps.append(m)
    return in_maps


def _get_nc():
    if 'nc' not in _CACHE:
        _CACHE['nc'] = _build()
    return _CACHE['nc']


def kernel(**inputs) -> np.ndarray:
    inputs = {k: np.asarray(v) for k, v in inputs.items()}
    in_maps = _prep_in_maps(**inputs)
    nc = _get_nc()
    res = bass_utils.run_bass_kernel_spmd(nc, in_maps, core_ids=list(range(NC)))
    logits = np.concatenate([res.results[c]['logits'] for c in range(NC)], axis=1)
    return logits.astype(np.float32)



# revision 4
# speedup vs baseline: 8.0614x; 8.0614x over previous
"""Self-contained Trainium2 Bass kernel for the int4-quantized 4-layer Llama decode problem.

Strategy: tensor-parallel over 8 NeuronCores (attention heads + FFN hidden dim),
sequence-parallel residual (each core keeps a feature-major fp32 residual shard
[D, T/8] in SBUF), AllGather before QKV/MLP, ReduceScatter after o-proj/down-proj.
Weights are host-packed to fp8e4 (int4 values are exact in e4m3) and used directly
as the stationary matmul operand against bf16 activations; dequant scales are
applied to the matmul outputs (per-out-channel) or to the activations (down-proj
input channels). Activation/residual traffic between SBUF and the DRAM collective
bounce buffers moves in single strided DMAs per 2048-feature block.
Only the last position of each sequence goes through layer-4 Q/attention/MLP and
the LM head.
"""
import sys

sys.path.insert(0, '/opt/trn_rl_repo')

import numpy as np
import ml_dtypes
from contextlib import ExitStack

import concourse.bass as bass
import concourse.tile as tile
from concourse import bacc, mybir
from concourse import bass_utils

# model dims (hardcoded per problem spec)
L, D, H, HD, KVH, DFF, V, B, S = 4, 2048, 32, 64, 8, 8192, 32000, 4, 1024
NC = 8
T = B * S              # 4096 tokens
TB = T // NC           # 512 tokens per core shard
QH = H // NC           # 4 local q heads
DQ = QH * HD           # 256 local q dims
DFFS = DFF // NC       # 1024 local ffn dims
VS = V // NC           # 4000 local vocab
KC = D // 128          # 16 feature chunks
MFF = DFFS // 128      # 8
ROPE_THETA = 500000.0
NEG = np.float32(-1e9)
EPS = 1e-5

f32 = mybir.dt.float32
bf16 = mybir.dt.bfloat16
fp8 = mybir.dt.float8e4
i32 = mybir.dt.int32

AX = mybir.AxisListType.X
MUL = mybir.AluOpType.mult
AF = mybir.ActivationFunctionType

_CACHE = {}
TLSIM = False  # single-core cost-model sim mode (collectives -> DMA copies)


# ----------------------------------------------------------------------------
# bass program
# ----------------------------------------------------------------------------

def _declare_inputs(nc):
    I = {}
    I['x0'] = nc.dram_tensor('x0', [D, TB], f32, kind='ExternalInput').ap()
    for l in range(L):
        I[f'wqkv{l}'] = nc.dram_tensor(f'wqkv{l}', [128, KC * 384], fp8, kind='ExternalInput').ap()
        I[f'sqkv{l}'] = nc.dram_tensor(f'sqkv{l}', [128, 3], f32, kind='ExternalInput').ap()
        I[f'wo{l}'] = nc.dram_tensor(f'wo{l}', [128, 2 * D], fp8, kind='ExternalInput').ap()
        I[f'so{l}'] = nc.dram_tensor(f'so{l}', [128, KC], f32, kind='ExternalInput').ap()
        I[f'wg{l}'] = nc.dram_tensor(f'wg{l}', [128, KC * DFFS], fp8, kind='ExternalInput').ap()
        I[f'sg{l}'] = nc.dram_tensor(f'sg{l}', [128, MFF], f32, kind='ExternalInput').ap()
        I[f'wu{l}'] = nc.dram_tensor(f'wu{l}', [128, KC * DFFS], fp8, kind='ExternalInput').ap()
        I[f'wd{l}'] = nc.dram_tensor(f'wd{l}', [128, MFF * D], fp8, kind='ExternalInput').ap()
        I[f'sud{l}'] = nc.dram_tensor(f'sud{l}', [128, MFF], f32, kind='ExternalInput').ap()
        I[f'sd{l}'] = nc.dram_tensor(f'sd{l}', [128, KC], f32, kind='ExternalInput').ap()
        I[f'ga{l}'] = nc.dram_tensor(f'ga{l}', [128, KC], f32, kind='ExternalInput').ap()
        I[f'gm{l}'] = nc.dram_tensor(f'gm{l}', [128, KC], f32, kind='ExternalInput').ap()
    I['gf'] = nc.dram_tensor('gf', [128, KC], f32, kind='ExternalInput').ap()
    I['cosq'] = nc.dram_tensor('cosq', [128, T], bf16, kind='ExternalInput').ap()
    I['sinq'] = nc.dram_tensor('sinq', [128, T], bf16, kind='ExternalInput').ap()
    I['cosq4'] = nc.dram_tensor('cosq4', [128, 4], f32, kind='ExternalInput').ap()
    I['sinq4'] = nc.dram_tensor('sinq4', [128, 4], f32, kind='ExternalInput').ap()
    I['trimask'] = nc.dram_tensor('trimask', [128, 128], f32, kind='ExternalInput').ap()
    I['identbf'] = nc.dram_tensor('identbf', [128, 128], bf16, kind='ExternalInput').ap()
    I['ones_k'] = nc.dram_tensor('ones_k', [128, 1], f32, kind='ExternalInput').ap()
    I['ones_m'] = nc.dram_tensor('ones_m', [1, 128], f32, kind='ExternalInput').ap()
    I['onebf'] = nc.dram_tensor('onebf', [1, 1], bf16, kind='ExternalInput').ap()
    I['embT'] = nc.dram_tensor('embT', [D, VS], bf16, kind='ExternalInput').ap()
    return I


def _fm_norm(nc, tc, ctx, src, gamma_ap, width, out_tiles, tag):
    """Feature-major rmsnorm: src = list of KC sbuf [128,width] f32 APs.
    Writes out_tiles (KC APs, caller-allocated, any dtype)."""
    with tc.tile_pool(name=f'np_{tag}', bufs=2) as sp, \
         tc.tile_pool(name=f'npp_{tag}', bufs=2, space='PSUM') as pp:
        C = ctx['const']
        ssum = pp.tile([1, width], f32, name=f'nsum_{tag}')
        for k in range(KC):
            xsq = sp.tile([128, width], f32, name=f'nxsq_{tag}', bufs=3)
            nc.vector.tensor_mul(xsq[:], src[k][:], src[k][:])
            nc.tensor.matmul(ssum[:], C['ones_k'][:], xsq[:],
                             start=(k == 0), stop=(k == KC - 1))
        sq = sp.tile([1, width], f32, name=f'nsq_{tag}')
        nc.scalar.activation(sq[:], ssum[:], AF.Sqrt, bias=ctx['eps'][0:1, :],
                             scale=1.0 / D)
        rstd = sp.tile([1, width], f32, name=f'nrstd_{tag}')
        nc.vector.reciprocal(rstd[:], sq[:])
        bcp = pp.tile([128, width], f32, name=f'nbc_{tag}')
        nc.tensor.matmul(bcp[:], C['ones_m'][:], rstd[:], start=True, stop=True)
        rbc = sp.tile([128, width], f32, name=f'nrbc_{tag}')
        nc.scalar.copy(rbc[:], bcp[:])
        for k in range(KC):
            nc.vector.scalar_tensor_tensor(
                out=out_tiles[k][:], in0=src[k][:], scalar=gamma_ap[:, k:k + 1],
                in1=rbc[:], op0=MUL, op1=MUL)


def _r3(dram_ap, nchunks=KC):
    """[(k p), c] DRAM slice -> [p, k, c] AP for batched strided DMA."""
    return dram_ap.rearrange("(k p) c -> p k c", k=nchunks)


def _s3(sb_ap, nchunks=KC):
    """[p, (k c)] SBUF tile -> [p, k, c] AP."""
    return sb_ap.rearrange("p (k c) -> p k c", k=nchunks)


def _build(reps=1):
    nc = bacc.Bacc('TRN2', target_bir_lowering=False, debug=False,
                   num_devices=(1 if TLSIM else NC))
    I = _declare_inputs(nc)
    logits_out = nc.dram_tensor('logits', [4, VS], f32, kind='ExternalOutput').ap()

    with tile.TileContext(nc) as tc, ExitStack() as top:
        const_p = top.enter_context(tc.tile_pool(name='constp', bufs=1))
        resid_p = top.enter_context(tc.tile_pool(name='residp', bufs=1))
        dram_p = top.enter_context(tc.tile_pool(name='dramp', bufs=1, space='DRAM'))

        C = {}
        for cn, shape, dt in [('cosq', [128, T], bf16), ('sinq', [128, T], bf16),
                              ('cosq4', [128, 4], f32), ('sinq4', [128, 4], f32),
                              ('trimask', [128, 128], f32), ('identbf', [128, 128], bf16),
                              ('ones_k', [128, 1], f32), ('ones_m', [1, 128], f32),
                              ('onebf', [1, 1], bf16), ('gf', [128, KC], f32)]:
            t = const_p.tile(shape, dt, name=f'c_{cn}')
            nc.sync.dma_start(out=t[:], in_=I[cn][:])
            C[cn] = t
        for l in range(L):
            for cn in ('sqkv', 'so', 'sg', 'sud', 'sd', 'ga', 'gm'):
                shp = [128, {'sqkv': 3, 'so': KC, 'sg': MFF, 'sud': MFF,
                             'sd': KC, 'ga': KC, 'gm': KC}[cn]]
                t = const_p.tile(shp, f32, name=f'c_{cn}{l}')
                nc.sync.dma_start(out=t[:], in_=I[f'{cn}{l}'][:])
                C[f'{cn}{l}'] = t
        epst = const_p.tile([128, 1], f32, name='c_eps')
        nc.vector.memset(epst[:], EPS)
        ctx = {'const': C, 'eps': epst}

        for _rep in range(reps):
            _body(nc, tc, top, I, C, ctx, dram_p, resid_p, logits_out)

    nc.compile()
    return nc


def _body(nc, tc, top, I, C, ctx, dram_p, resid_p, logits_out):
    # persistent residual shard [D, TB] fp32 as one [128, KC*TB] tile
    xsb = resid_p.tile([128, KC * TB], f32, name='xsh')
    nc.sync.dma_start(out=_s3(xsb[:]), in_=_r3(I['x0'][:]))
    xs = [xsb[:, k * TB:(k + 1) * TB] for k in range(KC)]

    # DRAM bounce buffers for collectives
    rs_in = dram_p.tile([NC * D, TB], bf16, name='rs_in')
    rs_out = dram_p.tile([D, TB], bf16, name='rs_out')
    RG = [list(range(NC))]

    def allgather_norm(src_tiles, gamma_ap, tag):
        """norm src -> bf16 -> ag_in -> AllGather; returns ag_out tile."""
        ag_in = dram_p.tile([D, TB], bf16, name=f'ag_in_{tag}')
        ag_out = dram_p.tile([NC * D, TB], bf16, name=f'ag_out_{tag}',
                             addr_space=('Local' if TLSIM else 'Shared'))
        with tc.tile_pool(name=f'agp_{tag}', bufs=2) as sp:
            xnall = sp.tile([128, KC * TB], bf16, name=f'xn_{tag}')
            outs = [xnall[:, k * TB:(k + 1) * TB] for k in range(KC)]
            _fm_norm(nc, tc, ctx, src_tiles, gamma_ap, TB, outs, tag)
            nc.sync.dma_start(out=_r3(ag_in[:]), in_=_s3(xnall[:]))
        if TLSIM:
            for r in range(NC):
                nc.sync.dma_start(out=ag_out[r * D:(r + 1) * D, :], in_=ag_in[:])
        else:
            nc.gpsimd.collective_compute(
                'AllGather', mybir.AluOpType.bypass, replica_groups=RG,
                ins=[ag_in.opt()], outs=[ag_out.opt()])
        return ag_out

    def reduce_scatter_add(tag):
        """ReduceScatter rs_in -> rs_out; add into xs."""
        if TLSIM:
            nc.sync.dma_start(out=rs_out[:], in_=rs_in[0:D, :])
        else:
            nc.gpsimd.collective_compute(
                'ReduceScatter', mybir.AluOpType.add, replica_groups=RG,
                ins=[rs_in.opt()], outs=[rs_out.opt()])
        with tc.tile_pool(name=f'rsp_{tag}', bufs=2) as sp:
            rt = sp.tile([128, KC * TB], bf16, name=f'rs_{tag}')
            nc.sync.dma_start(out=_s3(rt[:]), in_=_r3(rs_out[:]))
            for k in range(KC):
                nc.vector.tensor_add(xs[k][:], xs[k][:], rt[:, k * TB:(k + 1) * TB])

    def load_xn(sp, xn_buf, tb, tag, bufs=3):
        xnb = sp.tile([128, KC * 512], bf16, name=f'xnl_{tag}', bufs=bufs)
        nc.sync.dma_start(out=_s3(xnb[:]),
                          in_=_r3(xn_buf[tb * D:(tb + 1) * D, :]))
        return [xnb[:, k * 512:(k + 1) * 512] for k in range(KC)]

    # ------------------------------------------------------------------
    # per-layer phases
    # ------------------------------------------------------------------

    def qkv_attention(l, xn_buf, last_layer):
        """Full attention block for layer l. For last_layer, q/attention are
        computed only for the 4 last-position tokens."""
        sqkv = C[f'sqkv{l}']
        with ExitStack() as ph:
            wsp = ph.enter_context(tc.tile_pool(name=f'wq_{l}', bufs=1))
            wq8 = wsp.tile([128, KC * 384], fp8, name=f'wqkv{l}')
            nc.sync.dma_start(out=wq8[:], in_=I[f'wqkv{l}'][:])

            def wq_sl(k, m):
                return wq8[:, k * 384 + m * 128: k * 384 + (m + 1) * 128]

            atp = ph.enter_context(tc.tile_pool(name=f'at_{l}', bufs=1))
            qsb = None
            if not last_layer:
                qsb = [atp.tile([128, T], bf16, name=f'qsb{l}_{m}') for m in range(2)]
            ksb = atp.tile([128, T], bf16, name=f'ksb{l}')
            vt = {}
            for b in range(B):
                for kb in range(8):
                    vt[(b, kb)] = atp.tile([128, 64], bf16, name=f'vt{l}_{b}_{kb}')

            with tc.tile_pool(name=f'qk_{l}', bufs=2) as sp, \
                 tc.tile_pool(name=f'qkp_{l}', bufs=2, space='PSUM') as pp:
                for tb in range(NC):
                    xn = load_xn(sp, xn_buf, tb, f'q{l}', bufs=3)
                    col = tb * 512
                    mlist = [2] if last_layer else [0, 1, 2]
                    for m in mlist:
                        ps = pp.tile([128, 512], f32, name=f'qkvps{l}', bufs=3)
                        for k in range(KC):
                            nc.tensor.matmul(
                                ps[:], wq_sl(k, m), xn[k][:],
                                start=(k == 0), stop=(k == KC - 1))
                        if m < 2:
                            qf = sp.tile([128, 512], f32, name=f'qf{l}')
                            nc.vector.tensor_scalar_mul(qf[:], ps[:], sqkv[:, m:m + 1])
                            qs = sp.tile([128, 512], f32, name=f'qs{l}')
                            for g, src_g in ((0, 1), (1, 0), (2, 3), (3, 2)):
                                eng = nc.scalar if g % 2 == 0 else nc.vector
                                (eng.copy if g % 2 == 0 else eng.tensor_copy)(
                                    qs[g * 32:(g + 1) * 32, :],
                                    qf[src_g * 32:(src_g + 1) * 32, :])
                            m1 = sp.tile([128, 512], f32, name=f'm1{l}')
                            nc.vector.tensor_mul(m1[:], qf[:], C['cosq'][:, col:col + 512])
                            m2 = sp.tile([128, 512], f32, name=f'm2{l}')
                            nc.vector.tensor_mul(m2[:], qs[:], C['sinq'][:, col:col + 512])
                            nc.vector.tensor_add(qsb[m][:, col:col + 512], m1[:], m2[:])
                        else:
                            kf = sp.tile([64, 512], f32, name=f'kf{l}')
                            nc.vector.tensor_scalar_mul(kf[:], ps[0:64, :], sqkv[0:64, 2:3])
                            ks = sp.tile([64, 512], f32, name=f'ks{l}')
                            nc.scalar.copy(ks[0:32, :], kf[32:64, :])
                            nc.scalar.copy(ks[32:64, :], kf[0:32, :])
                            m1k = sp.tile([64, 512], f32, name=f'm1k{l}')
                            nc.vector.tensor_mul(m1k[:], kf[:], C['cosq'][0:64, col:col + 512])
                            m2k = sp.tile([64, 512], f32, name=f'm2k{l}')
                            nc.vector.tensor_mul(m2k[:], ks[:], C['sinq'][0:64, col:col + 512])
                            nc.vector.tensor_add(ksb[0:64, col:col + 512], m1k[:], m2k[:])
                            nc.vector.tensor_add(ksb[64:128, col:col + 512], m1k[:], m2k[:])
                            vf = sp.tile([64, 512], bf16, name=f'vf{l}')
                            nc.vector.tensor_scalar_mul(vf[:], ps[64:128, :], sqkv[64:128, 2:3])
                            b = tb // 2
                            for j in range(4):
                                kb = (tb % 2) * 4 + j
                                vps = pp.tile([128, 64], f32, name=f'vtp{l}', bufs=2)
                                nc.tensor.matmul(vps[:], vf[:, j * 128:(j + 1) * 128],
                                                 C['identbf'][0:64, 0:64],
                                                 start=True, stop=True)
                                nc.scalar.copy(vt[(b, kb)][:], vps[:])

            if last_layer:
                return ksb, vt, wq_sl, ph.pop_all()

            # ---- attention core (layers 0..2) ----
            attnf = [atp.tile([128, T], bf16, name=f'af{l}_{m}') for m in range(2)]
            with tc.tile_pool(name=f'sc_{l}', bufs=2) as sp, \
                 tc.tile_pool(name=f'scp_{l}', bufs=2, space='PSUM') as pp:
                for b in range(B):
                    for h in range(QH):
                        qrows = ((h % 2) * 64, (h % 2) * 64 + 64)
                        qt_tile = qsb[h // 2]
                        psb = []
                        dgs = []
                        for qt in range(8):
                            W = (qt + 1) * 128
                            sps = pp.tile([128, 1024], f32, name=f'sps{l}', bufs=2)
                            for c0 in range(0, W, 512):
                                cw = min(512, W - c0)
                                nc.tensor.matmul(
                                    sps[:, c0:c0 + cw],
                                    qt_tile[qrows[0]:qrows[1],
                                            b * 1024 + qt * 128: b * 1024 + qt * 128 + 128],
                                    ksb[qrows[0]:qrows[1],
                                        b * 1024 + c0: b * 1024 + c0 + cw],
                                    start=True, stop=True)
                            nc.vector.tensor_add(sps[:, qt * 128:W],
                                                 sps[:, qt * 128:W], C['trimask'][:])
                            nm = sp.tile([128, 1], f32, name=f'nm{l}', bufs=3)
                            nc.vector.tensor_reduce(out=nm[:], in_=sps[:, 0:W],
                                                    axis=AX, op=mybir.AluOpType.max,
                                                    negate=True)
                            pt = sp.tile([128, 1024], bf16, name=f'pexp{l}_{qt}')
                            den = sp.tile([128, 1], f32, name=f'den{l}', bufs=3)
                            nc.scalar.activation(pt[:, 0:W], sps[:, 0:W], AF.Exp,
                                                 bias=nm[:], scale=1.0,
                                                 accum_out=den[:])
                            rden = sp.tile([128, 1], f32, name=f'rden{l}', bufs=3)
                            nc.vector.reciprocal(rden[:], den[:])
                            dg = sp.tile([128, 128], bf16, name=f'dg{l}_{qt}')
                            nc.vector.tensor_scalar_mul(dg[:], C['identbf'][:], rden[:])
                            psb.append(pt)
                            dgs.append(dg)
                        for Hh in range(2):
                            pv = pp.tile([64, 512], f32, name=f'pvps{l}', bufs=2)
                            for kb in range(4 * Hh + 4):
                                qt0 = max(kb, 4 * Hh)
                                ptp = pp.tile([128, 512], f32, name=f'ptp{l}', bufs=2)
                                for qt in range(qt0, 4 * Hh + 4):
                                    nc.tensor.matmul(
                                        ptp[:, (qt - 4 * Hh) * 128:(qt - 4 * Hh + 1) * 128],
                                        psb[qt][:, kb * 128:(kb + 1) * 128],
                                        dgs[qt][:], start=True, stop=True)
                                cs = (qt0 - 4 * Hh) * 128
                                pts = sp.tile([128, 512], bf16, name=f'pts{l}', bufs=3)
                                eng = nc.vector if kb % 2 == 0 else nc.scalar
                                (eng.tensor_copy if kb % 2 == 0 else eng.copy)(
                                    pts[:, cs:512], ptp[:, cs:512])
                                nc.tensor.matmul(pv[:, cs:512], vt[(b, kb)][:],
                                                 pts[:, cs:512],
                                                 start=(kb == 0), stop=(kb == 4 * Hh + 3))
                            nc.scalar.copy(
                                attnf[h // 2][(h % 2) * 64:(h % 2) * 64 + 64,
                                              b * 1024 + Hh * 512: b * 1024 + Hh * 512 + 512],
                                pv[:])

            # ---- o-proj ----
            so = C[f'so{l}']
            with tc.tile_pool(name=f'wo_{l}', bufs=1) as wsp2, \
                 tc.tile_pool(name=f'op_{l}', bufs=2) as sp, \
                 tc.tile_pool(name=f'opp_{l}', bufs=3, space='PSUM') as pp:
                wo8 = wsp2.tile([128, 2 * D], fp8, name=f'wo{l}')
                nc.sync.dma_start(out=wo8[:], in_=I[f'wo{l}'][:])
                for tb in range(NC):
                    oball = sp.tile([128, KC * 512], bf16, name=f'ob{l}', bufs=2)
                    for m in range(KC):
                        ops = pp.tile([128, 512], f32, name=f'ops{l}', bufs=3)
                        for kc in range(2):
                            nc.tensor.matmul(
                                ops[:], wo8[:, kc * D + m * 128: kc * D + (m + 1) * 128],
                                attnf[kc][:, tb * 512:(tb + 1) * 512],
                                start=(kc == 0), stop=(kc == 1))
                        ob = oball[:, m * 512:(m + 1) * 512]
                        if m % 2 == 0:
                            nc.vector.tensor_scalar_mul(ob[:], ops[:], so[:, m:m + 1])
                        else:
                            nc.scalar.activation(ob[:], ops[:], AF.Copy,
                                                 scale=so[:, m:m + 1])
                    nc.sync.dma_start(out=_r3(rs_in[tb * D:(tb + 1) * D, :]),
                                      in_=_s3(oball[:]))
        return None

    def mlp(l, xn_buf):
        """MLP block for layers 0..2 (full T tokens)."""
        sg, sud, sd = C[f'sg{l}'], C[f'sud{l}'], C[f'sd{l}']
        with ExitStack() as ph:
            wsp = ph.enter_context(tc.tile_pool(name=f'wm_{l}', bufs=1))
            wg8 = wsp.tile([128, KC * DFFS], fp8, name=f'wg{l}')
            nc.sync.dma_start(out=wg8[:], in_=I[f'wg{l}'][:])
            wu8 = wsp.tile([128, KC * DFFS], fp8, name=f'wu{l}')
            nc.sync.dma_start(out=wu8[:], in_=I[f'wu{l}'][:])
            wd8 = wsp.tile([128, MFF * D], fp8, name=f'wd{l}')
            nc.sync.dma_start(out=wd8[:], in_=I[f'wd{l}'][:])
            with tc.tile_pool(name=f'ml_{l}', bufs=2) as sp, \
                 tc.tile_pool(name=f'mlp_{l}', bufs=2, space='PSUM') as pp:
                for tb in range(NC):
                    xn = load_xn(sp, xn_buf, tb, f'm{l}', bufs=2)
                    hmall = sp.tile([128, MFF * 512], bf16, name=f'hm{l}', bufs=2)
                    for mf in range(MFF):
                        gps = pp.tile([128, 512], f32, name=f'gps{l}', bufs=2)
                        for k in range(KC):
                            nc.tensor.matmul(
                                gps[:], wg8[:, k * DFFS + mf * 128: k * DFFS + (mf + 1) * 128],
                                xn[k][:], start=(k == 0), stop=(k == KC - 1))
                        ups = pp.tile([128, 512], f32, name=f'ups{l}', bufs=2)
                        for k in range(KC):
                            nc.tensor.matmul(
                                ups[:], wu8[:, k * DFFS + mf * 128: k * DFFS + (mf + 1) * 128],
                                xn[k][:], start=(k == 0), stop=(k == KC - 1))
                        gsb = sp.tile([128, 512], bf16, name=f'gsb{l}', bufs=2)
                        nc.scalar.activation(gsb[:], gps[:], AF.Silu,
                                             scale=sg[:, mf:mf + 1])
                        nc.vector.scalar_tensor_tensor(
                            out=hmall[:, mf * 512:(mf + 1) * 512], in0=ups[:],
                            scalar=sud[:, mf:mf + 1], in1=gsb[:], op0=MUL, op1=MUL)
                    dball = sp.tile([128, KC * 512], bf16, name=f'db{l}', bufs=2)
                    for mo in range(KC):
                        dps = pp.tile([128, 512], f32, name=f'dps{l}', bufs=3)
                        for k in range(MFF):
                            nc.tensor.matmul(
                                dps[:], wd8[:, k * D + mo * 128: k * D + (mo + 1) * 128],
                                hmall[:, k * 512:(k + 1) * 512],
                                start=(k == 0), stop=(k == MFF - 1))
                        db = dball[:, mo * 512:(mo + 1) * 512]
                        if mo % 2 == 0:
                            nc.vector.tensor_scalar_mul(db[:], dps[:], sd[:, mo:mo + 1])
                        else:
                            nc.scalar.activation(db[:], dps[:], AF.Copy,
                                                 scale=sd[:, mo:mo + 1])
                    nc.sync.dma_start(out=_r3(rs_in[tb * D:(tb + 1) * D, :]),
                                      in_=_s3(dball[:]))

    # ------------------------------------------------------------------
    # layers 0..2
    # ------------------------------------------------------------------
    for l in range(L - 1):
        agb = allgather_norm(xs, C[f'ga{l}'][:], f'a{l}')
        qkv_attention(l, agb, last_layer=False)
        reduce_scatter_add(f'o{l}')
        agb = allgather_norm(xs, C[f'gm{l}'][:], f'm{l}')
        mlp(l, agb)
        reduce_scatter_add(f'd{l}')

    # ------------------------------------------------------------------
    # layer 3 (last): only last-position tokens through q/attn/mlp/head
    # ------------------------------------------------------------------
    l = L - 1
    lx_in = dram_p.tile([D, 1], f32, name='lx_in')
    lx_out = dram_p.tile([NC * D, 1], f32, name='lx_out',
                         addr_space=('Local' if TLSIM else 'Shared'))
    ar_in = dram_p.tile([D, 4], f32, name='ar_in')
    ar_out = dram_p.tile([D, 4], f32, name='ar_out',
                         addr_space=('Local' if TLSIM else 'Shared'))
    ar2_in = dram_p.tile([D, 4], f32, name='ar2_in')
    ar2_out = dram_p.tile([D, 4], f32, name='ar2_out',
                          addr_space=('Local' if TLSIM else 'Shared'))

    for k in range(KC):
        nc.sync.dma_start(out=lx_in[k * 128:(k + 1) * 128, :],
                          in_=xs[k][:, 511:512])
    if TLSIM:
        for r in range(NC):
            nc.sync.dma_start(out=lx_out[r * D:(r + 1) * D, :], in_=lx_in[:])
    else:
        nc.gpsimd.collective_compute('AllGather', mybir.AluOpType.bypass,
                                     replica_groups=RG, ins=[lx_in.opt()],
                                     outs=[lx_out.opt()])
    l4p = top.enter_context(tc.tile_pool(name='l4p', bufs=1))
    lastx = []
    for k in range(KC):
        t = l4p.tile([128, 4], f32, name=f'lastx{k}')
        src = bass.AP(tensor=lx_out.tensor, offset=lx_out[:].offset + D + k * 128,
                      ap=[[1, 128], [2 * D, 4]])
        nc.sync.dma_start(out=t[:], in_=src)
        lastx.append(t)

    # norm for q (on last-position tokens)
    qn4 = [l4p.tile([128, 4], bf16, name=f'qn4_{k}') for k in range(KC)]
    _fm_norm(nc, tc, ctx, lastx, C[f'ga{l}'][:], 4, qn4, 'q4')

    # full norm + AG for k/v
    agb = allgather_norm(xs, C[f'ga{l}'][:], f'a{l}')
    ksb, vt, wq_sl, wctx = qkv_attention(l, agb, last_layer=True)

    sqkv = C[f'sqkv{l}']
    q4h = [l4p.tile([64, 4], bf16, name=f'q4h_{h}') for h in range(QH)]
    at4 = [l4p.tile([64, 4], bf16, name=f'at4_{h}') for h in range(QH)]
    with tc.tile_pool(name='l4qs', bufs=2) as sp, \
         tc.tile_pool(name='l4qp', bufs=2, space='PSUM') as pp:
        for m in range(2):
            ps = pp.tile([128, 4], f32, name='q4ps', bufs=2)
            for k in range(KC):
                nc.tensor.matmul(ps[:], wq_sl(k, m), qn4[k][:],
                                 start=(k == 0), stop=(k == KC - 1))
            qf = sp.tile([128, 4], f32, name='q4f')
            nc.vector.tensor_scalar_mul(qf[:], ps[:], sqkv[:, m:m + 1])
            qs = sp.tile([128, 4], f32, name='q4s')
            for g, src_g in ((0, 1), (1, 0), (2, 3), (3, 2)):
                nc.vector.tensor_copy(qs[g * 32:(g + 1) * 32, :],
                                      qf[src_g * 32:(src_g + 1) * 32, :])
            m1 = sp.tile([128, 4], f32, name='q4m1')
            nc.vector.tensor_mul(m1[:], qf[:], C['cosq4'][:])
            m2 = sp.tile([128, 4], f32, name='q4m2')
            nc.vector.tensor_mul(m2[:], qs[:], C['sinq4'][:])
            for sub in range(2):
                nc.vector.tensor_add(q4h[m * 2 + sub][:],
                                     m1[sub * 64:(sub + 1) * 64, :],
                                     m2[sub * 64:(sub + 1) * 64, :])

    # attention for 4 last tokens
    with tc.tile_pool(name='l4as', bufs=2) as sp, \
         tc.tile_pool(name='l4ap', bufs=1, space='PSUM') as pp:
        for b in range(B):
            for h in range(QH):
                s4 = pp.tile([1, 1024], f32, name='s4ps', bufs=2)
                for c0 in range(0, 1024, 512):
                    nc.tensor.matmul(s4[:, c0:c0 + 512],
                                     q4h[h][:, b:b + 1],
                                     ksb[0:64, b * 1024 + c0: b * 1024 + c0 + 512],
                                     start=True, stop=True)
                nm = sp.tile([1, 1], f32, name='nm4', bufs=3)
                nc.vector.tensor_reduce(out=nm[:], in_=s4[:], axis=AX,
                                        op=mybir.AluOpType.max, negate=True)
                p4 = sp.tile([1, 1024], bf16, name='p4', bufs=2)
                den = sp.tile([1, 1], f32, name='den4', bufs=3)
                nc.scalar.activation(p4[:], s4[:], AF.Exp, bias=nm[:], scale=1.0,
                                     accum_out=den[:])
                rden = sp.tile([1, 1], f32, name='rden4', bufs=3)
                nc.vector.reciprocal(rden[:], den[:])
                rbcp = pp.tile([128, 1], f32, name='rbcp', bufs=1)
                nc.tensor.matmul(rbcp[:], C['ones_m'][:], rden[:], start=True, stop=True)
                rbc = sp.tile([128, 1], f32, name='rbc4', bufs=3)
                nc.scalar.copy(rbc[:], rbcp[:])
                pt4p = pp.tile([128, 8], f32, name='pt4p', bufs=1)
                for kb in range(8):
                    nc.tensor.matmul(pt4p[:, kb:kb + 1], p4[:, kb * 128:(kb + 1) * 128],
                                     C['onebf'][:], start=True, stop=True)
                pt4 = sp.tile([128, 8], bf16, name='pt4', bufs=2)
                nc.vector.tensor_scalar_mul(pt4[:], pt4p[:], rbc[:])
                pv4 = pp.tile([64, 1], f32, name='pv4', bufs=2)
                for kb in range(8):
                    nc.tensor.matmul(pv4[:], vt[(b, kb)][:], pt4[:, kb:kb + 1],
                                     start=(kb == 0), stop=(kb == 7))
                nc.scalar.copy(at4[h][:, b:b + 1], pv4[:])
    wctx.close()

    # o-proj for 4 tokens
    so = C[f'so{l}']
    with tc.tile_pool(name='wo3', bufs=1) as wsp2, \
         tc.tile_pool(name='l4os', bufs=2) as sp, \
         tc.tile_pool(name='l4op', bufs=2, space='PSUM') as pp:
        wo4h = []
        for h in range(QH):
            r0 = (h % 2) * 64
            wt = wsp2.tile([64, D], fp8, name=f'wo4t_{h}')
            nc.sync.dma_start(out=wt[:],
                              in_=I[f'wo{l}'][r0:r0 + 64, (h // 2) * D:(h // 2 + 1) * D])
            wo4h.append(wt)
        for m in range(KC):
            ops = pp.tile([128, 4], f32, name='o4ps', bufs=2)
            for h in range(QH):
                nc.tensor.matmul(
                    ops[:], wo4h[h][:, m * 128:(m + 1) * 128],
                    at4[h][:], start=(h == 0), stop=(h == QH - 1))
            ob = sp.tile([128, 4], f32, name='o4b', bufs=3)
            nc.vector.tensor_scalar_mul(ob[:], ops[:], so[:, m:m + 1])
            nc.sync.dma_start(out=ar_in[m * 128:(m + 1) * 128, :], in_=ob[:])

    if TLSIM:
        nc.sync.dma_start(out=ar_out[:], in_=ar_in[:])
    else:
        nc.gpsimd.collective_compute('AllReduce', mybir.AluOpType.add,
                                     replica_groups=RG, ins=[ar_in.opt()],
                                     outs=[ar_out.opt()])

    # residual add (4 tokens)
    x4 = []
    with tc.tile_pool(name='l4r', bufs=3) as sp:
        for k in range(KC):
            rt = sp.tile([128, 4], f32, name='ar4l')
            nc.sync.dma_start(out=rt[:], in_=ar_out[k * 128:(k + 1) * 128, :])
            t = l4p.tile([128, 4], f32, name=f'x4_{k}')
            nc.vector.tensor_add(t[:], lastx[k][:], rt[:])
            x4.append(t)

    # norm2 + tiny MLP
    xn4 = [l4p.tile([128, 4], bf16, name=f'xn4_{k}') for k in range(KC)]
    _fm_norm(nc, tc, ctx, x4, C[f'gm{l}'][:], 4, xn4, 'm4')
    sg, sud, sd = C[f'sg{l}'], C[f'sud{l}'], C[f'sd{l}']
    with ExitStack() as ph:
        wsp = ph.enter_context(tc.tile_pool(name='wm3', bufs=1))
        wg8 = wsp.tile([128, KC * DFFS], fp8, name='wg3t')
        nc.sync.dma_start(out=wg8[:], in_=I[f'wg{l}'][:])
        wu8 = wsp.tile([128, KC * DFFS], fp8, name='wu3t')
        nc.sync.dma_start(out=wu8[:], in_=I[f'wu{l}'][:])
        wd8 = wsp.tile([128, MFF * D], fp8, name='wd3t')
        nc.sync.dma_start(out=wd8[:], in_=I[f'wd{l}'][:])
        with tc.tile_pool(name='m4s', bufs=2) as sp, \
             tc.tile_pool(name='m4p', bufs=2, space='PSUM') as pp:
            hm = []
            for mf in range(MFF):
                gps = pp.tile([128, 4], f32, name='g4ps', bufs=2)
                for k in range(KC):
                    nc.tensor.matmul(
                        gps[:], wg8[:, k * DFFS + mf * 128: k * DFFS + (mf + 1) * 128],
                        xn4[k][:], start=(k == 0), stop=(k == KC - 1))
                ups = pp.tile([128, 4], f32, name='u4ps', bufs=2)
                for k in range(KC):
                    nc.tensor.matmul(
                        ups[:], wu8[:, k * DFFS + mf * 128: k * DFFS + (mf + 1) * 128],
                        xn4[k][:], start=(k == 0), stop=(k == KC - 1))
                gsb = sp.tile([128, 4], bf16, name='g4sb', bufs=3)
                nc.scalar.activation(gsb[:], gps[:], AF.Silu, scale=sg[:, mf:mf + 1])
                ht = sp.tile([128, 4], bf16, name=f'h4_{mf}')
                nc.vector.scalar_tensor_tensor(
                    out=ht[:], in0=ups[:], scalar=sud[:, mf:mf + 1],
                    in1=gsb[:], op0=MUL, op1=MUL)
                hm.append(ht)
            for mo in range(KC):
                dps = pp.tile([128, 4], f32, name='d4ps', bufs=2)
                for k in range(MFF):
                    nc.tensor.matmul(
                        dps[:], wd8[:, k * D + mo * 128: k * D + (mo + 1) * 128],
                        hm[k][:], start=(k == 0), stop=(k == MFF - 1))
                db = sp.tile([128, 4], f32, name='d4b', bufs=3)
                nc.vector.tensor_scalar_mul(db[:], dps[:], sd[:, mo:mo + 1])
                nc.sync.dma_start(out=ar2_in[mo * 128:(mo + 1) * 128, :], in_=db[:])

    if TLSIM:
        nc.sync.dma_start(out=ar2_out[:], in_=ar2_in[:])
    else:
        nc.gpsimd.collective_compute('AllReduce', mybir.AluOpType.add,
                                     replica_groups=RG, ins=[ar2_in.opt()],
                                     outs=[ar2_out.opt()])

    # final residual + final norm + LM head
    with tc.tile_pool(name='fhs', bufs=2) as sp, \
         tc.tile_pool(name='fhp', bufs=2, space='PSUM') as pp:
        xf = []
        for k in range(KC):
            rt = sp.tile([128, 4], f32, name='ar4l2', bufs=3)
            nc.sync.dma_start(out=rt[:], in_=ar2_out[k * 128:(k + 1) * 128, :])
            t = l4p.tile([128, 4], f32, name=f'xf_{k}')
            nc.vector.tensor_add(t[:], x4[k][:], rt[:])
            xf.append(t)
        xfn = [l4p.tile([128, 4], bf16, name=f'xfn_{k}') for k in range(KC)]
        _fm_norm(nc, tc, ctx, xf, C['gf'][:], 4, xfn, 'f4')
        nch = (VS + 511) // 512
        for n in range(nch):
            cw = min(512, VS - n * 512)
            hps = pp.tile([4, 512], f32, name='hps', bufs=2)
            et = sp.tile([128, KC * 512], bf16, name='embt', bufs=3)
            nc.sync.dma_start(
                out=et[:].rearrange("p (k c) -> p k c", k=KC)[:, :, 0:cw],
                in_=_r3(I['embT'][:, n * 512:n * 512 + cw]))
            for k in range(KC):
                nc.tensor.matmul(hps[:, 0:cw], xfn[k][:],
                                 et[:, k * 512:k * 512 + cw],
                                 start=(k == 0), stop=(k == KC - 1))
            lsb = sp.tile([4, 512], f32, name='lsb', bufs=3)
            nc.scalar.copy(lsb[:, 0:cw], hps[:, 0:cw])
            nc.sync.dma_start(out=logits_out[:, n * 512:n * 512 + cw],
                              in_=lsb[:, 0:cw])


# ----------------------------------------------------------------------------
# host-side prep
# ----------------------------------------------------------------------------

def _chunk_pack(a, nchunks):
    """[(nchunks*128), cols] -> [128, nchunks*cols] fp8 (chunk-major in free dim)."""
    cols = a.shape[1]
    return np.ascontiguousarray(
        a.reshape(nchunks, 128, cols).transpose(1, 0, 2).reshape(128, nchunks * cols)
        .astype(ml_dtypes.float8_e4m3))


def _prep_in_maps(token_ids, embed, gamma_attn, gamma_mlp, gamma_final,
                  wq, sq, wk, sk, wv, sv, wo, so, wg, sg, wu, su, wd, sd):
    half = HD // 2
    inv = ROPE_THETA ** (-np.arange(half, dtype=np.float32) * 2.0 / HD)
    ang = np.arange(S, dtype=np.float32)[:, None] * inv          # [S, 32]
    cos1 = np.cos(ang).T.astype(np.float32)                      # [32, S]
    sin1 = np.sin(ang).T.astype(np.float32)
    cos64 = np.concatenate([cos1, cos1], 0)                      # [64, S]
    sin64s = np.concatenate([-sin1, sin1], 0)
    cosq = np.tile(np.concatenate([cos64, cos64], 0), (1, B))    # [128, T]
    sinq = np.tile(np.concatenate([sin64s, sin64s], 0), (1, B))
    cosq4 = np.repeat(cosq[:, S - 1:S], 4, axis=1).copy()
    sinq4 = np.repeat(sinq[:, S - 1:S], 4, axis=1).copy()

    ii, jj = np.meshgrid(np.arange(128), np.arange(128), indexing='ij')
    trimask = np.where(jj <= ii, 0.0, NEG).astype(np.float32)

    tok = np.asarray(token_ids).reshape(T)
    x0full = np.ascontiguousarray(embed[tok].T.astype(np.float32))  # [D, T]
    embT = np.ascontiguousarray(embed.T.astype(ml_dtypes.bfloat16))  # [D, V]

    def percol(a):
        return np.ascontiguousarray(a.reshape(-1, 128).T.astype(np.float32))

    common = {
        'cosq': np.ascontiguousarray(cosq.astype(ml_dtypes.bfloat16)),
        'sinq': np.ascontiguousarray(sinq.astype(ml_dtypes.bfloat16)),
        'cosq4': cosq4, 'sinq4': sinq4, 'trimask': trimask,
        'identbf': np.eye(128, dtype=ml_dtypes.bfloat16),
        'ones_k': np.ones((128, 1), np.float32),
        'ones_m': np.ones((1, 128), np.float32),
        'onebf': np.ones((1, 1), ml_dtypes.bfloat16),
        'gf': percol(gamma_final),
    }
    in_maps = []
    for c in range(NC):
        m = dict(common)
        m['x0'] = np.ascontiguousarray(x0full[:, c * TB:(c + 1) * TB])
        m['embT'] = np.ascontiguousarray(embT[:, c * VS:(c + 1) * VS])
        for l in range(L):
            qsl = slice(c * DQ, (c + 1) * DQ)
            ksl = slice(c * HD, (c + 1) * HD)
            fsl = slice(c * DFFS, (c + 1) * DFFS)
            m[f'wqkv{l}'] = _chunk_pack(np.concatenate(
                [wq[l][qsl].T, wk[l][ksl].T, wv[l][ksl].T], axis=1), KC)
            sq_l = sq[l][qsl] * np.float32(1.0 / np.sqrt(HD))
            m[f'sqkv{l}'] = np.ascontiguousarray(np.stack(
                [sq_l[0:128], sq_l[128:256],
                 np.concatenate([sk[l][ksl], sv[l][ksl]])], axis=1).astype(np.float32))
            m[f'wo{l}'] = _chunk_pack(wo[l][:, qsl].T, 2)
            m[f'so{l}'] = percol(so[l])
            m[f'wg{l}'] = _chunk_pack(wg[l][fsl].T, KC)
            m[f'sg{l}'] = percol(sg[l][fsl])
            m[f'wu{l}'] = _chunk_pack(wu[l][fsl].T, KC)
            m[f'wd{l}'] = _chunk_pack(wd[l][:, fsl].T, MFF)
            m[f'sud{l}'] = percol(su[l][fsl])
            m[f'sd{l}'] = percol(sd[l])
            m[f'ga{l}'] = percol(gamma_attn[l])
            m[f'gm{l}'] = percol(gamma_mlp[l])
        in_maps.append(m)
    return in_maps


def _get_nc():
    if 'nc' not in _CACHE:
        _CACHE['nc'] = _build()
    return _CACHE['nc']


def kernel(**inputs) -> np.ndarray:
    inputs = {k: np.asarray(v) for k, v in inputs.items()}
    in_maps = _prep_in_maps(**inputs)
    nc = _get_nc()
    res = bass_utils.run_bass_kernel_spmd(nc, in_maps, core_ids=list(range(NC)))
    logits = np.concatenate([res.results[c]['logits'] for c in range(NC)], axis=1)
    return logits.astype(np.float32)


# revision 5
# speedup vs baseline: 11.9950x; 1.4879x over previous
"""Self-contained Trainium2 Bass kernel for the int4-quantized 4-layer Llama decode problem.

Strategy: tensor-parallel over 8 NeuronCores (attention heads + FFN hidden dim),
sequence-parallel residual (each core keeps a feature-major fp32 residual shard
[D, T/8] in SBUF), AllGather before QKV/MLP, ReduceScatter after o-proj/down-proj.
Weights are host-packed to fp8e4 (int4 values are exact in e4m3) and used directly
as the stationary matmul operand against bf16 activations; dequant scales are
applied to the matmul outputs (per-out-channel) or to the activations (down-proj
input channels). Activation/residual traffic between SBUF and the DRAM collective
bounce buffers moves in single strided DMAs per 2048-feature block.
Only the last position of each sequence goes through layer-4 Q/attention/MLP and
the LM head.
"""
import sys

sys.path.insert(0, '/opt/trn_rl_repo')

import numpy as np
import ml_dtypes
from contextlib import ExitStack

import concourse.bass as bass
import concourse.tile as tile
from concourse import bacc, mybir
from concourse import bass_utils

# model dims (hardcoded per problem spec)
L, D, H, HD, KVH, DFF, V, B, S = 4, 2048, 32, 64, 8, 8192, 32000, 4, 1024
NC = 8
T = B * S              # 4096 tokens
TB = T // NC           # 512 tokens per core shard
QH = H // NC           # 4 local q heads
DQ = QH * HD           # 256 local q dims
DFFS = DFF // NC       # 1024 local ffn dims
VS = V // NC           # 4000 local vocab
KC = D // 128          # 16 feature chunks
MFF = DFFS // 128      # 8
ROPE_THETA = 500000.0
NEG = np.float32(-1e9)
EPS = 1e-5

f32 = mybir.dt.float32
bf16 = mybir.dt.bfloat16
fp8 = mybir.dt.float8e4
i32 = mybir.dt.int32

AX = mybir.AxisListType.X
MUL = mybir.AluOpType.mult
AF = mybir.ActivationFunctionType

_CACHE = {}
TLSIM = False  # single-core cost-model sim mode (collectives -> DMA copies)


# ----------------------------------------------------------------------------
# bass program
# ----------------------------------------------------------------------------

def _declare_inputs(nc):
    I = {}
    I['x0'] = nc.dram_tensor('x0', [D, TB], f32, kind='ExternalInput').ap()
    for l in range(L):
        I[f'wqkv{l}'] = nc.dram_tensor(f'wqkv{l}', [128, KC * 384], fp8, kind='ExternalInput').ap()
        I[f'sqkv{l}'] = nc.dram_tensor(f'sqkv{l}', [128, 3], f32, kind='ExternalInput').ap()
        I[f'wo{l}'] = nc.dram_tensor(f'wo{l}', [128, 2 * D], fp8, kind='ExternalInput').ap()
        I[f'so{l}'] = nc.dram_tensor(f'so{l}', [128, KC], f32, kind='ExternalInput').ap()
        I[f'wg{l}'] = nc.dram_tensor(f'wg{l}', [128, KC * DFFS], fp8, kind='ExternalInput').ap()
        I[f'sg{l}'] = nc.dram_tensor(f'sg{l}', [128, MFF], f32, kind='ExternalInput').ap()
        I[f'wu{l}'] = nc.dram_tensor(f'wu{l}', [128, KC * DFFS], fp8, kind='ExternalInput').ap()
        I[f'wd{l}'] = nc.dram_tensor(f'wd{l}', [128, MFF * D], fp8, kind='ExternalInput').ap()
        I[f'sud{l}'] = nc.dram_tensor(f'sud{l}', [128, MFF], f32, kind='ExternalInput').ap()
        I[f'sd{l}'] = nc.dram_tensor(f'sd{l}', [128, KC], f32, kind='ExternalInput').ap()
        I[f'ga{l}'] = nc.dram_tensor(f'ga{l}', [128, KC], f32, kind='ExternalInput').ap()
        I[f'gm{l}'] = nc.dram_tensor(f'gm{l}', [128, KC], f32, kind='ExternalInput').ap()
    I['gf'] = nc.dram_tensor('gf', [128, KC], f32, kind='ExternalInput').ap()
    I['cosq'] = nc.dram_tensor('cosq', [128, T], bf16, kind='ExternalInput').ap()
    I['sinq'] = nc.dram_tensor('sinq', [128, T], bf16, kind='ExternalInput').ap()
    I['cosq4'] = nc.dram_tensor('cosq4', [128, 4], f32, kind='ExternalInput').ap()
    I['sinq4'] = nc.dram_tensor('sinq4', [128, 4], f32, kind='ExternalInput').ap()
    I['trimask'] = nc.dram_tensor('trimask', [128, 128], f32, kind='ExternalInput').ap()
    I['identbf'] = nc.dram_tensor('identbf', [128, 128], bf16, kind='ExternalInput').ap()
    I['ones_k'] = nc.dram_tensor('ones_k', [128, 1], f32, kind='ExternalInput').ap()
    I['ones_m'] = nc.dram_tensor('ones_m', [1, 128], f32, kind='ExternalInput').ap()
    I['onebf'] = nc.dram_tensor('onebf', [1, 1], bf16, kind='ExternalInput').ap()
    I['embT'] = nc.dram_tensor('embT', [D, VS], bf16, kind='ExternalInput').ap()
    return I


def _fm_norm(nc, tc, ctx, src, gamma_ap, width, out_tiles, tag):
    """Feature-major rmsnorm: src = list of KC sbuf [128,width] f32 APs.
    Writes out_tiles (KC APs, caller-allocated, any dtype)."""
    with tc.tile_pool(name=f'np_{tag}', bufs=2) as sp, \
         tc.tile_pool(name=f'npp_{tag}', bufs=2, space='PSUM') as pp:
        C = ctx['const']
        ssum = pp.tile([1, width], f32, name=f'nsum_{tag}')
        for k in range(KC):
            xsq = sp.tile([128, width], f32, name=f'nxsq_{tag}', bufs=3)
            nc.vector.tensor_mul(xsq[:], src[k][:], src[k][:])
            nc.tensor.matmul(ssum[:], C['ones_k'][:], xsq[:],
                             start=(k == 0), stop=(k == KC - 1))
        sq = sp.tile([1, width], f32, name=f'nsq_{tag}')
        nc.scalar.activation(sq[:], ssum[:], AF.Sqrt, bias=ctx['eps'][0:1, :],
                             scale=1.0 / D)
        rstd = sp.tile([1, width], f32, name=f'nrstd_{tag}')
        nc.vector.reciprocal(rstd[:], sq[:])
        bcp = pp.tile([128, width], f32, name=f'nbc_{tag}')
        nc.tensor.matmul(bcp[:], C['ones_m'][:], rstd[:], start=True, stop=True)
        rbc = sp.tile([128, width], f32, name=f'nrbc_{tag}')
        nc.scalar.copy(rbc[:], bcp[:])
        for k in range(KC):
            nc.vector.scalar_tensor_tensor(
                out=out_tiles[k][:], in0=src[k][:], scalar=gamma_ap[:, k:k + 1],
                in1=rbc[:], op0=MUL, op1=MUL)


def _r3(dram_ap, nchunks=KC):
    """[(k p), c] DRAM slice -> [p, k, c] AP for batched strided DMA."""
    return dram_ap.rearrange("(k p) c -> p k c", k=nchunks)


def _s3(sb_ap, nchunks=KC):
    """[p, (k c)] SBUF tile -> [p, k, c] AP."""
    return sb_ap.rearrange("p (k c) -> p k c", k=nchunks)


def _build(reps=1):
    nc = bacc.Bacc('TRN2', target_bir_lowering=False, debug=False,
                   num_devices=(1 if TLSIM else NC))
    I = _declare_inputs(nc)
    logits_out = nc.dram_tensor('logits', [4, VS], f32, kind='ExternalOutput').ap()

    with tile.TileContext(nc) as tc, ExitStack() as top:
        const_p = top.enter_context(tc.tile_pool(name='constp', bufs=1))
        resid_p = top.enter_context(tc.tile_pool(name='residp', bufs=1))
        dram_p = top.enter_context(tc.tile_pool(name='dramp', bufs=1, space='DRAM'))

        C = {}
        for cn, shape, dt in [('cosq', [128, T], bf16), ('sinq', [128, T], bf16),
                              ('cosq4', [128, 4], f32), ('sinq4', [128, 4], f32),
                              ('trimask', [128, 128], f32), ('identbf', [128, 128], bf16),
                              ('ones_k', [128, 1], f32), ('ones_m', [1, 128], f32),
                              ('onebf', [1, 1], bf16), ('gf', [128, KC], f32)]:
            t = const_p.tile(shape, dt, name=f'c_{cn}')
            nc.sync.dma_start(out=t[:], in_=I[cn][:])
            C[cn] = t
        for l in range(L):
            for cn in ('sqkv', 'so', 'sg', 'sud', 'sd', 'ga', 'gm'):
                shp = [128, {'sqkv': 3, 'so': KC, 'sg': MFF, 'sud': MFF,
                             'sd': KC, 'ga': KC, 'gm': KC}[cn]]
                t = const_p.tile(shp, f32, name=f'c_{cn}{l}')
                nc.sync.dma_start(out=t[:], in_=I[f'{cn}{l}'][:])
                C[f'{cn}{l}'] = t
        epst = const_p.tile([128, 1], f32, name='c_eps')
        nc.vector.memset(epst[:], EPS)
        ctx = {'const': C, 'eps': epst}

        for _rep in range(reps):
            _body(nc, tc, top, I, C, ctx, dram_p, resid_p, logits_out)

    nc.compile()
    return nc


def _body(nc, tc, top, I, C, ctx, dram_p, resid_p, logits_out):
    # persistent residual shard [D, TB] fp32 as one [128, KC*TB] tile
    xsb = resid_p.tile([128, KC * TB], f32, name='xsh')
    nc.sync.dma_start(out=_s3(xsb[:]), in_=_r3(I['x0'][:]))
    xs = [xsb[:, k * TB:(k + 1) * TB] for k in range(KC)]

    # DRAM bounce buffers for collectives
    rs_in = dram_p.tile([NC * D, TB], bf16, name='rs_in')
    rs_out = dram_p.tile([D, TB], bf16, name='rs_out')
    RG = [list(range(NC))]

    def allgather_norm(src_tiles, gamma_ap, tag):
        """norm src -> bf16 -> ag_in -> AllGather; returns ag_out tile."""
        ag_in = dram_p.tile([D, TB], fp8, name=f'ag_in_{tag}')
        ag_out = dram_p.tile([NC * D, TB], fp8, name=f'ag_out_{tag}',
                             addr_space=('Local' if TLSIM else 'Shared'))
        with tc.tile_pool(name=f'agp_{tag}', bufs=2) as sp:
            xnall = sp.tile([128, KC * TB], fp8, name=f'xn_{tag}')
            outs = [xnall[:, k * TB:(k + 1) * TB] for k in range(KC)]
            _fm_norm(nc, tc, ctx, src_tiles, gamma_ap, TB, outs, tag)
            nc.sync.dma_start(out=_r3(ag_in[:]), in_=_s3(xnall[:]))
        if TLSIM:
            for r in range(NC):
                nc.sync.dma_start(out=ag_out[r * D:(r + 1) * D, :], in_=ag_in[:])
        else:
            nc.gpsimd.collective_compute(
                'AllGather', mybir.AluOpType.bypass, replica_groups=RG,
                ins=[ag_in.opt()], outs=[ag_out.opt()])
        return ag_out

    def reduce_scatter_add(tag):
        """ReduceScatter rs_in -> rs_out; add into xs."""
        if TLSIM:
            nc.sync.dma_start(out=rs_out[:], in_=rs_in[0:D, :])
        else:
            nc.gpsimd.collective_compute(
                'ReduceScatter', mybir.AluOpType.add, replica_groups=RG,
                ins=[rs_in.opt()], outs=[rs_out.opt()])
        with tc.tile_pool(name=f'rsp_{tag}', bufs=2) as sp:
            rt = sp.tile([128, KC * TB], bf16, name=f'rs_{tag}')
            nc.sync.dma_start(out=_s3(rt[:]), in_=_r3(rs_out[:]))
            for k in range(KC):
                nc.vector.tensor_add(xs[k][:], xs[k][:], rt[:, k * TB:(k + 1) * TB])

    def load_xn(sp, xn_buf, tb, tag, bufs=3):
        xnb = sp.tile([128, KC * 512], fp8, name=f'xnl_{tag}', bufs=bufs)
        nc.sync.dma_start(out=_s3(xnb[:]),
                          in_=_r3(xn_buf[tb * D:(tb + 1) * D, :]))
        return [xnb[:, k * 512:(k + 1) * 512] for k in range(KC)]

    # ------------------------------------------------------------------
    # per-layer phases
    # ------------------------------------------------------------------

    def qkv_attention(l, xn_buf, last_layer):
        """Full attention block for layer l. For last_layer, q/attention are
        computed only for the 4 last-position tokens."""
        sqkv = C[f'sqkv{l}']
        with ExitStack() as ph:
            wsp = ph.enter_context(tc.tile_pool(name=f'wq_{l}', bufs=1))
            wq8 = wsp.tile([128, KC * 384], fp8, name=f'wqkv{l}')
            nc.sync.dma_start(out=wq8[:], in_=I[f'wqkv{l}'][:])

            def wq_sl(k, m):
                return wq8[:, k * 384 + m * 128: k * 384 + (m + 1) * 128]

            atp = ph.enter_context(tc.tile_pool(name=f'at_{l}', bufs=1))
            qsb = None
            if not last_layer:
                qsb = [atp.tile([128, T], bf16, name=f'qsb{l}_{m}') for m in range(2)]
            ksb = atp.tile([128, T], bf16, name=f'ksb{l}')
            vt = {}
            for b in range(B):
                for kb in range(8):
                    vt[(b, kb)] = atp.tile([128, 64], bf16, name=f'vt{l}_{b}_{kb}')

            with tc.tile_pool(name=f'qk_{l}', bufs=2) as sp, \
                 tc.tile_pool(name=f'qkp_{l}', bufs=2, space='PSUM') as pp:
                for tb in range(NC):
                    xn = load_xn(sp, xn_buf, tb, f'q{l}', bufs=3)
                    col = tb * 512
                    mlist = [2] if last_layer else [0, 1, 2]
                    for m in mlist:
                        ps = pp.tile([128, 512], f32, name=f'qkvps{l}', bufs=3)
                        for k in range(KC):
                            nc.tensor.matmul(
                                ps[:], wq_sl(k, m), xn[k][:],
                                start=(k == 0), stop=(k == KC - 1))
                        if m < 2:
                            qf = sp.tile([128, 512], f32, name=f'qf{l}')
                            nc.vector.tensor_scalar_mul(qf[:], ps[:], sqkv[:, m:m + 1])
                            qs = sp.tile([128, 512], f32, name=f'qs{l}')
                            for g, src_g in ((0, 1), (1, 0), (2, 3), (3, 2)):
                                eng = nc.scalar if g % 2 == 0 else nc.vector
                                (eng.copy if g % 2 == 0 else eng.tensor_copy)(
                                    qs[g * 32:(g + 1) * 32, :],
                                    qf[src_g * 32:(src_g + 1) * 32, :])
                            m1 = sp.tile([128, 512], f32, name=f'm1{l}')
                            nc.vector.tensor_mul(m1[:], qf[:], C['cosq'][:, col:col + 512])
                            m2 = sp.tile([128, 512], f32, name=f'm2{l}')
                            nc.vector.tensor_mul(m2[:], qs[:], C['sinq'][:, col:col + 512])
                            nc.vector.tensor_add(qsb[m][:, col:col + 512], m1[:], m2[:])
                        else:
                            kf = sp.tile([64, 512], f32, name=f'kf{l}')
                            nc.vector.tensor_scalar_mul(kf[:], ps[0:64, :], sqkv[0:64, 2:3])
                            ks = sp.tile([64, 512], f32, name=f'ks{l}')
                            nc.scalar.copy(ks[0:32, :], kf[32:64, :])
                            nc.scalar.copy(ks[32:64, :], kf[0:32, :])
                            m1k = sp.tile([64, 512], f32, name=f'm1k{l}')
                            nc.vector.tensor_mul(m1k[:], kf[:], C['cosq'][0:64, col:col + 512])
                            m2k = sp.tile([64, 512], f32, name=f'm2k{l}')
                            nc.vector.tensor_mul(m2k[:], ks[:], C['sinq'][0:64, col:col + 512])
                            nc.vector.tensor_add(ksb[0:64, col:col + 512], m1k[:], m2k[:])
                            nc.vector.tensor_add(ksb[64:128, col:col + 512], m1k[:], m2k[:])
                            vf = sp.tile([64, 512], bf16, name=f'vf{l}')
                            nc.vector.tensor_scalar_mul(vf[:], ps[64:128, :], sqkv[64:128, 2:3])
                            b = tb // 2
                            for j in range(4):
                                kb = (tb % 2) * 4 + j
                                vps = pp.tile([128, 64], f32, name=f'vtp{l}', bufs=2)
                                nc.tensor.matmul(vps[:], vf[:, j * 128:(j + 1) * 128],
                                                 C['identbf'][0:64, 0:64],
                                                 start=True, stop=True)
                                nc.scalar.copy(vt[(b, kb)][:], vps[:])

            if last_layer:
                return ksb, vt, wq_sl, ph.pop_all()

            # ---- attention core (layers 0..2) ----
            attnf = [atp.tile([128, T], bf16, name=f'af{l}_{m}') for m in range(2)]
            with tc.tile_pool(name=f'sc_{l}', bufs=2) as sp, \
                 tc.tile_pool(name=f'scp_{l}', bufs=2, space='PSUM') as pp:
                for b in range(B):
                    for h in range(QH):
                        qrows = ((h % 2) * 64, (h % 2) * 64 + 64)
                        qt_tile = qsb[h // 2]
                        psb = []
                        dgs = []
                        for qt in range(8):
                            W = (qt + 1) * 128
                            sps = pp.tile([128, 1024], f32, name=f'sps{l}', bufs=2)
                            for c0 in range(0, W, 512):
                                cw = min(512, W - c0)
                                nc.tensor.matmul(
                                    sps[:, c0:c0 + cw],
                                    qt_tile[qrows[0]:qrows[1],
                                            b * 1024 + qt * 128: b * 1024 + qt * 128 + 128],
                                    ksb[qrows[0]:qrows[1],
                                        b * 1024 + c0: b * 1024 + c0 + cw],
                                    start=True, stop=True)
                            nc.vector.tensor_add(sps[:, qt * 128:W],
                                                 sps[:, qt * 128:W], C['trimask'][:])
                            nm = sp.tile([128, 1], f32, name=f'nm{l}', bufs=3)
                            nc.vector.tensor_reduce(out=nm[:], in_=sps[:, 0:W],
                                                    axis=AX, op=mybir.AluOpType.max,
                                                    negate=True)
                            pt = sp.tile([128, 1024], bf16, name=f'pexp{l}_{qt}')
                            den = sp.tile([128, 1], f32, name=f'den{l}', bufs=3)
                            nc.scalar.activation(pt[:, 0:W], sps[:, 0:W], AF.Exp,
                                                 bias=nm[:], scale=1.0,
                                                 accum_out=den[:])
                            rden = sp.tile([128, 1], f32, name=f'rden{l}', bufs=3)
                            nc.vector.reciprocal(rden[:], den[:])
                            dg = sp.tile([128, 128], bf16, name=f'dg{l}_{qt}')
                            nc.vector.tensor_scalar_mul(dg[:], C['identbf'][:], rden[:])
                            psb.append(pt)
                            dgs.append(dg)
                        for Hh in range(2):
                            pv = pp.tile([64, 512], f32, name=f'pvps{l}', bufs=2)
                            for kb in range(4 * Hh + 4):
                                qt0 = max(kb, 4 * Hh)
                                ptp = pp.tile([128, 512], f32, name=f'ptp{l}', bufs=2)
                                for qt in range(qt0, 4 * Hh + 4):
                                    nc.tensor.matmul(
                                        ptp[:, (qt - 4 * Hh) * 128:(qt - 4 * Hh + 1) * 128],
                                        psb[qt][:, kb * 128:(kb + 1) * 128],
                                        dgs[qt][:], start=True, stop=True)
                                cs = (qt0 - 4 * Hh) * 128
                                pts = sp.tile([128, 512], bf16, name=f'pts{l}', bufs=3)
                                eng = nc.vector if kb % 2 == 0 else nc.scalar
                                (eng.tensor_copy if kb % 2 == 0 else eng.copy)(
                                    pts[:, cs:512], ptp[:, cs:512])
                                nc.tensor.matmul(pv[:, cs:512], vt[(b, kb)][:],
                                                 pts[:, cs:512],
                                                 start=(kb == 0), stop=(kb == 4 * Hh + 3))
                            nc.scalar.copy(
                                attnf[h // 2][(h % 2) * 64:(h % 2) * 64 + 64,
                                              b * 1024 + Hh * 512: b * 1024 + Hh * 512 + 512],
                                pv[:])

            # ---- o-proj ----
            so = C[f'so{l}']
            with tc.tile_pool(name=f'wo_{l}', bufs=1) as wsp2, \
                 tc.tile_pool(name=f'op_{l}', bufs=2) as sp, \
                 tc.tile_pool(name=f'opp_{l}', bufs=3, space='PSUM') as pp:
                wo8 = wsp2.tile([128, 2 * D], fp8, name=f'wo{l}')
                nc.sync.dma_start(out=wo8[:], in_=I[f'wo{l}'][:])
                for tb in range(NC):
                    oball = sp.tile([128, KC * 512], bf16, name=f'ob{l}', bufs=2)
                    for m in range(KC):
                        ops = pp.tile([128, 512], f32, name=f'ops{l}', bufs=3)
                        for kc in range(2):
                            nc.tensor.matmul(
                                ops[:], wo8[:, kc * D + m * 128: kc * D + (m + 1) * 128],
                                attnf[kc][:, tb * 512:(tb + 1) * 512],
                                start=(kc == 0), stop=(kc == 1))
                        ob = oball[:, m * 512:(m + 1) * 512]
                        if m % 2 == 0:
                            nc.vector.tensor_scalar_mul(ob[:], ops[:], so[:, m:m + 1])
                        else:
                            nc.scalar.activation(ob[:], ops[:], AF.Copy,
                                                 scale=so[:, m:m + 1])
                    nc.sync.dma_start(out=_r3(rs_in[tb * D:(tb + 1) * D, :]),
                                      in_=_s3(oball[:]))
        return None

    def mlp(l, xn_buf):
        """MLP block for layers 0..2 (full T tokens)."""
        sg, sud, sd = C[f'sg{l}'], C[f'sud{l}'], C[f'sd{l}']
        with ExitStack() as ph:
            wsp = ph.enter_context(tc.tile_pool(name=f'wm_{l}', bufs=1))
            wg8 = wsp.tile([128, KC * DFFS], fp8, name=f'wg{l}')
            nc.sync.dma_start(out=wg8[:], in_=I[f'wg{l}'][:])
            wu8 = wsp.tile([128, KC * DFFS], fp8, name=f'wu{l}')
            nc.sync.dma_start(out=wu8[:], in_=I[f'wu{l}'][:])
            wd8 = wsp.tile([128, MFF * D], fp8, name=f'wd{l}')
            nc.sync.dma_start(out=wd8[:], in_=I[f'wd{l}'][:])
            with tc.tile_pool(name=f'ml_{l}', bufs=2) as sp, \
                 tc.tile_pool(name=f'mlp_{l}', bufs=2, space='PSUM') as pp:
                for tb in range(NC):
                    xn = load_xn(sp, xn_buf, tb, f'm{l}', bufs=2)
                    hmall = sp.tile([128, MFF * 512], bf16, name=f'hm{l}', bufs=2)
                    for mf in range(MFF):
                        gps = pp.tile([128, 512], f32, name=f'gps{l}', bufs=2)
                        for k in range(KC):
                            nc.tensor.matmul(
                                gps[:], wg8[:, k * DFFS + mf * 128: k * DFFS + (mf + 1) * 128],
                                xn[k][:], start=(k == 0), stop=(k == KC - 1))
                        ups = pp.tile([128, 512], f32, name=f'ups{l}', bufs=2)
                        for k in range(KC):
                            nc.tensor.matmul(
                                ups[:], wu8[:, k * DFFS + mf * 128: k * DFFS + (mf + 1) * 128],
                                xn[k][:], start=(k == 0), stop=(k == KC - 1))
                        gsb = sp.tile([128, 512], bf16, name=f'gsb{l}', bufs=2)
                        nc.scalar.activation(gsb[:], gps[:], AF.Silu,
                                             scale=sg[:, mf:mf + 1])
                        nc.vector.scalar_tensor_tensor(
                            out=hmall[:, mf * 512:(mf + 1) * 512], in0=ups[:],
                            scalar=sud[:, mf:mf + 1], in1=gsb[:], op0=MUL, op1=MUL)
                    dball = sp.tile([128, KC * 512], bf16, name=f'db{l}', bufs=2)
                    for mo in range(KC):
                        dps = pp.tile([128, 512], f32, name=f'dps{l}', bufs=3)
                        for k in range(MFF):
                            nc.tensor.matmul(
                                dps[:], wd8[:, k * D + mo * 128: k * D + (mo + 1) * 128],
                                hmall[:, k * 512:(k + 1) * 512],
                                start=(k == 0), stop=(k == MFF - 1))
                        db = dball[:, mo * 512:(mo + 1) * 512]
                        if mo % 2 == 0:
                            nc.vector.tensor_scalar_mul(db[:], dps[:], sd[:, mo:mo + 1])
                        else:
                            nc.scalar.activation(db[:], dps[:], AF.Copy,
                                                 scale=sd[:, mo:mo + 1])
                    nc.sync.dma_start(out=_r3(rs_in[tb * D:(tb + 1) * D, :]),
                                      in_=_s3(dball[:]))

    # ------------------------------------------------------------------
    # layers 0..2
    # ------------------------------------------------------------------
    for l in range(L - 1):
        agb = allgather_norm(xs, C[f'ga{l}'][:], f'a{l}')
        qkv_attention(l, agb, last_layer=False)
        reduce_scatter_add(f'o{l}')
        agb = allgather_norm(xs, C[f'gm{l}'][:], f'm{l}')
        mlp(l, agb)
        reduce_scatter_add(f'd{l}')

    # ------------------------------------------------------------------
    # layer 3 (last): only last-position tokens through q/attn/mlp/head
    # ------------------------------------------------------------------
    l = L - 1
    lx_in = dram_p.tile([D, 1], f32, name='lx_in')
    lx_out = dram_p.tile([NC * D, 1], f32, name='lx_out',
                         addr_space=('Local' if TLSIM else 'Shared'))
    ar_in = dram_p.tile([D, 4], f32, name='ar_in')
    ar_out = dram_p.tile([D, 4], f32, name='ar_out',
                         addr_space=('Local' if TLSIM else 'Shared'))
    ar2_in = dram_p.tile([D, 4], f32, name='ar2_in')
    ar2_out = dram_p.tile([D, 4], f32, name='ar2_out',
                          addr_space=('Local' if TLSIM else 'Shared'))

    for k in range(KC):
        nc.sync.dma_start(out=lx_in[k * 128:(k + 1) * 128, :],
                          in_=xs[k][:, 511:512])
    if TLSIM:
        for r in range(NC):
            nc.sync.dma_start(out=lx_out[r * D:(r + 1) * D, :], in_=lx_in[:])
    else:
        nc.gpsimd.collective_compute('AllGather', mybir.AluOpType.bypass,
                                     replica_groups=RG, ins=[lx_in.opt()],
                                     outs=[lx_out.opt()])
    l4p = top.enter_context(tc.tile_pool(name='l4p', bufs=1))
    lastx = []
    for k in range(KC):
        t = l4p.tile([128, 4], f32, name=f'lastx{k}')
        src = bass.AP(tensor=lx_out.tensor, offset=lx_out[:].offset + D + k * 128,
                      ap=[[1, 128], [2 * D, 4]])
        nc.sync.dma_start(out=t[:], in_=src)
        lastx.append(t)

    # norm for q (on last-position tokens)
    qn4 = [l4p.tile([128, 4], bf16, name=f'qn4_{k}') for k in range(KC)]
    _fm_norm(nc, tc, ctx, lastx, C[f'ga{l}'][:], 4, qn4, 'q4')

    # full norm + AG for k/v
    agb = allgather_norm(xs, C[f'ga{l}'][:], f'a{l}')
    ksb, vt, wq_sl, wctx = qkv_attention(l, agb, last_layer=True)

    sqkv = C[f'sqkv{l}']
    q4h = [l4p.tile([64, 4], bf16, name=f'q4h_{h}') for h in range(QH)]
    at4 = [l4p.tile([64, 4], bf16, name=f'at4_{h}') for h in range(QH)]
    with tc.tile_pool(name='l4qs', bufs=2) as sp, \
         tc.tile_pool(name='l4qp', bufs=2, space='PSUM') as pp:
        for m in range(2):
            ps = pp.tile([128, 4], f32, name='q4ps', bufs=2)
            for k in range(KC):
                nc.tensor.matmul(ps[:], wq_sl(k, m), qn4[k][:],
                                 start=(k == 0), stop=(k == KC - 1))
            qf = sp.tile([128, 4], f32, name='q4f')
            nc.vector.tensor_scalar_mul(qf[:], ps[:], sqkv[:, m:m + 1])
            qs = sp.tile([128, 4], f32, name='q4s')
            for g, src_g in ((0, 1), (1, 0), (2, 3), (3, 2)):
                nc.vector.tensor_copy(qs[g * 32:(g + 1) * 32, :],
                                      qf[src_g * 32:(src_g + 1) * 32, :])
            m1 = sp.tile([128, 4], f32, name='q4m1')
            nc.vector.tensor_mul(m1[:], qf[:], C['cosq4'][:])
            m2 = sp.tile([128, 4], f32, name='q4m2')
            nc.vector.tensor_mul(m2[:], qs[:], C['sinq4'][:])
            for sub in range(2):
                nc.vector.tensor_add(q4h[m * 2 + sub][:],
                                     m1[sub * 64:(sub + 1) * 64, :],
                                     m2[sub * 64:(sub + 1) * 64, :])

    # attention for 4 last tokens
    with tc.tile_pool(name='l4as', bufs=2) as sp, \
         tc.tile_pool(name='l4ap', bufs=1, space='PSUM') as pp:
        for b in range(B):
            for h in range(QH):
                s4 = pp.tile([1, 1024], f32, name='s4ps', bufs=2)
                for c0 in range(0, 1024, 512):
                    nc.tensor.matmul(s4[:, c0:c0 + 512],
                                     q4h[h][:, b:b + 1],
                                     ksb[0:64, b * 1024 + c0: b * 1024 + c0 + 512],
                                     start=True, stop=True)
                nm = sp.tile([1, 1], f32, name='nm4', bufs=3)
                nc.vector.tensor_reduce(out=nm[:], in_=s4[:], axis=AX,
                                        op=mybir.AluOpType.max, negate=True)
                p4 = sp.tile([1, 1024], bf16, name='p4', bufs=2)
                den = sp.tile([1, 1], f32, name='den4', bufs=3)
                nc.scalar.activation(p4[:], s4[:], AF.Exp, bias=nm[:], scale=1.0,
                                     accum_out=den[:])
                rden = sp.tile([1, 1], f32, name='rden4', bufs=3)
                nc.vector.reciprocal(rden[:], den[:])
                rbcp = pp.tile([128, 1], f32, name='rbcp', bufs=1)
                nc.tensor.matmul(rbcp[:], C['ones_m'][:], rden[:], start=True, stop=True)
                rbc = sp.tile([128, 1], f32, name='rbc4', bufs=3)
                nc.scalar.copy(rbc[:], rbcp[:])
                pt4p = pp.tile([128, 8], f32, name='pt4p', bufs=1)
                for kb in range(8):
                    nc.tensor.matmul(pt4p[:, kb:kb + 1], p4[:, kb * 128:(kb + 1) * 128],
                                     C['onebf'][:], start=True, stop=True)
                pt4 = sp.tile([128, 8], bf16, name='pt4', bufs=2)
                nc.vector.tensor_scalar_mul(pt4[:], pt4p[:], rbc[:])
                pv4 = pp.tile([64, 1], f32, name='pv4', bufs=2)
                for kb in range(8):
                    nc.tensor.matmul(pv4[:], vt[(b, kb)][:], pt4[:, kb:kb + 1],
                                     start=(kb == 0), stop=(kb == 7))
                nc.scalar.copy(at4[h][:, b:b + 1], pv4[:])
    wctx.close()

    # o-proj for 4 tokens
    so = C[f'so{l}']
    with tc.tile_pool(name='wo3', bufs=1) as wsp2, \
         tc.tile_pool(name='l4os', bufs=2) as sp, \
         tc.tile_pool(name='l4op', bufs=2, space='PSUM') as pp:
        wo4h = []
        for h in range(QH):
            r0 = (h % 2) * 64
            wt = wsp2.tile([64, D], fp8, name=f'wo4t_{h}')
            nc.sync.dma_start(out=wt[:],
                              in_=I[f'wo{l}'][r0:r0 + 64, (h // 2) * D:(h // 2 + 1) * D])
            wo4h.append(wt)
        for m in range(KC):
            ops = pp.tile([128, 4], f32, name='o4ps', bufs=2)
            for h in range(QH):
                nc.tensor.matmul(
                    ops[:], wo4h[h][:, m * 128:(m + 1) * 128],
                    at4[h][:], start=(h == 0), stop=(h == QH - 1))
            ob = sp.tile([128, 4], f32, name='o4b', bufs=3)
            nc.vector.tensor_scalar_mul(ob[:], ops[:], so[:, m:m + 1])
            nc.sync.dma_start(out=ar_in[m * 128:(m + 1) * 128, :], in_=ob[:])

    if TLSIM:
        nc.sync.dma_start(out=ar_out[:], in_=ar_in[:])
    else:
        nc.gpsimd.collective_compute('AllReduce', mybir.AluOpType.add,
                                     replica_groups=RG, ins=[ar_in.opt()],
                                     outs=[ar_out.opt()])

    # residual add (4 tokens)
    x4 = []
    with tc.tile_pool(name='l4r', bufs=3) as sp:
        for k in range(KC):
            rt = sp.tile([128, 4], f32, name='ar4l')
            nc.sync.dma_start(out=rt[:], in_=ar_out[k * 128:(k + 1) * 128, :])
            t = l4p.tile([128, 4], f32, name=f'x4_{k}')
            nc.vector.tensor_add(t[:], lastx[k][:], rt[:])
            x4.append(t)

    # norm2 + tiny MLP
    xn4 = [l4p.tile([128, 4], bf16, name=f'xn4_{k}') for k in range(KC)]
    _fm_norm(nc, tc, ctx, x4, C[f'gm{l}'][:], 4, xn4, 'm4')
    sg, sud, sd = C[f'sg{l}'], C[f'sud{l}'], C[f'sd{l}']
    with ExitStack() as ph:
        wsp = ph.enter_context(tc.tile_pool(name='wm3', bufs=1))
        wg8 = wsp.tile([128, KC * DFFS], fp8, name='wg3t')
        nc.sync.dma_start(out=wg8[:], in_=I[f'wg{l}'][:])
        wu8 = wsp.tile([128, KC * DFFS], fp8, name='wu3t')
        nc.sync.dma_start(out=wu8[:], in_=I[f'wu{l}'][:])
        wd8 = wsp.tile([128, MFF * D], fp8, name='wd3t')
        nc.sync.dma_start(out=wd8[:], in_=I[f'wd{l}'][:])
        with tc.tile_pool(name='m4s', bufs=2) as sp, \
             tc.tile_pool(name='m4p', bufs=2, space='PSUM') as pp:
            hm = []
            for mf in range(MFF):
                gps = pp.tile([128, 4], f32, name='g4ps', bufs=2)
                for k in range(KC):
                    nc.tensor.matmul(
                        gps[:], wg8[:, k * DFFS + mf * 128: k * DFFS + (mf + 1) * 128],
                        xn4[k][:], start=(k == 0), stop=(k == KC - 1))
                ups = pp.tile([128, 4], f32, name='u4ps', bufs=2)
                for k in range(KC):
                    nc.tensor.matmul(
                        ups[:], wu8[:, k * DFFS + mf * 128: k * DFFS + (mf + 1) * 128],
                        xn4[k][:], start=(k == 0), stop=(k == KC - 1))
                gsb = sp.tile([128, 4], bf16, name='g4sb', bufs=3)
                nc.scalar.activation(gsb[:], gps[:], AF.Silu, scale=sg[:, mf:mf + 1])
                ht = sp.tile([128, 4], bf16, name=f'h4_{mf}')
                nc.vector.scalar_tensor_tensor(
                    out=ht[:], in0=ups[:], scalar=sud[:, mf:mf + 1],
                    in1=gsb[:], op0=MUL, op1=MUL)
                hm.append(ht)
            for mo in range(KC):
                dps = pp.tile([128, 4], f32, name='d4ps', bufs=2)
                for k in range(MFF):
                    nc.tensor.matmul(
                        dps[:], wd8[:, k * D + mo * 128: k * D + (mo + 1) * 128],
                        hm[k][:], start=(k == 0), stop=(k == MFF - 1))
                db = sp.tile([128, 4], f32, name='d4b', bufs=3)
                nc.vector.tensor_scalar_mul(db[:], dps[:], sd[:, mo:mo + 1])
                nc.sync.dma_start(out=ar2_in[mo * 128:(mo + 1) * 128, :], in_=db[:])

    if TLSIM:
        nc.sync.dma_start(out=ar2_out[:], in_=ar2_in[:])
    else:
        nc.gpsimd.collective_compute('AllReduce', mybir.AluOpType.add,
                                     replica_groups=RG, ins=[ar2_in.opt()],
                                     outs=[ar2_out.opt()])

    # final residual + final norm + LM head
    with tc.tile_pool(name='fhs', bufs=2) as sp, \
         tc.tile_pool(name='fhp', bufs=2, space='PSUM') as pp:
        xf = []
        for k in range(KC):
            rt = sp.tile([128, 4], f32, name='ar4l2', bufs=3)
            nc.sync.dma_start(out=rt[:], in_=ar2_out[k * 128:(k + 1) * 128, :])
            t = l4p.tile([128, 4], f32, name=f'xf_{k}')
            nc.vector.tensor_add(t[:], x4[k][:], rt[:])
            xf.append(t)
        xfn = [l4p.tile([128, 4], bf16, name=f'xfn_{k}') for k in range(KC)]
        _fm_norm(nc, tc, ctx, xf, C['gf'][:], 4, xfn, 'f4')
        nch = (VS + 511) // 512
        for n in range(nch):
            cw = min(512, VS - n * 512)
            hps = pp.tile([4, 512], f32, name='hps', bufs=2)
            et = sp.tile([128, KC * 512], bf16, name='embt', bufs=3)
            nc.sync.dma_start(
                out=et[:].rearrange("p (k c) -> p k c", k=KC)[:, :, 0:cw],
                in_=_r3(I['embT'][:, n * 512:n * 512 + cw]))
            for k in range(KC):
                nc.tensor.matmul(hps[:, 0:cw], xfn[k][:],
                                 et[:, k * 512:k * 512 + cw],
                                 start=(k == 0), stop=(k == KC - 1))
            lsb = sp.tile([4, 512], f32, name='lsb', bufs=3)
            nc.scalar.copy(lsb[:, 0:cw], hps[:, 0:cw])
            nc.sync.dma_start(out=logits_out[:, n * 512:n * 512 + cw],
                              in_=lsb[:, 0:cw])


# ----------------------------------------------------------------------------
# host-side prep
# ----------------------------------------------------------------------------

def _chunk_pack(a, nchunks):
    """[(nchunks*128), cols] -> [128, nchunks*cols] fp8 (chunk-major in free dim)."""
    cols = a.shape[1]
    return np.ascontiguousarray(
        a.reshape(nchunks, 128, cols).transpose(1, 0, 2).reshape(128, nchunks * cols)
        .astype(ml_dtypes.float8_e4m3))


def _prep_in_maps(token_ids, embed, gamma_attn, gamma_mlp, gamma_final,
                  wq, sq, wk, sk, wv, sv, wo, so, wg, sg, wu, su, wd, sd):
    half = HD // 2
    inv = ROPE_THETA ** (-np.arange(half, dtype=np.float32) * 2.0 / HD)
    ang = np.arange(S, dtype=np.float32)[:, None] * inv          # [S, 32]
    cos1 = np.cos(ang).T.astype(np.float32)                      # [32, S]
    sin1 = np.sin(ang).T.astype(np.float32)
    cos64 = np.concatenate([cos1, cos1], 0)                      # [64, S]
    sin64s = np.concatenate([-sin1, sin1], 0)
    cosq = np.tile(np.concatenate([cos64, cos64], 0), (1, B))    # [128, T]
    sinq = np.tile(np.concatenate([sin64s, sin64s], 0), (1, B))
    cosq4 = np.repeat(cosq[:, S - 1:S], 4, axis=1).copy()
    sinq4 = np.repeat(sinq[:, S - 1:S], 4, axis=1).copy()

    ii, jj = np.meshgrid(np.arange(128), np.arange(128), indexing='ij')
    trimask = np.where(jj <= ii, 0.0, NEG).astype(np.float32)

    tok = np.asarray(token_ids).reshape(T)
    x0full = np.ascontiguousarray(embed[tok].T.astype(np.float32))  # [D, T]
    embT = np.ascontiguousarray(embed.T.astype(ml_dtypes.bfloat16))  # [D, V]

    def percol(a):
        return np.ascontiguousarray(a.reshape(-1, 128).T.astype(np.float32))

    common = {
        'cosq': np.ascontiguousarray(cosq.astype(ml_dtypes.bfloat16)),
        'sinq': np.ascontiguousarray(sinq.astype(ml_dtypes.bfloat16)),
        'cosq4': cosq4, 'sinq4': sinq4, 'trimask': trimask,
        'identbf': np.eye(128, dtype=ml_dtypes.bfloat16),
        'ones_k': np.ones((128, 1), np.float32),
        'ones_m': np.ones((1, 128), np.float32),
        'onebf': np.ones((1, 1), ml_dtypes.bfloat16),
        'gf': percol(gamma_final),
    }
    in_maps = []
    for c in range(NC):
        m = dict(common)
        m['x0'] = np.ascontiguousarray(x0full[:, c * TB:(c + 1) * TB])
        m['embT'] = np.ascontiguousarray(embT[:, c * VS:(c + 1) * VS])
        for l in range(L):
            qsl = slice(c * DQ, (c + 1) * DQ)
            ksl = slice(c * HD, (c + 1) * HD)
            fsl = slice(c * DFFS, (c + 1) * DFFS)
            m[f'wqkv{l}'] = _chunk_pack(np.concatenate(
                [wq[l][qsl].T, wk[l][ksl].T, wv[l][ksl].T], axis=1), KC)
            sq_l = sq[l][qsl] * np.float32(1.0 / np.sqrt(HD))
            m[f'sqkv{l}'] = np.ascontiguousarray(np.stack(
                [sq_l[0:128], sq_l[128:256],
                 np.concatenate([sk[l][ksl], sv[l][ksl]])], axis=1).astype(np.float32))
            m[f'wo{l}'] = _chunk_pack(wo[l][:, qsl].T, 2)
            m[f'so{l}'] = percol(so[l])
            m[f'wg{l}'] = _chunk_pack(wg[l][fsl].T, KC)
            m[f'sg{l}'] = percol(sg[l][fsl])
            m[f'wu{l}'] = _chunk_pack(wu[l][fsl].T, KC)
            m[f'wd{l}'] = _chunk_pack(wd[l][:, fsl].T, MFF)
            m[f'sud{l}'] = percol(su[l][fsl])
            m[f'sd{l}'] = percol(sd[l])
            m[f'ga{l}'] = percol(gamma_attn[l])
            m[f'gm{l}'] = percol(gamma_mlp[l])
        in_maps.append(m)
    return in_maps


def _get_nc():
    if 'nc' not in _CACHE:
        _CACHE['nc'] = _build()
    return _CACHE['nc']


def kernel(**inputs) -> np.ndarray:
    inputs = {k: np.asarray(v) for k, v in inputs.items()}
    in_maps = _prep_in_maps(**inputs)
    nc = _get_nc()
    res = bass_utils.run_bass_kernel_spmd(nc, in_maps, core_ids=list(range(NC)))
    logits = np.concatenate([res.results[c]['logits'] for c in range(NC)], axis=1)
    return logits.astype(np.float32)


# revision 8
# speedup vs baseline: 13.2871x; 1.1077x over previous
"""Self-contained Trainium2 Bass kernel for the int4-quantized 4-layer Llama decode problem.

Strategy: tensor-parallel over 8 NeuronCores (attention heads + FFN hidden dim),
sequence-parallel residual (each core keeps a feature-major fp32 residual shard
[D, T/8] in SBUF), AllGather before QKV/MLP, ReduceScatter after o-proj/down-proj.
Weights are host-packed to fp8e4 (int4 values are exact in e4m3) and used directly
as the stationary matmul operand against bf16 activations; dequant scales are
applied to the matmul outputs (per-out-channel) or to the activations (down-proj
input channels). Activation/residual traffic between SBUF and the DRAM collective
bounce buffers moves in single strided DMAs per 2048-feature block.
Only the last position of each sequence goes through layer-4 Q/attention/MLP and
the LM head.
"""
import sys

sys.path.insert(0, '/opt/trn_rl_repo')

import numpy as np
import ml_dtypes
from contextlib import ExitStack

import concourse.bass as bass
import concourse.tile as tile
from concourse import bacc, mybir
from concourse import bass_utils

# model dims (hardcoded per problem spec)
L, D, H, HD, KVH, DFF, V, B, S = 4, 2048, 32, 64, 8, 8192, 32000, 4, 1024
NC = 8
T = B * S              # 4096 tokens
TB = T // NC           # 512 tokens per core shard
QH = H // NC           # 4 local q heads
DQ = QH * HD           # 256 local q dims
DFFS = DFF // NC       # 1024 local ffn dims
VS = V // NC           # 4000 local vocab
KC = D // 128          # 16 feature chunks
MFF = DFFS // 128      # 8
ROPE_THETA = 500000.0
NEG = np.float32(-1e9)
EPS = 1e-5

f32 = mybir.dt.float32
bf16 = mybir.dt.bfloat16
fp8 = mybir.dt.float8e4
i32 = mybir.dt.int32

AX = mybir.AxisListType.X
MUL = mybir.AluOpType.mult
AF = mybir.ActivationFunctionType

_CACHE = {}
TLSIM = False  # single-core cost-model sim mode (collectives -> DMA copies)


# ----------------------------------------------------------------------------
# bass program
# ----------------------------------------------------------------------------

def _declare_inputs(nc):
    I = {}
    I['x0'] = nc.dram_tensor('x0', [D, TB], f32, kind='ExternalInput').ap()
    for l in range(L):
        I[f'wqkv{l}'] = nc.dram_tensor(f'wqkv{l}', [128, KC * 384], fp8, kind='ExternalInput').ap()
        I[f'sqkv{l}'] = nc.dram_tensor(f'sqkv{l}', [128, 3], f32, kind='ExternalInput').ap()
        I[f'wo{l}'] = nc.dram_tensor(f'wo{l}', [128, 2 * D], fp8, kind='ExternalInput').ap()
        I[f'so{l}'] = nc.dram_tensor(f'so{l}', [128, KC], f32, kind='ExternalInput').ap()
        I[f'wg{l}'] = nc.dram_tensor(f'wg{l}', [128, KC * DFFS], fp8, kind='ExternalInput').ap()
        I[f'sg{l}'] = nc.dram_tensor(f'sg{l}', [128, MFF], f32, kind='ExternalInput').ap()
        I[f'wu{l}'] = nc.dram_tensor(f'wu{l}', [128, KC * DFFS], fp8, kind='ExternalInput').ap()
        I[f'wd{l}'] = nc.dram_tensor(f'wd{l}', [128, MFF * D], fp8, kind='ExternalInput').ap()
        I[f'sud{l}'] = nc.dram_tensor(f'sud{l}', [128, MFF], f32, kind='ExternalInput').ap()
        I[f'sd{l}'] = nc.dram_tensor(f'sd{l}', [128, KC], f32, kind='ExternalInput').ap()
        I[f'ga{l}'] = nc.dram_tensor(f'ga{l}', [128, KC], f32, kind='ExternalInput').ap()
        I[f'gm{l}'] = nc.dram_tensor(f'gm{l}', [128, KC], f32, kind='ExternalInput').ap()
    I['gf'] = nc.dram_tensor('gf', [128, KC], f32, kind='ExternalInput').ap()
    I['cosq'] = nc.dram_tensor('cosq', [128, T], bf16, kind='ExternalInput').ap()
    I['sinq'] = nc.dram_tensor('sinq', [128, T], bf16, kind='ExternalInput').ap()
    I['cosq4'] = nc.dram_tensor('cosq4', [128, 4], f32, kind='ExternalInput').ap()
    I['sinq4'] = nc.dram_tensor('sinq4', [128, 4], f32, kind='ExternalInput').ap()
    I['trimask'] = nc.dram_tensor('trimask', [128, 128], f32, kind='ExternalInput').ap()
    I['identbf'] = nc.dram_tensor('identbf', [128, 128], bf16, kind='ExternalInput').ap()
    I['ones_k'] = nc.dram_tensor('ones_k', [128, 1], f32, kind='ExternalInput').ap()
    I['ones_m'] = nc.dram_tensor('ones_m', [1, 128], f32, kind='ExternalInput').ap()
    I['onebf'] = nc.dram_tensor('onebf', [1, 1], bf16, kind='ExternalInput').ap()
    I['embT'] = nc.dram_tensor('embT', [D, VS], bf16, kind='ExternalInput').ap()
    return I


def _fm_norm(nc, tc, ctx, src, gamma_ap, width, out_tiles, tag):
    """Feature-major rmsnorm: src = list of KC sbuf [128,width] f32 APs.
    Writes out_tiles (KC APs, caller-allocated, any dtype)."""
    with tc.tile_pool(name=f'np_{tag}', bufs=2) as sp, \
         tc.tile_pool(name=f'npp_{tag}', bufs=2, space='PSUM') as pp:
        C = ctx['const']
        ssum = pp.tile([1, width], f32, name=f'nsum_{tag}')
        for k in range(KC):
            xsq = sp.tile([128, width], f32, name=f'nxsq_{tag}', bufs=3)
            nc.vector.tensor_mul(xsq[:], src[k][:], src[k][:])
            nc.tensor.matmul(ssum[:], C['ones_k'][:], xsq[:],
                             start=(k == 0), stop=(k == KC - 1))
        sq = sp.tile([1, width], f32, name=f'nsq_{tag}')
        nc.scalar.activation(sq[:], ssum[:], AF.Sqrt, bias=ctx['eps'][0:1, :],
                             scale=1.0 / D)
        rstd = sp.tile([1, width], f32, name=f'nrstd_{tag}')
        nc.vector.reciprocal(rstd[:], sq[:])
        bcp = pp.tile([128, width], f32, name=f'nbc_{tag}')
        nc.tensor.matmul(bcp[:], C['ones_m'][:], rstd[:], start=True, stop=True)
        rbc = sp.tile([128, width], f32, name=f'nrbc_{tag}')
        nc.scalar.copy(rbc[:], bcp[:])
        for k in range(KC):
            nc.vector.scalar_tensor_tensor(
                out=out_tiles[k][:], in0=src[k][:], scalar=gamma_ap[:, k:k + 1],
                in1=rbc[:], op0=MUL, op1=MUL)


def _r3(dram_ap, nchunks=KC):
    """[(k p), c] DRAM slice -> [p, k, c] AP for batched strided DMA."""
    return dram_ap.rearrange("(k p) c -> p k c", k=nchunks)


def _s3(sb_ap, nchunks=KC):
    """[p, (k c)] SBUF tile -> [p, k, c] AP."""
    return sb_ap.rearrange("p (k c) -> p k c", k=nchunks)


def _build(reps=1):
    nc = bacc.Bacc('TRN2', target_bir_lowering=False, debug=False,
                   num_devices=(1 if TLSIM else NC))
    I = _declare_inputs(nc)
    logits_out = nc.dram_tensor('logits', [4, VS], f32, kind='ExternalOutput').ap()

    with tile.TileContext(nc) as tc, ExitStack() as top:
        const_p = top.enter_context(tc.tile_pool(name='constp', bufs=1))
        resid_p = top.enter_context(tc.tile_pool(name='residp', bufs=1))
        dram_p = top.enter_context(tc.tile_pool(name='dramp', bufs=1, space='DRAM'))

        C = {}
        for cn, shape, dt in [('cosq', [128, T], bf16), ('sinq', [128, T], bf16),
                              ('cosq4', [128, 4], f32), ('sinq4', [128, 4], f32),
                              ('trimask', [128, 128], f32), ('identbf', [128, 128], bf16),
                              ('ones_k', [128, 1], f32), ('ones_m', [1, 128], f32),
                              ('onebf', [1, 1], bf16), ('gf', [128, KC], f32)]:
            t = const_p.tile(shape, dt, name=f'c_{cn}')
            nc.sync.dma_start(out=t[:], in_=I[cn][:])
            C[cn] = t
        for l in range(L):
            for cn in ('sqkv', 'so', 'sg', 'sud', 'sd', 'ga', 'gm'):
                shp = [128, {'sqkv': 3, 'so': KC, 'sg': MFF, 'sud': MFF,
                             'sd': KC, 'ga': KC, 'gm': KC}[cn]]
                t = const_p.tile(shp, f32, name=f'c_{cn}{l}')
                nc.sync.dma_start(out=t[:], in_=I[f'{cn}{l}'][:])
                C[f'{cn}{l}'] = t
        epst = const_p.tile([128, 1], f32, name='c_eps')
        nc.vector.memset(epst[:], EPS)
        ctx = {'const': C, 'eps': epst}

        for _rep in range(reps):
            _body(nc, tc, top, I, C, ctx, dram_p, resid_p, logits_out)

    nc.compile()
    return nc


def _body(nc, tc, top, I, C, ctx, dram_p, resid_p, logits_out):
    # persistent residual shard [D, TB] fp32 as one [128, KC*TB] tile
    xsb = resid_p.tile([128, KC * TB], f32, name='xsh')
    nc.sync.dma_start(out=_s3(xsb[:]), in_=_r3(I['x0'][:]))
    xs = [xsb[:, k * TB:(k + 1) * TB] for k in range(KC)]

    # DRAM bounce buffers for collectives
    rs_in = dram_p.tile([NC * D, TB], bf16, name='rs_in')
    rs_out = dram_p.tile([D, TB], bf16, name='rs_out')
    RG = [list(range(NC))]

    def allgather_norm(src_tiles, gamma_ap, tag):
        """norm src -> bf16 -> ag_in -> AllGather; returns ag_out tile."""
        ag_in = dram_p.tile([D, TB], fp8, name=f'ag_in_{tag}')
        ag_out = dram_p.tile([NC * D, TB], fp8, name=f'ag_out_{tag}',
                             addr_space=('Local' if TLSIM else 'Shared'))
        with tc.tile_pool(name=f'agp_{tag}', bufs=2) as sp:
            xnall = sp.tile([128, KC * TB], fp8, name=f'xn_{tag}')
            outs = [xnall[:, k * TB:(k + 1) * TB] for k in range(KC)]
            _fm_norm(nc, tc, ctx, src_tiles, gamma_ap, TB, outs, tag)
            nc.sync.dma_start(out=_r3(ag_in[:]), in_=_s3(xnall[:]))
        if TLSIM:
            for r in range(NC):
                nc.sync.dma_start(out=ag_out[r * D:(r + 1) * D, :], in_=ag_in[:])
        else:
            nc.gpsimd.collective_compute(
                'AllGather', mybir.AluOpType.bypass, replica_groups=RG,
                ins=[ag_in.opt()], outs=[ag_out.opt()])
        return ag_out

    def reduce_scatter_add(tag):
        """ReduceScatter rs_in -> rs_out; add into xs."""
        if TLSIM:
            nc.sync.dma_start(out=rs_out[:], in_=rs_in[0:D, :])
        else:
            nc.gpsimd.collective_compute(
                'ReduceScatter', mybir.AluOpType.add, replica_groups=RG,
                ins=[rs_in.opt()], outs=[rs_out.opt()])
        with tc.tile_pool(name=f'rsp_{tag}', bufs=2) as sp:
            rt = sp.tile([128, KC * TB], bf16, name=f'rs_{tag}')
            nc.sync.dma_start(out=_s3(rt[:]), in_=_r3(rs_out[:]))
            for k in range(KC):
                nc.vector.tensor_add(xs[k][:], xs[k][:], rt[:, k * TB:(k + 1) * TB])

    def load_xn(sp, xn_buf, tb, tag, bufs=3):
        xnb = sp.tile([128, KC * 512], fp8, name=f'xnl_{tag}', bufs=bufs)
        nc.sync.dma_start(out=_s3(xnb[:]),
                          in_=_r3(xn_buf[tb * D:(tb + 1) * D, :]))
        return [xnb[:, k * 512:(k + 1) * 512] for k in range(KC)]

    # ------------------------------------------------------------------
    # per-layer phases
    # ------------------------------------------------------------------

    def qkv_attention(l, xn_buf, last_layer):
        """Full attention block for layer l. For last_layer, q/attention are
        computed only for the 4 last-position tokens."""
        sqkv = C[f'sqkv{l}']
        with ExitStack() as ph:
            wsp = ph.enter_context(tc.tile_pool(name=f'wq_{l}', bufs=1))
            wq8 = wsp.tile([128, KC * 384], fp8, name=f'wqkv{l}')
            nc.sync.dma_start(out=wq8[:], in_=I[f'wqkv{l}'][:])

            def wq_sl(k, m):
                return wq8[:, k * 384 + m * 128: k * 384 + (m + 1) * 128]

            atp = ph.enter_context(tc.tile_pool(name=f'at_{l}', bufs=1))
            qsb = None
            if not last_layer:
                qsb = [atp.tile([128, T], bf16, name=f'qsb{l}_{m}') for m in range(2)]
            ksb = atp.tile([128, T], bf16, name=f'ksb{l}')
            vt = {}
            for b in range(B):
                for kb in range(8):
                    vt[(b, kb)] = atp.tile([128, 64], bf16, name=f'vt{l}_{b}_{kb}')

            with tc.tile_pool(name=f'qk_{l}', bufs=2) as sp, \
                 tc.tile_pool(name=f'qkp_{l}', bufs=2, space='PSUM') as pp:
                for tb in range(NC):
                    xn = load_xn(sp, xn_buf, tb, f'q{l}', bufs=3)
                    col = tb * 512
                    mlist = [2] if last_layer else [0, 1, 2]
                    for m in mlist:
                        ps = pp.tile([128, 512], f32, name=f'qkvps{l}', bufs=3)
                        for k in range(KC):
                            nc.tensor.matmul(
                                ps[:], wq_sl(k, m), xn[k][:],
                                start=(k == 0), stop=(k == KC - 1))
                        if m < 2:
                            qf = sp.tile([128, 512], f32, name=f'qf{l}')
                            nc.vector.tensor_scalar_mul(qf[:], ps[:], sqkv[:, m:m + 1])
                            qs = sp.tile([128, 512], f32, name=f'qs{l}')
                            for g, src_g in ((0, 1), (1, 0), (2, 3), (3, 2)):
                                eng = nc.scalar if g % 2 == 0 else nc.vector
                                (eng.copy if g % 2 == 0 else eng.tensor_copy)(
                                    qs[g * 32:(g + 1) * 32, :],
                                    qf[src_g * 32:(src_g + 1) * 32, :])
                            m1 = sp.tile([128, 512], f32, name=f'm1{l}')
                            nc.vector.tensor_mul(m1[:], qf[:], C['cosq'][:, col:col + 512])
                            m2 = sp.tile([128, 512], f32, name=f'm2{l}')
                            nc.vector.tensor_mul(m2[:], qs[:], C['sinq'][:, col:col + 512])
                            nc.vector.tensor_add(qsb[m][:, col:col + 512], m1[:], m2[:])
                        else:
                            kf = sp.tile([64, 512], f32, name=f'kf{l}')
                            nc.vector.tensor_scalar_mul(kf[:], ps[0:64, :], sqkv[0:64, 2:3])
                            ks = sp.tile([64, 512], f32, name=f'ks{l}')
                            nc.scalar.copy(ks[0:32, :], kf[32:64, :])
                            nc.scalar.copy(ks[32:64, :], kf[0:32, :])
                            m1k = sp.tile([64, 512], f32, name=f'm1k{l}')
                            nc.vector.tensor_mul(m1k[:], kf[:], C['cosq'][0:64, col:col + 512])
                            m2k = sp.tile([64, 512], f32, name=f'm2k{l}')
                            nc.vector.tensor_mul(m2k[:], ks[:], C['sinq'][0:64, col:col + 512])
                            nc.vector.tensor_add(ksb[0:64, col:col + 512], m1k[:], m2k[:])
                            nc.vector.tensor_add(ksb[64:128, col:col + 512], m1k[:], m2k[:])
                            vf = sp.tile([64, 512], bf16, name=f'vf{l}')
                            nc.vector.tensor_scalar_mul(vf[:], ps[64:128, :], sqkv[64:128, 2:3])
                            b = tb // 2
                            for j in range(4):
                                kb = (tb % 2) * 4 + j
                                vps = pp.tile([128, 64], f32, name=f'vtp{l}', bufs=2)
                                nc.tensor.matmul(vps[:], vf[:, j * 128:(j + 1) * 128],
                                                 C['identbf'][0:64, 0:64],
                                                 start=True, stop=True)
                                nc.scalar.copy(vt[(b, kb)][:], vps[:])

            if last_layer:
                return ksb, vt, wq_sl, ph.pop_all()

            # ---- attention core (layers 0..2) ----
            attnf = [atp.tile([128, T], bf16, name=f'af{l}_{m}') for m in range(2)]
            with tc.tile_pool(name=f'sc_{l}', bufs=2) as sp, \
                 tc.tile_pool(name=f'scp_{l}', bufs=2, space='PSUM') as pp:
                for b in range(B):
                    for h in range(QH):
                        qrows = ((h % 2) * 64, (h % 2) * 64 + 64)
                        qt_tile = qsb[h // 2]
                        psb = []
                        dgs = []
                        for qt in range(8):
                            W = (qt + 1) * 128
                            sps = pp.tile([128, 1024], f32, name=f'sps{l}', bufs=2)
                            for c0 in range(0, W, 512):
                                cw = min(512, W - c0)
                                nc.tensor.matmul(
                                    sps[:, c0:c0 + cw],
                                    qt_tile[qrows[0]:qrows[1],
                                            b * 1024 + qt * 128: b * 1024 + qt * 128 + 128],
                                    ksb[qrows[0]:qrows[1],
                                        b * 1024 + c0: b * 1024 + c0 + cw],
                                    start=True, stop=True)
                            nc.vector.tensor_add(sps[:, qt * 128:W],
                                                 sps[:, qt * 128:W], C['trimask'][:])
                            nm = sp.tile([128, 1], f32, name=f'nm{l}', bufs=3)
                            nc.vector.tensor_reduce(out=nm[:], in_=sps[:, 0:W],
                                                    axis=AX, op=mybir.AluOpType.max,
                                                    negate=True)
                            pt = sp.tile([128, 1024], bf16, name=f'pexp{l}_{qt}')
                            den = sp.tile([128, 1], f32, name=f'den{l}', bufs=3)
                            nc.scalar.activation(pt[:, 0:W], sps[:, 0:W], AF.Exp,
                                                 bias=nm[:], scale=1.0,
                                                 accum_out=den[:])
                            rden = sp.tile([128, 1], f32, name=f'rden{l}', bufs=3)
                            nc.vector.reciprocal(rden[:], den[:])
                            dg = sp.tile([128, 128], bf16, name=f'dg{l}_{qt}')
                            nc.vector.tensor_scalar_mul(dg[:], C['identbf'][:], rden[:])
                            psb.append(pt)
                            dgs.append(dg)
                        for Hh in range(2):
                            pv = pp.tile([64, 512], f32, name=f'pvps{l}', bufs=2)
                            for kb in range(4 * Hh + 4):
                                qt0 = max(kb, 4 * Hh)
                                ptp = pp.tile([128, 512], f32, name=f'ptp{l}', bufs=2)
                                for qt in range(qt0, 4 * Hh + 4):
                                    nc.tensor.matmul(
                                        ptp[:, (qt - 4 * Hh) * 128:(qt - 4 * Hh + 1) * 128],
                                        psb[qt][:, kb * 128:(kb + 1) * 128],
                                        dgs[qt][:], start=True, stop=True)
                                cs = (qt0 - 4 * Hh) * 128
                                pts = sp.tile([128, 512], bf16, name=f'pts{l}', bufs=3)
                                eng = nc.vector if kb % 2 == 0 else nc.scalar
                                (eng.tensor_copy if kb % 2 == 0 else eng.copy)(
                                    pts[:, cs:512], ptp[:, cs:512])
                                nc.tensor.matmul(pv[:, cs:512], vt[(b, kb)][:],
                                                 pts[:, cs:512],
                                                 start=(kb == 0), stop=(kb == 4 * Hh + 3))
                            nc.scalar.copy(
                                attnf[h // 2][(h % 2) * 64:(h % 2) * 64 + 64,
                                              b * 1024 + Hh * 512: b * 1024 + Hh * 512 + 512],
                                pv[:])

            # ---- o-proj ----
            so = C[f'so{l}']
            with tc.tile_pool(name=f'wo_{l}', bufs=1) as wsp2, \
                 tc.tile_pool(name=f'op_{l}', bufs=2) as sp, \
                 tc.tile_pool(name=f'opp_{l}', bufs=3, space='PSUM') as pp:
                wo8 = wsp2.tile([128, 2 * D], fp8, name=f'wo{l}')
                nc.sync.dma_start(out=wo8[:], in_=I[f'wo{l}'][:])
                for tb in range(NC):
                    oball = sp.tile([128, KC * 512], bf16, name=f'ob{l}', bufs=2)
                    for m in range(KC):
                        ops = pp.tile([128, 512], f32, name=f'ops{l}', bufs=3)
                        for kc in range(2):
                            nc.tensor.matmul(
                                ops[:], wo8[:, kc * D + m * 128: kc * D + (m + 1) * 128],
                                attnf[kc][:, tb * 512:(tb + 1) * 512],
                                start=(kc == 0), stop=(kc == 1))
                        ob = oball[:, m * 512:(m + 1) * 512]
                        if m % 2 == 0:
                            nc.vector.tensor_scalar_mul(ob[:], ops[:], so[:, m:m + 1])
                        else:
                            nc.scalar.activation(ob[:], ops[:], AF.Copy,
                                                 scale=so[:, m:m + 1])
                    nc.sync.dma_start(out=_r3(rs_in[tb * D:(tb + 1) * D, :]),
                                      in_=_s3(oball[:]))
        return None

    def mlp(l, xn_buf):
        """MLP block for layers 0..2 (full T tokens)."""
        sg, sud, sd = C[f'sg{l}'], C[f'sud{l}'], C[f'sd{l}']
        with ExitStack() as ph:
            wsp = ph.enter_context(tc.tile_pool(name=f'wm_{l}', bufs=1))
            wg8 = wsp.tile([128, KC * DFFS], fp8, name=f'wg{l}')
            nc.sync.dma_start(out=wg8[:], in_=I[f'wg{l}'][:])
            wu8 = wsp.tile([128, KC * DFFS], fp8, name=f'wu{l}')
            nc.sync.dma_start(out=wu8[:], in_=I[f'wu{l}'][:])
            wd8 = wsp.tile([128, MFF * D], fp8, name=f'wd{l}')
            nc.sync.dma_start(out=wd8[:], in_=I[f'wd{l}'][:])
            with tc.tile_pool(name=f'ml_{l}', bufs=2) as sp, \
                 tc.tile_pool(name=f'mlp_{l}', bufs=2, space='PSUM') as pp:
                for tb in range(NC):
                    xn = load_xn(sp, xn_buf, tb, f'm{l}', bufs=2)
                    hmall = sp.tile([128, MFF * 512], bf16, name=f'hm{l}', bufs=2)
                    for mf in range(MFF):
                        gps = pp.tile([128, 512], f32, name=f'gps{l}', bufs=2)
                        for k in range(KC):
                            nc.tensor.matmul(
                                gps[:], wg8[:, k * DFFS + mf * 128: k * DFFS + (mf + 1) * 128],
                                xn[k][:], start=(k == 0), stop=(k == KC - 1))
                        ups = pp.tile([128, 512], f32, name=f'ups{l}', bufs=2)
                        for k in range(KC):
                            nc.tensor.matmul(
                                ups[:], wu8[:, k * DFFS + mf * 128: k * DFFS + (mf + 1) * 128],
                                xn[k][:], start=(k == 0), stop=(k == KC - 1))
                        gsb = sp.tile([128, 512], bf16, name=f'gsb{l}', bufs=2)
                        nc.scalar.activation(gsb[:], gps[:], AF.Silu,
                                             scale=sg[:, mf:mf + 1])
                        nc.vector.scalar_tensor_tensor(
                            out=hmall[:, mf * 512:(mf + 1) * 512], in0=ups[:],
                            scalar=sud[:, mf:mf + 1], in1=gsb[:], op0=MUL, op1=MUL)
                    dball = sp.tile([128, KC * 512], bf16, name=f'db{l}', bufs=2)
                    for mo in range(KC):
                        dps = pp.tile([128, 512], f32, name=f'dps{l}', bufs=3)
                        for k in range(MFF):
                            nc.tensor.matmul(
                                dps[:], wd8[:, k * D + mo * 128: k * D + (mo + 1) * 128],
                                hmall[:, k * 512:(k + 1) * 512],
                                start=(k == 0), stop=(k == MFF - 1))
                        db = dball[:, mo * 512:(mo + 1) * 512]
                        if mo % 2 == 0:
                            nc.vector.tensor_scalar_mul(db[:], dps[:], sd[:, mo:mo + 1])
                        else:
                            nc.scalar.activation(db[:], dps[:], AF.Copy,
                                                 scale=sd[:, mo:mo + 1])
                    nc.sync.dma_start(out=_r3(rs_in[tb * D:(tb + 1) * D, :]),
                                      in_=_s3(dball[:]))

    # ------------------------------------------------------------------
    # layers 0..2
    # ------------------------------------------------------------------
    for l in range(L - 1):
        agb = allgather_norm(xs, C[f'ga{l}'][:], f'a{l}')
        qkv_attention(l, agb, last_layer=False)
        reduce_scatter_add(f'o{l}')
        agb = allgather_norm(xs, C[f'gm{l}'][:], f'm{l}')
        mlp(l, agb)
        reduce_scatter_add(f'd{l}')

    # ------------------------------------------------------------------
    # layer 3 (last): only last-position tokens through q/attn/mlp/head
    # ------------------------------------------------------------------
    l = L - 1
    lx_in = dram_p.tile([D, 1], f32, name='lx_in')
    lx_out = dram_p.tile([NC * D, 1], f32, name='lx_out',
                         addr_space=('Local' if TLSIM else 'Shared'))
    ar_in = dram_p.tile([D, 4], f32, name='ar_in')
    ar_out = dram_p.tile([D, 4], f32, name='ar_out',
                         addr_space=('Local' if TLSIM else 'Shared'))
    ar2_in = dram_p.tile([D, 4], f32, name='ar2_in')
    ar2_out = dram_p.tile([D, 4], f32, name='ar2_out',
                          addr_space=('Local' if TLSIM else 'Shared'))

    for k in range(KC):
        nc.sync.dma_start(out=lx_in[k * 128:(k + 1) * 128, :],
                          in_=xs[k][:, 511:512])
    if TLSIM:
        for r in range(NC):
            nc.sync.dma_start(out=lx_out[r * D:(r + 1) * D, :], in_=lx_in[:])
    else:
        nc.gpsimd.collective_compute('AllGather', mybir.AluOpType.bypass,
                                     replica_groups=RG, ins=[lx_in.opt()],
                                     outs=[lx_out.opt()])
    l4p = top.enter_context(tc.tile_pool(name='l4p', bufs=1))
    lastx = []
    for k in range(KC):
        t = l4p.tile([128, 4], f32, name=f'lastx{k}')
        src = bass.AP(tensor=lx_out.tensor, offset=lx_out[:].offset + D + k * 128,
                      ap=[[1, 128], [2 * D, 4]])
        nc.sync.dma_start(out=t[:], in_=src)
        lastx.append(t)

    # norm for q (on last-position tokens)
    qn4 = [l4p.tile([128, 4], bf16, name=f'qn4_{k}') for k in range(KC)]
    _fm_norm(nc, tc, ctx, lastx, C[f'ga{l}'][:], 4, qn4, 'q4')

    # full norm + AG for k/v
    agb = allgather_norm(xs, C[f'ga{l}'][:], f'a{l}')
    ksb, vt, wq_sl, wctx = qkv_attention(l, agb, last_layer=True)

    sqkv = C[f'sqkv{l}']
    q4h = [l4p.tile([64, 4], bf16, name=f'q4h_{h}') for h in range(QH)]
    at4 = [l4p.tile([64, 4], bf16, name=f'at4_{h}') for h in range(QH)]
    with tc.tile_pool(name='l4qs', bufs=2) as sp, \
         tc.tile_pool(name='l4qp', bufs=2, space='PSUM') as pp:
        for m in range(2):
            ps = pp.tile([128, 4], f32, name='q4ps', bufs=2)
            for k in range(KC):
                nc.tensor.matmul(ps[:], wq_sl(k, m), qn4[k][:],
                                 start=(k == 0), stop=(k == KC - 1))
            qf = sp.tile([128, 4], f32, name='q4f')
            nc.vector.tensor_scalar_mul(qf[:], ps[:], sqkv[:, m:m + 1])
            qs = sp.tile([128, 4], f32, name='q4s')
            for g, src_g in ((0, 1), (1, 0), (2, 3), (3, 2)):
                nc.vector.tensor_copy(qs[g * 32:(g + 1) * 32, :],
                                      qf[src_g * 32:(src_g + 1) * 32, :])
            m1 = sp.tile([128, 4], f32, name='q4m1')
            nc.vector.tensor_mul(m1[:], qf[:], C['cosq4'][:])
            m2 = sp.tile([128, 4], f32, name='q4m2')
            nc.vector.tensor_mul(m2[:], qs[:], C['sinq4'][:])
            for sub in range(2):
                nc.vector.tensor_add(q4h[m * 2 + sub][:],
                                     m1[sub * 64:(sub + 1) * 64, :],
                                     m2[sub * 64:(sub + 1) * 64, :])

    # attention for 4 last tokens
    with tc.tile_pool(name='l4as', bufs=2) as sp, \
         tc.tile_pool(name='l4ap', bufs=1, space='PSUM') as pp:
        for b in range(B):
            for h in range(QH):
                s4 = pp.tile([1, 1024], f32, name='s4ps', bufs=2)
                for c0 in range(0, 1024, 512):
                    nc.tensor.matmul(s4[:, c0:c0 + 512],
                                     q4h[h][:, b:b + 1],
                                     ksb[0:64, b * 1024 + c0: b * 1024 + c0 + 512],
                                     start=True, stop=True)
                nm = sp.tile([1, 1], f32, name='nm4', bufs=3)
                nc.vector.tensor_reduce(out=nm[:], in_=s4[:], axis=AX,
                                        op=mybir.AluOpType.max, negate=True)
                p4 = sp.tile([1, 1024], bf16, name='p4', bufs=2)
                den = sp.tile([1, 1], f32, name='den4', bufs=3)
                nc.scalar.activation(p4[:], s4[:], AF.Exp, bias=nm[:], scale=1.0,
                                     accum_out=den[:])
                rden = sp.tile([1, 1], f32, name='rden4', bufs=3)
                nc.vector.reciprocal(rden[:], den[:])
                rbcp = pp.tile([128, 1], f32, name='rbcp', bufs=1)
                nc.tensor.matmul(rbcp[:], C['ones_m'][:], rden[:], start=True, stop=True)
                rbc = sp.tile([128, 1], f32, name='rbc4', bufs=3)
                nc.scalar.copy(rbc[:], rbcp[:])
                pt4p = pp.tile([128, 8], f32, name='pt4p', bufs=1)
                for kb in range(8):
                    nc.tensor.matmul(pt4p[:, kb:kb + 1], p4[:, kb * 128:(kb + 1) * 128],
                                     C['onebf'][:], start=True, stop=True)
                pt4 = sp.tile([128, 8], bf16, name='pt4', bufs=2)
                nc.vector.tensor_scalar_mul(pt4[:], pt4p[:], rbc[:])
                pv4 = pp.tile([64, 1], f32, name='pv4', bufs=2)
                for kb in range(8):
                    nc.tensor.matmul(pv4[:], vt[(b, kb)][:], pt4[:, kb:kb + 1],
                                     start=(kb == 0), stop=(kb == 7))
                nc.scalar.copy(at4[h][:, b:b + 1], pv4[:])
    wctx.close()

    # o-proj for 4 tokens
    so = C[f'so{l}']
    with tc.tile_pool(name='wo3', bufs=1) as wsp2, \
         tc.tile_pool(name='l4os', bufs=2) as sp, \
         tc.tile_pool(name='l4op', bufs=2, space='PSUM') as pp:
        wo4h = []
        for h in range(QH):
            r0 = (h % 2) * 64
            wt = wsp2.tile([64, D], fp8, name=f'wo4t_{h}')
            nc.sync.dma_start(out=wt[:],
                              in_=I[f'wo{l}'][r0:r0 + 64, (h // 2) * D:(h // 2 + 1) * D])
            wo4h.append(wt)
        for m in range(KC):
            ops = pp.tile([128, 4], f32, name='o4ps', bufs=2)
            for h in range(QH):
                nc.tensor.matmul(
                    ops[:], wo4h[h][:, m * 128:(m + 1) * 128],
                    at4[h][:], start=(h == 0), stop=(h == QH - 1))
            ob = sp.tile([128, 4], f32, name='o4b', bufs=3)
            nc.vector.tensor_scalar_mul(ob[:], ops[:], so[:, m:m + 1])
            nc.sync.dma_start(out=ar_in[m * 128:(m + 1) * 128, :], in_=ob[:])

    if TLSIM:
        nc.sync.dma_start(out=ar_out[:], in_=ar_in[:])
    else:
        nc.gpsimd.collective_compute('AllReduce', mybir.AluOpType.add,
                                     replica_groups=RG, ins=[ar_in.opt()],
                                     outs=[ar_out.opt()])

    # residual add (4 tokens)
    x4 = []
    with tc.tile_pool(name='l4r', bufs=3) as sp:
        for k in range(KC):
            rt = sp.tile([128, 4], f32, name='ar4l')
            nc.sync.dma_start(out=rt[:], in_=ar_out[k * 128:(k + 1) * 128, :])
            t = l4p.tile([128, 4], f32, name=f'x4_{k}')
            nc.vector.tensor_add(t[:], lastx[k][:], rt[:])
            x4.append(t)

    # norm2 + tiny MLP
    xn4 = [l4p.tile([128, 4], bf16, name=f'xn4_{k}') for k in range(KC)]
    _fm_norm(nc, tc, ctx, x4, C[f'gm{l}'][:], 4, xn4, 'm4')
    sg, sud, sd = C[f'sg{l}'], C[f'sud{l}'], C[f'sd{l}']
    with ExitStack() as ph:
        wsp = ph.enter_context(tc.tile_pool(name='wm3', bufs=1))
        wg8 = wsp.tile([128, KC * DFFS], fp8, name='wg3t')
        nc.sync.dma_start(out=wg8[:], in_=I[f'wg{l}'][:])
        wu8 = wsp.tile([128, KC * DFFS], fp8, name='wu3t')
        nc.sync.dma_start(out=wu8[:], in_=I[f'wu{l}'][:])
        wd8 = wsp.tile([128, MFF * D], fp8, name='wd3t')
        nc.sync.dma_start(out=wd8[:], in_=I[f'wd{l}'][:])
        with tc.tile_pool(name='m4s', bufs=2) as sp, \
             tc.tile_pool(name='m4p', bufs=2, space='PSUM') as pp:
            hm = []
            for mf in range(MFF):
                gps = pp.tile([128, 4], f32, name='g4ps', bufs=2)
                for k in range(KC):
                    nc.tensor.matmul(
                        gps[:], wg8[:, k * DFFS + mf * 128: k * DFFS + (mf + 1) * 128],
                        xn4[k][:], start=(k == 0), stop=(k == KC - 1))
                ups = pp.tile([128, 4], f32, name='u4ps', bufs=2)
                for k in range(KC):
                    nc.tensor.matmul(
                        ups[:], wu8[:, k * DFFS + mf * 128: k * DFFS + (mf + 1) * 128],
                        xn4[k][:], start=(k == 0), stop=(k == KC - 1))
                gsb = sp.tile([128, 4], bf16, name='g4sb', bufs=3)
                nc.scalar.activation(gsb[:], gps[:], AF.Silu, scale=sg[:, mf:mf + 1])
                ht = sp.tile([128, 4], bf16, name=f'h4_{mf}')
                nc.vector.scalar_tensor_tensor(
                    out=ht[:], in0=ups[:], scalar=sud[:, mf:mf + 1],
                    in1=gsb[:], op0=MUL, op1=MUL)
                hm.append(ht)
            for mo in range(KC):
                dps = pp.tile([128, 4], f32, name='d4ps', bufs=2)
                for k in range(MFF):
                    nc.tensor.matmul(
                        dps[:], wd8[:, k * D + mo * 128: k * D + (mo + 1) * 128],
                        hm[k][:], start=(k == 0), stop=(k == MFF - 1))
                db = sp.tile([128, 4], f32, name='d4b', bufs=3)
                nc.vector.tensor_scalar_mul(db[:], dps[:], sd[:, mo:mo + 1])
                nc.sync.dma_start(out=ar2_in[mo * 128:(mo + 1) * 128, :], in_=db[:])

    if TLSIM:
        nc.sync.dma_start(out=ar2_out[:], in_=ar2_in[:])
    else:
        nc.gpsimd.collective_compute('AllReduce', mybir.AluOpType.add,
                                     replica_groups=RG, ins=[ar2_in.opt()],
                                     outs=[ar2_out.opt()])

    # final residual + final norm + LM head
    with tc.tile_pool(name='fhs', bufs=2) as sp, \
         tc.tile_pool(name='fhp', bufs=2, space='PSUM') as pp:
        xf = []
        for k in range(KC):
            rt = sp.tile([128, 4], f32, name='ar4l2', bufs=3)
            nc.sync.dma_start(out=rt[:], in_=ar2_out[k * 128:(k + 1) * 128, :])
            t = l4p.tile([128, 4], f32, name=f'xf_{k}')
            nc.vector.tensor_add(t[:], x4[k][:], rt[:])
            xf.append(t)
        xfn = [l4p.tile([128, 4], bf16, name=f'xfn_{k}') for k in range(KC)]
        _fm_norm(nc, tc, ctx, xf, C['gf'][:], 4, xfn, 'f4')
        nch = (VS + 511) // 512
        for n in range(nch):
            cw = min(512, VS - n * 512)
            hps = pp.tile([4, 512], f32, name='hps', bufs=2)
            et = sp.tile([128, KC * 512], bf16, name='embt', bufs=3)
            nc.sync.dma_start(
                out=et[:].rearrange("p (k c) -> p k c", k=KC)[:, :, 0:cw],
                in_=_r3(I['embT'][:, n * 512:n * 512 + cw]))
            for k in range(KC):
                nc.tensor.matmul(hps[:, 0:cw], xfn[k][:],
                                 et[:, k * 512:k * 512 + cw],
                                 start=(k == 0), stop=(k == KC - 1))
            lsb = sp.tile([4, 512], f32, name='lsb', bufs=3)
            nc.scalar.copy(lsb[:, 0:cw], hps[:, 0:cw])
            nc.sync.dma_start(out=logits_out[:, n * 512:n * 512 + cw],
                              in_=lsb[:, 0:cw])


# ----------------------------------------------------------------------------
# host-side prep
# ----------------------------------------------------------------------------

def _chunk_pack(a, nchunks):
    """[(nchunks*128), cols] -> [128, nchunks*cols] fp8 (chunk-major in free dim)."""
    cols = a.shape[1]
    return np.ascontiguousarray(
        a.reshape(nchunks, 128, cols).transpose(1, 0, 2).reshape(128, nchunks * cols)
        .astype(ml_dtypes.float8_e4m3))


def _prep_in_maps(token_ids, embed, gamma_attn, gamma_mlp, gamma_final,
                  wq, sq, wk, sk, wv, sv, wo, so, wg, sg, wu, su, wd, sd):
    half = HD // 2
    inv = ROPE_THETA ** (-np.arange(half, dtype=np.float32) * 2.0 / HD)
    ang = np.arange(S, dtype=np.float32)[:, None] * inv          # [S, 32]
    cos1 = np.cos(ang).T.astype(np.float32)                      # [32, S]
    sin1 = np.sin(ang).T.astype(np.float32)
    cos64 = np.concatenate([cos1, cos1], 0)                      # [64, S]
    sin64s = np.concatenate([-sin1, sin1], 0)
    cosq = np.tile(np.concatenate([cos64, cos64], 0), (1, B))    # [128, T]
    sinq = np.tile(np.concatenate([sin64s, sin64s], 0), (1, B))
    cosq4 = np.repeat(cosq[:, S - 1:S], 4, axis=1).copy()
    sinq4 = np.repeat(sinq[:, S - 1:S], 4, axis=1).copy()

    ii, jj = np.meshgrid(np.arange(128), np.arange(128), indexing='ij')
    trimask = np.where(jj <= ii, 0.0, NEG).astype(np.float32)

    tok = np.asarray(token_ids).reshape(T)
    x0full = np.ascontiguousarray(embed[tok].T.astype(np.float32))  # [D, T]
    embT = np.ascontiguousarray(embed.T.astype(ml_dtypes.bfloat16))  # [D, V]

    def percol(a):
        return np.ascontiguousarray(a.reshape(-1, 128).T.astype(np.float32))

    common = {
        'cosq': np.ascontiguousarray(cosq.astype(ml_dtypes.bfloat16)),
        'sinq': np.ascontiguousarray(sinq.astype(ml_dtypes.bfloat16)),
        'cosq4': cosq4, 'sinq4': sinq4, 'trimask': trimask,
        'identbf': np.eye(128, dtype=ml_dtypes.bfloat16),
        'ones_k': np.ones((128, 1), np.float32),
        'ones_m': np.ones((1, 128), np.float32),
        'onebf': np.ones((1, 1), ml_dtypes.bfloat16),
        'gf': percol(gamma_final),
    }
    in_maps = []
    for c in range(NC):
        m = dict(common)
        m['x0'] = np.ascontiguousarray(x0full[:, c * TB:(c + 1) * TB])
        m['embT'] = np.ascontiguousarray(embT[:, c * VS:(c + 1) * VS])
        for l in range(L):
            qsl = slice(c * DQ, (c + 1) * DQ)
            ksl = slice(c * HD, (c + 1) * HD)
            fsl = slice(c * DFFS, (c + 1) * DFFS)
            m[f'wqkv{l}'] = _chunk_pack(np.concatenate(
                [wq[l][qsl].T, wk[l][ksl].T, wv[l][ksl].T], axis=1), KC)
            sq_l = sq[l][qsl] * np.float32(1.0 / np.sqrt(HD))
            m[f'sqkv{l}'] = np.ascontiguousarray(np.stack(
                [sq_l[0:128], sq_l[128:256],
                 np.concatenate([sk[l][ksl], sv[l][ksl]])], axis=1).astype(np.float32))
            m[f'wo{l}'] = _chunk_pack(wo[l][:, qsl].T, 2)
            m[f'so{l}'] = percol(so[l])
            m[f'wg{l}'] = _chunk_pack(wg[l][fsl].T, KC)
            m[f'sg{l}'] = percol(sg[l][fsl])
            m[f'wu{l}'] = _chunk_pack(wu[l][fsl].T, KC)
            m[f'wd{l}'] = _chunk_pack(wd[l][:, fsl].T, MFF)
            m[f'sud{l}'] = percol(su[l][fsl])
            m[f'sd{l}'] = percol(sd[l])
            m[f'ga{l}'] = percol(gamma_attn[l])
            m[f'gm{l}'] = percol(gamma_mlp[l])
        in_maps.append(m)
    return in_maps


def _get_nc():
    if 'nc' not in _CACHE:
        _CACHE['nc'] = _build()
    return _CACHE['nc']


def kernel(**inputs) -> np.ndarray:
    inputs = {k: np.asarray(v) for k, v in inputs.items()}
    in_maps = _prep_in_maps(**inputs)
    nc = _get_nc()
    res = bass_utils.run_bass_kernel_spmd(nc, in_maps, core_ids=list(range(NC)))
    logits = np.concatenate([res.results[c]['logits'] for c in range(NC)], axis=1)
    return logits.astype(np.float32)


# revision 11
# speedup vs baseline: 13.4569x; 1.0128x over previous
"""Self-contained Trainium2 Bass kernel for the int4-quantized 4-layer Llama decode problem.

Strategy: tensor-parallel over 8 NeuronCores (attention heads + FFN hidden dim),
sequence-parallel residual (each core keeps a feature-major fp32 residual shard
[D, T/8] in SBUF), AllGather before QKV/MLP, ReduceScatter after o-proj/down-proj.
Weights are host-packed to fp8e4 (int4 values are exact in e4m3) and used directly
as the stationary matmul operand against bf16 activations; dequant scales are
applied to the matmul outputs (per-out-channel) or to the activations (down-proj
input channels). Activation/residual traffic between SBUF and the DRAM collective
bounce buffers moves in single strided DMAs per 2048-feature block.
Only the last position of each sequence goes through layer-4 Q/attention/MLP and
the LM head.
"""
import sys

sys.path.insert(0, '/opt/trn_rl_repo')

import numpy as np
import ml_dtypes
from contextlib import ExitStack

import concourse.bass as bass
import concourse.tile as tile
from concourse import bacc, mybir
from concourse import bass_utils

# model dims (hardcoded per problem spec)
L, D, H, HD, KVH, DFF, V, B, S = 4, 2048, 32, 64, 8, 8192, 32000, 4, 1024
NC = 8
T = B * S              # 4096 tokens
TB = T // NC           # 512 tokens per core shard
QH = H // NC           # 4 local q heads
DQ = QH * HD           # 256 local q dims
DFFS = DFF // NC       # 1024 local ffn dims
VS = V // NC           # 4000 local vocab
KC = D // 128          # 16 feature chunks
MFF = DFFS // 128      # 8
ROPE_THETA = 500000.0
NEG = np.float32(-1e9)
EPS = 1e-5

f32 = mybir.dt.float32
bf16 = mybir.dt.bfloat16
fp8 = mybir.dt.float8e4
i32 = mybir.dt.int32

AX = mybir.AxisListType.X
MUL = mybir.AluOpType.mult
AF = mybir.ActivationFunctionType

_CACHE = {}
TLSIM = False  # single-core cost-model sim mode (collectives -> DMA copies)


# ----------------------------------------------------------------------------
# bass program
# ----------------------------------------------------------------------------

def _declare_inputs(nc):
    I = {}
    I['x0'] = nc.dram_tensor('x0', [D, TB], f32, kind='ExternalInput').ap()
    for l in range(L):
        I[f'wqkv{l}'] = nc.dram_tensor(f'wqkv{l}', [128, KC * 384], fp8, kind='ExternalInput').ap()
        I[f'sqkv{l}'] = nc.dram_tensor(f'sqkv{l}', [128, 3], f32, kind='ExternalInput').ap()
        I[f'wo{l}'] = nc.dram_tensor(f'wo{l}', [128, 2 * D], fp8, kind='ExternalInput').ap()
        I[f'so{l}'] = nc.dram_tensor(f'so{l}', [128, KC], f32, kind='ExternalInput').ap()
        I[f'wg{l}'] = nc.dram_tensor(f'wg{l}', [128, KC * DFFS], fp8, kind='ExternalInput').ap()
        I[f'sg{l}'] = nc.dram_tensor(f'sg{l}', [128, MFF], f32, kind='ExternalInput').ap()
        I[f'wu{l}'] = nc.dram_tensor(f'wu{l}', [128, KC * DFFS], fp8, kind='ExternalInput').ap()
        I[f'wd{l}'] = nc.dram_tensor(f'wd{l}', [128, MFF * D], fp8, kind='ExternalInput').ap()
        I[f'sud{l}'] = nc.dram_tensor(f'sud{l}', [128, MFF], f32, kind='ExternalInput').ap()
        I[f'sd{l}'] = nc.dram_tensor(f'sd{l}', [128, KC], f32, kind='ExternalInput').ap()
        I[f'ga{l}'] = nc.dram_tensor(f'ga{l}', [128, KC], f32, kind='ExternalInput').ap()
        I[f'gm{l}'] = nc.dram_tensor(f'gm{l}', [128, KC], f32, kind='ExternalInput').ap()
    I['gf'] = nc.dram_tensor('gf', [128, KC], f32, kind='ExternalInput').ap()
    I['cosq'] = nc.dram_tensor('cosq', [128, T], bf16, kind='ExternalInput').ap()
    I['sinq'] = nc.dram_tensor('sinq', [128, T], bf16, kind='ExternalInput').ap()
    I['cosq4'] = nc.dram_tensor('cosq4', [128, 4], f32, kind='ExternalInput').ap()
    I['sinq4'] = nc.dram_tensor('sinq4', [128, 4], f32, kind='ExternalInput').ap()
    I['trimask'] = nc.dram_tensor('trimask', [128, 128], f32, kind='ExternalInput').ap()
    I['identbf'] = nc.dram_tensor('identbf', [128, 128], bf16, kind='ExternalInput').ap()
    I['ones_k'] = nc.dram_tensor('ones_k', [128, 1], f32, kind='ExternalInput').ap()
    I['ones_m'] = nc.dram_tensor('ones_m', [1, 128], f32, kind='ExternalInput').ap()
    I['onebf'] = nc.dram_tensor('onebf', [1, 1], bf16, kind='ExternalInput').ap()
    I['embT'] = nc.dram_tensor('embT', [D, VS], bf16, kind='ExternalInput').ap()
    return I


def _fm_norm(nc, tc, ctx, src, gamma_ap, width, out_tiles, tag):
    """Feature-major rmsnorm: src = list of KC sbuf [128,width] f32 APs.
    Writes out_tiles (KC APs, caller-allocated, any dtype)."""
    with tc.tile_pool(name=f'np_{tag}', bufs=2) as sp, \
         tc.tile_pool(name=f'npp_{tag}', bufs=2, space='PSUM') as pp:
        C = ctx['const']
        ssum = pp.tile([1, width], f32, name=f'nsum_{tag}')
        for k in range(KC):
            xsq = sp.tile([128, width], f32, name=f'nxsq_{tag}', bufs=4)
            if k % 2 == 0:
                nc.vector.tensor_mul(xsq[:], src[k][:], src[k][:])
            else:
                nc.scalar.activation(xsq[:], src[k][:], AF.Square)
            nc.tensor.matmul(ssum[:], C['ones_k'][:], xsq[:],
                             start=(k == 0), stop=(k == KC - 1))
        sq = sp.tile([1, width], f32, name=f'nsq_{tag}')
        nc.scalar.activation(sq[:], ssum[:], AF.Sqrt, bias=ctx['eps'][0:1, :],
                             scale=1.0 / D)
        rstd = sp.tile([1, width], f32, name=f'nrstd_{tag}')
        nc.vector.reciprocal(rstd[:], sq[:])
        bcp = pp.tile([128, width], f32, name=f'nbc_{tag}')
        nc.tensor.matmul(bcp[:], C['ones_m'][:], rstd[:], start=True, stop=True)
        rbc = sp.tile([128, width], f32, name=f'nrbc_{tag}')
        nc.scalar.copy(rbc[:], bcp[:])
        for k in range(KC):
            nc.vector.scalar_tensor_tensor(
                out=out_tiles[k][:], in0=src[k][:], scalar=gamma_ap[:, k:k + 1],
                in1=rbc[:], op0=MUL, op1=MUL)


def _r3(dram_ap, nchunks=KC):
    """[(k p), c] DRAM slice -> [p, k, c] AP for batched strided DMA."""
    return dram_ap.rearrange("(k p) c -> p k c", k=nchunks)


def _s3(sb_ap, nchunks=KC):
    """[p, (k c)] SBUF tile -> [p, k, c] AP."""
    return sb_ap.rearrange("p (k c) -> p k c", k=nchunks)


def _build(reps=1):
    nc = bacc.Bacc('TRN2', target_bir_lowering=False, debug=False,
                   num_devices=(1 if TLSIM else NC))
    I = _declare_inputs(nc)
    logits_out = nc.dram_tensor('logits', [4, VS], f32, kind='ExternalOutput').ap()

    with tile.TileContext(nc) as tc, ExitStack() as top:
        const_p = top.enter_context(tc.tile_pool(name='constp', bufs=1))
        resid_p = top.enter_context(tc.tile_pool(name='residp', bufs=1))
        dram_p = top.enter_context(tc.tile_pool(name='dramp', bufs=1, space='DRAM'))

        C = {}
        for cn, shape, dt in [('cosq', [128, T], bf16), ('sinq', [128, T], bf16),
                              ('cosq4', [128, 4], f32), ('sinq4', [128, 4], f32),
                              ('trimask', [128, 128], f32), ('identbf', [128, 128], bf16),
                              ('ones_k', [128, 1], f32), ('ones_m', [1, 128], f32),
                              ('onebf', [1, 1], bf16), ('gf', [128, KC], f32)]:
            t = const_p.tile(shape, dt, name=f'c_{cn}')
            nc.sync.dma_start(out=t[:], in_=I[cn][:])
            C[cn] = t
        for l in range(L):
            for cn in ('sqkv', 'so', 'sg', 'sud', 'sd', 'ga', 'gm'):
                shp = [128, {'sqkv': 3, 'so': KC, 'sg': MFF, 'sud': MFF,
                             'sd': KC, 'ga': KC, 'gm': KC}[cn]]
                t = const_p.tile(shp, f32, name=f'c_{cn}{l}')
                nc.sync.dma_start(out=t[:], in_=I[f'{cn}{l}'][:])
                C[f'{cn}{l}'] = t
        epst = const_p.tile([128, 1], f32, name='c_eps')
        nc.vector.memset(epst[:], EPS)
        ctx = {'const': C, 'eps': epst}

        for _rep in range(reps):
            _body(nc, tc, top, I, C, ctx, dram_p, resid_p, logits_out)

    nc.compile()
    return nc


def _body(nc, tc, top, I, C, ctx, dram_p, resid_p, logits_out):
    # persistent residual shard [D, TB] fp32 as one [128, KC*TB] tile
    xsb = resid_p.tile([128, KC * TB], f32, name='xsh')
    nc.sync.dma_start(out=_s3(xsb[:]), in_=_r3(I['x0'][:]))
    xs = [xsb[:, k * TB:(k + 1) * TB] for k in range(KC)]

    # DRAM bounce buffers for collectives
    rs_in = dram_p.tile([NC * D, TB], bf16, name='rs_in')
    rs_out = dram_p.tile([D, TB], bf16, name='rs_out')
    RG = [list(range(NC))]

    def allgather_norm(src_tiles, gamma_ap, tag):
        """norm src -> bf16 -> ag_in -> AllGather; returns ag_out tile."""
        ag_in = dram_p.tile([D, TB], fp8, name=f'ag_in_{tag}')
        ag_out = dram_p.tile([NC * D, TB], fp8, name=f'ag_out_{tag}',
                             addr_space=('Local' if TLSIM else 'Shared'))
        with tc.tile_pool(name=f'agp_{tag}', bufs=2) as sp:
            xnall = sp.tile([128, KC * TB], fp8, name=f'xn_{tag}')
            outs = [xnall[:, k * TB:(k + 1) * TB] for k in range(KC)]
            _fm_norm(nc, tc, ctx, src_tiles, gamma_ap, TB, outs, tag)
            nc.sync.dma_start(out=_r3(ag_in[:]), in_=_s3(xnall[:]))
        if TLSIM:
            for r in range(NC):
                nc.sync.dma_start(out=ag_out[r * D:(r + 1) * D, :], in_=ag_in[:])
        else:
            nc.gpsimd.collective_compute(
                'AllGather', mybir.AluOpType.bypass, replica_groups=RG,
                ins=[ag_in.opt()], outs=[ag_out.opt()])
        return ag_out

    def reduce_scatter_add(tag):
        """ReduceScatter rs_in -> rs_out; add into xs."""
        if TLSIM:
            nc.sync.dma_start(out=rs_out[:], in_=rs_in[0:D, :])
        else:
            nc.gpsimd.collective_compute(
                'ReduceScatter', mybir.AluOpType.add, replica_groups=RG,
                ins=[rs_in.opt()], outs=[rs_out.opt()])
        with tc.tile_pool(name=f'rsp_{tag}', bufs=2) as sp:
            rt = sp.tile([128, KC * TB], bf16, name=f'rs_{tag}')
            nc.sync.dma_start(out=_s3(rt[:]), in_=_r3(rs_out[:]))
            for k in range(KC):
                nc.vector.tensor_add(xs[k][:], xs[k][:], rt[:, k * TB:(k + 1) * TB])

    def load_xn(sp, xn_buf, tb, tag, bufs=3):
        xnb = sp.tile([128, KC * 512], fp8, name=f'xnl_{tag}', bufs=bufs)
        nc.sync.dma_start(out=_s3(xnb[:]),
                          in_=_r3(xn_buf[tb * D:(tb + 1) * D, :]))
        return [xnb[:, k * 512:(k + 1) * 512] for k in range(KC)]

    # ------------------------------------------------------------------
    # per-layer phases
    # ------------------------------------------------------------------

    def qkv_attention(l, xn_buf, last_layer):
        """Full attention block for layer l. For last_layer, q/attention are
        computed only for the 4 last-position tokens."""
        sqkv = C[f'sqkv{l}']
        with ExitStack() as ph:
            wsp = ph.enter_context(tc.tile_pool(name=f'wq_{l}', bufs=1))
            wq8 = wsp.tile([128, KC * 384], fp8, name=f'wqkv{l}')
            nc.sync.dma_start(out=wq8[:], in_=I[f'wqkv{l}'][:])

            def wq_sl(k, m):
                return wq8[:, k * 384 + m * 128: k * 384 + (m + 1) * 128]

            atp = ph.enter_context(tc.tile_pool(name=f'at_{l}', bufs=1))
            qsb = None
            if not last_layer:
                qsb = [atp.tile([128, T], bf16, name=f'qsb{l}_{m}') for m in range(2)]
            ksb = atp.tile([128, T], bf16, name=f'ksb{l}')
            vt = {}
            for b in range(B):
                for kb in range(8):
                    vt[(b, kb)] = atp.tile([128, 64], bf16, name=f'vt{l}_{b}_{kb}')

            with tc.tile_pool(name=f'qk_{l}', bufs=2) as sp, \
                 tc.tile_pool(name=f'qkp_{l}', bufs=2, space='PSUM') as pp:
                for tb in range(NC):
                    xn = load_xn(sp, xn_buf, tb, f'q{l}', bufs=3)
                    col = tb * 512
                    mlist = [2] if last_layer else [0, 1, 2]
                    for m in mlist:
                        ps = pp.tile([128, 512], f32, name=f'qkvps{l}', bufs=4)
                        for k in range(KC):
                            nc.tensor.matmul(
                                ps[:], wq_sl(k, m), xn[k][:],
                                start=(k == 0), stop=(k == KC - 1))
                        if m < 2:
                            qf = sp.tile([128, 512], f32, name=f'qf{l}')
                            nc.vector.tensor_scalar_mul(qf[:], ps[:], sqkv[:, m:m + 1])
                            qs = sp.tile([128, 512], f32, name=f'qs{l}')
                            for g, src_g in ((0, 1), (1, 0), (2, 3), (3, 2)):
                                eng = nc.scalar if g % 2 == 0 else nc.vector
                                (eng.copy if g % 2 == 0 else eng.tensor_copy)(
                                    qs[g * 32:(g + 1) * 32, :],
                                    qf[src_g * 32:(src_g + 1) * 32, :])
                            m1 = sp.tile([128, 512], f32, name=f'm1{l}')
                            nc.vector.tensor_mul(m1[:], qf[:], C['cosq'][:, col:col + 512])
                            m2 = sp.tile([128, 512], f32, name=f'm2{l}')
                            nc.vector.tensor_mul(m2[:], qs[:], C['sinq'][:, col:col + 512])
                            nc.vector.tensor_add(qsb[m][:, col:col + 512], m1[:], m2[:])
                        else:
                            kf = sp.tile([64, 512], f32, name=f'kf{l}')
                            nc.vector.tensor_scalar_mul(kf[:], ps[0:64, :], sqkv[0:64, 2:3])
                            ks = sp.tile([64, 512], f32, name=f'ks{l}')
                            nc.scalar.copy(ks[0:32, :], kf[32:64, :])
                            nc.scalar.copy(ks[32:64, :], kf[0:32, :])
                            m1k = sp.tile([64, 512], f32, name=f'm1k{l}')
                            nc.vector.tensor_mul(m1k[:], kf[:], C['cosq'][0:64, col:col + 512])
                            m2k = sp.tile([64, 512], f32, name=f'm2k{l}')
                            nc.vector.tensor_mul(m2k[:], ks[:], C['sinq'][0:64, col:col + 512])
                            nc.vector.tensor_add(ksb[0:64, col:col + 512], m1k[:], m2k[:])
                            nc.vector.tensor_add(ksb[64:128, col:col + 512], m1k[:], m2k[:])
                            vf = sp.tile([64, 512], bf16, name=f'vf{l}')
                            nc.vector.tensor_scalar_mul(vf[:], ps[64:128, :], sqkv[64:128, 2:3])
                            b = tb // 2
                            for j in range(4):
                                kb = (tb % 2) * 4 + j
                                vps = pp.tile([128, 64], f32, name=f'vtp{l}', bufs=2)
                                nc.tensor.matmul(vps[:], vf[:, j * 128:(j + 1) * 128],
                                                 C['identbf'][0:64, 0:64],
                                                 start=True, stop=True)
                                nc.scalar.copy(vt[(b, kb)][:], vps[:])

            if last_layer:
                return ksb, vt, wq_sl, ph.pop_all()

            # ---- attention core (layers 0..2) ----
            attnf = [atp.tile([128, T], bf16, name=f'af{l}_{m}') for m in range(2)]
            with tc.tile_pool(name=f'sc_{l}', bufs=2) as sp, \
                 tc.tile_pool(name=f'scp_{l}', bufs=2, space='PSUM') as pp:
                for b in range(B):
                    for h in range(QH):
                        qrows = ((h % 2) * 64, (h % 2) * 64 + 64)
                        qt_tile = qsb[h // 2]
                        psb = []
                        dgs = []
                        for qt in range(8):
                            W = (qt + 1) * 128
                            sps = pp.tile([128, 1024], f32, name=f'sps{l}', bufs=2)
                            for c0 in range(0, W, 512):
                                cw = min(512, W - c0)
                                nc.tensor.matmul(
                                    sps[:, c0:c0 + cw],
                                    qt_tile[qrows[0]:qrows[1],
                                            b * 1024 + qt * 128: b * 1024 + qt * 128 + 128],
                                    ksb[qrows[0]:qrows[1],
                                        b * 1024 + c0: b * 1024 + c0 + cw],
                                    start=True, stop=True)
                            nc.vector.tensor_add(sps[:, qt * 128:W],
                                                 sps[:, qt * 128:W], C['trimask'][:])
                            nm = sp.tile([128, 1], f32, name=f'nm{l}', bufs=3)
                            nc.vector.tensor_reduce(out=nm[:], in_=sps[:, 0:W],
                                                    axis=AX, op=mybir.AluOpType.max,
                                                    negate=True)
                            pt = sp.tile([128, 1024], bf16, name=f'pexp{l}_{qt}')
                            den = sp.tile([128, 1], f32, name=f'den{l}', bufs=3)
                            nc.scalar.activation(pt[:, 0:W], sps[:, 0:W], AF.Exp,
                                                 bias=nm[:], scale=1.0,
                                                 accum_out=den[:])
                            rden = sp.tile([128, 1], f32, name=f'rden{l}', bufs=3)
                            nc.vector.reciprocal(rden[:], den[:])
                            dg = sp.tile([128, 128], bf16, name=f'dg{l}_{qt}')
                            nc.vector.tensor_scalar_mul(dg[:], C['identbf'][:], rden[:])
                            psb.append(pt)
                            dgs.append(dg)
                        for Hh in range(2):
                            pv = pp.tile([64, 512], f32, name=f'pvps{l}', bufs=2)
                            for kb in range(4 * Hh + 4):
                                qt0 = max(kb, 4 * Hh)
                                ptp = pp.tile([128, 512], f32, name=f'ptp{l}', bufs=2)
                                for qt in range(qt0, 4 * Hh + 4):
                                    nc.tensor.matmul(
                                        ptp[:, (qt - 4 * Hh) * 128:(qt - 4 * Hh + 1) * 128],
                                        psb[qt][:, kb * 128:(kb + 1) * 128],
                                        dgs[qt][:], start=True, stop=True)
                                cs = (qt0 - 4 * Hh) * 128
                                pts = sp.tile([128, 512], bf16, name=f'pts{l}', bufs=3)
                                eng = nc.vector if kb % 2 == 0 else nc.scalar
                                (eng.tensor_copy if kb % 2 == 0 else eng.copy)(
                                    pts[:, cs:512], ptp[:, cs:512])
                                nc.tensor.matmul(pv[:, cs:512], vt[(b, kb)][:],
                                                 pts[:, cs:512],
                                                 start=(kb == 0), stop=(kb == 4 * Hh + 3))
                            nc.scalar.copy(
                                attnf[h // 2][(h % 2) * 64:(h % 2) * 64 + 64,
                                              b * 1024 + Hh * 512: b * 1024 + Hh * 512 + 512],
                                pv[:])

            # ---- o-proj ----
            so = C[f'so{l}']
            with tc.tile_pool(name=f'wo_{l}', bufs=1) as wsp2, \
                 tc.tile_pool(name=f'op_{l}', bufs=2) as sp, \
                 tc.tile_pool(name=f'opp_{l}', bufs=3, space='PSUM') as pp:
                wo8 = wsp2.tile([128, 2 * D], fp8, name=f'wo{l}')
                nc.sync.dma_start(out=wo8[:], in_=I[f'wo{l}'][:])
                for tb in range(NC):
                    oball = sp.tile([128, KC * 512], bf16, name=f'ob{l}', bufs=2)
                    for m in range(KC):
                        ops = pp.tile([128, 512], f32, name=f'ops{l}', bufs=3)
                        for kc in range(2):
                            nc.tensor.matmul(
                                ops[:], wo8[:, kc * D + m * 128: kc * D + (m + 1) * 128],
                                attnf[kc][:, tb * 512:(tb + 1) * 512],
                                start=(kc == 0), stop=(kc == 1))
                        ob = oball[:, m * 512:(m + 1) * 512]
                        if m % 2 == 0:
                            nc.vector.tensor_scalar_mul(ob[:], ops[:], so[:, m:m + 1])
                        else:
                            nc.scalar.activation(ob[:], ops[:], AF.Copy,
                                                 scale=so[:, m:m + 1])
                    nc.sync.dma_start(out=_r3(rs_in[tb * D:(tb + 1) * D, :]),
                                      in_=_s3(oball[:]))
        return None

    def mlp(l, xn_buf):
        """MLP block for layers 0..2 (full T tokens)."""
        sg, sud, sd = C[f'sg{l}'], C[f'sud{l}'], C[f'sd{l}']
        with ExitStack() as ph:
            wsp = ph.enter_context(tc.tile_pool(name=f'wm_{l}', bufs=1))
            wg8 = wsp.tile([128, KC * DFFS], fp8, name=f'wg{l}')
            nc.sync.dma_start(out=wg8[:], in_=I[f'wg{l}'][:])
            wu8 = wsp.tile([128, KC * DFFS], fp8, name=f'wu{l}')
            nc.sync.dma_start(out=wu8[:], in_=I[f'wu{l}'][:])
            wd8 = wsp.tile([128, MFF * D], fp8, name=f'wd{l}')
            nc.sync.dma_start(out=wd8[:], in_=I[f'wd{l}'][:])
            with tc.tile_pool(name=f'ml_{l}', bufs=2) as sp, \
                 tc.tile_pool(name=f'mlp_{l}', bufs=2, space='PSUM') as pp:
                for tb in range(NC):
                    xn = load_xn(sp, xn_buf, tb, f'm{l}', bufs=2)
                    hmall = sp.tile([128, MFF * 512], bf16, name=f'hm{l}', bufs=2)
                    for mf in range(MFF):
                        gps = pp.tile([128, 512], f32, name=f'gps{l}', bufs=2)
                        for k in range(KC):
                            nc.tensor.matmul(
                                gps[:], wg8[:, k * DFFS + mf * 128: k * DFFS + (mf + 1) * 128],
                                xn[k][:], start=(k == 0), stop=(k == KC - 1))
                        ups = pp.tile([128, 512], f32, name=f'ups{l}', bufs=2)
                        for k in range(KC):
                            nc.tensor.matmul(
                                ups[:], wu8[:, k * DFFS + mf * 128: k * DFFS + (mf + 1) * 128],
                                xn[k][:], start=(k == 0), stop=(k == KC - 1))
                        gsb = sp.tile([128, 512], bf16, name=f'gsb{l}', bufs=2)
                        nc.scalar.activation(gsb[:], gps[:], AF.Silu,
                                             scale=sg[:, mf:mf + 1])
                        nc.vector.scalar_tensor_tensor(
                            out=hmall[:, mf * 512:(mf + 1) * 512], in0=ups[:],
                            scalar=sud[:, mf:mf + 1], in1=gsb[:], op0=MUL, op1=MUL)
                    dball = sp.tile([128, KC * 512], bf16, name=f'db{l}', bufs=2)
                    for mo in range(KC):
                        dps = pp.tile([128, 512], f32, name=f'dps{l}', bufs=3)
                        for k in range(MFF):
                            nc.tensor.matmul(
                                dps[:], wd8[:, k * D + mo * 128: k * D + (mo + 1) * 128],
                                hmall[:, k * 512:(k + 1) * 512],
                                start=(k == 0), stop=(k == MFF - 1))
                        db = dball[:, mo * 512:(mo + 1) * 512]
                        if mo % 2 == 0:
                            nc.vector.tensor_scalar_mul(db[:], dps[:], sd[:, mo:mo + 1])
                        else:
                            nc.scalar.activation(db[:], dps[:], AF.Copy,
                                                 scale=sd[:, mo:mo + 1])
                    nc.sync.dma_start(out=_r3(rs_in[tb * D:(tb + 1) * D, :]),
                                      in_=_s3(dball[:]))

    # ------------------------------------------------------------------
    # layers 0..2
    # ------------------------------------------------------------------
    for l in range(L - 1):
        agb = allgather_norm(xs, C[f'ga{l}'][:], f'a{l}')
        qkv_attention(l, agb, last_layer=False)
        reduce_scatter_add(f'o{l}')
        agb = allgather_norm(xs, C[f'gm{l}'][:], f'm{l}')
        mlp(l, agb)
        reduce_scatter_add(f'd{l}')

    # ------------------------------------------------------------------
    # layer 3 (last): only last-position tokens through q/attn/mlp/head
    # ------------------------------------------------------------------
    l = L - 1
    lx_in = dram_p.tile([D, 1], f32, name='lx_in')
    lx_out = dram_p.tile([NC * D, 1], f32, name='lx_out',
                         addr_space=('Local' if TLSIM else 'Shared'))
    ar_in = dram_p.tile([D, 4], f32, name='ar_in')
    ar_out = dram_p.tile([D, 4], f32, name='ar_out',
                         addr_space=('Local' if TLSIM else 'Shared'))
    ar2_in = dram_p.tile([D, 4], f32, name='ar2_in')
    ar2_out = dram_p.tile([D, 4], f32, name='ar2_out',
                          addr_space=('Local' if TLSIM else 'Shared'))

    for k in range(KC):
        nc.sync.dma_start(out=lx_in[k * 128:(k + 1) * 128, :],
                          in_=xs[k][:, 511:512])
    if TLSIM:
        for r in range(NC):
            nc.sync.dma_start(out=lx_out[r * D:(r + 1) * D, :], in_=lx_in[:])
    else:
        nc.gpsimd.collective_compute('AllGather', mybir.AluOpType.bypass,
                                     replica_groups=RG, ins=[lx_in.opt()],
                                     outs=[lx_out.opt()])
    l4p = top.enter_context(tc.tile_pool(name='l4p', bufs=1))
    lastx = []
    for k in range(KC):
        t = l4p.tile([128, 4], f32, name=f'lastx{k}')
        src = bass.AP(tensor=lx_out.tensor, offset=lx_out[:].offset + D + k * 128,
                      ap=[[1, 128], [2 * D, 4]])
        nc.sync.dma_start(out=t[:], in_=src)
        lastx.append(t)

    # norm for q (on last-position tokens)
    qn4 = [l4p.tile([128, 4], bf16, name=f'qn4_{k}') for k in range(KC)]
    _fm_norm(nc, tc, ctx, lastx, C[f'ga{l}'][:], 4, qn4, 'q4')

    # full norm + AG for k/v
    agb = allgather_norm(xs, C[f'ga{l}'][:], f'a{l}')
    ksb, vt, wq_sl, wctx = qkv_attention(l, agb, last_layer=True)

    sqkv = C[f'sqkv{l}']
    q4h = [l4p.tile([64, 4], bf16, name=f'q4h_{h}') for h in range(QH)]
    at4 = [l4p.tile([64, 4], bf16, name=f'at4_{h}') for h in range(QH)]
    with tc.tile_pool(name='l4qs', bufs=2) as sp, \
         tc.tile_pool(name='l4qp', bufs=2, space='PSUM') as pp:
        for m in range(2):
            ps = pp.tile([128, 4], f32, name='q4ps', bufs=2)
            for k in range(KC):
                nc.tensor.matmul(ps[:], wq_sl(k, m), qn4[k][:],
                                 start=(k == 0), stop=(k == KC - 1))
            qf = sp.tile([128, 4], f32, name='q4f')
            nc.vector.tensor_scalar_mul(qf[:], ps[:], sqkv[:, m:m + 1])
            qs = sp.tile([128, 4], f32, name='q4s')
            for g, src_g in ((0, 1), (1, 0), (2, 3), (3, 2)):
                nc.vector.tensor_copy(qs[g * 32:(g + 1) * 32, :],
                                      qf[src_g * 32:(src_g + 1) * 32, :])
            m1 = sp.tile([128, 4], f32, name='q4m1')
            nc.vector.tensor_mul(m1[:], qf[:], C['cosq4'][:])
            m2 = sp.tile([128, 4], f32, name='q4m2')
            nc.vector.tensor_mul(m2[:], qs[:], C['sinq4'][:])
            for sub in range(2):
                nc.vector.tensor_add(q4h[m * 2 + sub][:],
                                     m1[sub * 64:(sub + 1) * 64, :],
                                     m2[sub * 64:(sub + 1) * 64, :])

    # attention for 4 last tokens
    with tc.tile_pool(name='l4as', bufs=2) as sp, \
         tc.tile_pool(name='l4ap', bufs=1, space='PSUM') as pp:
        for b in range(B):
            for h in range(QH):
                s4 = pp.tile([1, 1024], f32, name='s4ps', bufs=2)
                for c0 in range(0, 1024, 512):
                    nc.tensor.matmul(s4[:, c0:c0 + 512],
                                     q4h[h][:, b:b + 1],
                                     ksb[0:64, b * 1024 + c0: b * 1024 + c0 + 512],
                                     start=True, stop=True)
                nm = sp.tile([1, 1], f32, name='nm4', bufs=3)
                nc.vector.tensor_reduce(out=nm[:], in_=s4[:], axis=AX,
                                        op=mybir.AluOpType.max, negate=True)
                p4 = sp.tile([1, 1024], bf16, name='p4', bufs=2)
                den = sp.tile([1, 1], f32, name='den4', bufs=3)
                nc.scalar.activation(p4[:], s4[:], AF.Exp, bias=nm[:], scale=1.0,
                                     accum_out=den[:])
                rden = sp.tile([1, 1], f32, name='rden4', bufs=3)
                nc.vector.reciprocal(rden[:], den[:])
                rbcp = pp.tile([128, 1], f32, name='rbcp', bufs=1)
                nc.tensor.matmul(rbcp[:], C['ones_m'][:], rden[:], start=True, stop=True)
                rbc = sp.tile([128, 1], f32, name='rbc4', bufs=3)
                nc.scalar.copy(rbc[:], rbcp[:])
                pt4p = pp.tile([128, 8], f32, name='pt4p', bufs=1)
                for kb in range(8):
                    nc.tensor.matmul(pt4p[:, kb:kb + 1], p4[:, kb * 128:(kb + 1) * 128],
                                     C['onebf'][:], start=True, stop=True)
                pt4 = sp.tile([128, 8], bf16, name='pt4', bufs=2)
                nc.vector.tensor_scalar_mul(pt4[:], pt4p[:], rbc[:])
                pv4 = pp.tile([64, 1], f32, name='pv4', bufs=2)
                for kb in range(8):
                    nc.tensor.matmul(pv4[:], vt[(b, kb)][:], pt4[:, kb:kb + 1],
                                     start=(kb == 0), stop=(kb == 7))
                nc.scalar.copy(at4[h][:, b:b + 1], pv4[:])
    wctx.close()

    # o-proj for 4 tokens
    so = C[f'so{l}']
    with tc.tile_pool(name='wo3', bufs=1) as wsp2, \
         tc.tile_pool(name='l4os', bufs=2) as sp, \
         tc.tile_pool(name='l4op', bufs=2, space='PSUM') as pp:
        wo4h = []
        for h in range(QH):
            r0 = (h % 2) * 64
            wt = wsp2.tile([64, D], fp8, name=f'wo4t_{h}')
            nc.sync.dma_start(out=wt[:],
                              in_=I[f'wo{l}'][r0:r0 + 64, (h // 2) * D:(h // 2 + 1) * D])
            wo4h.append(wt)
        for m in range(KC):
            ops = pp.tile([128, 4], f32, name='o4ps', bufs=2)
            for h in range(QH):
                nc.tensor.matmul(
                    ops[:], wo4h[h][:, m * 128:(m + 1) * 128],
                    at4[h][:], start=(h == 0), stop=(h == QH - 1))
            ob = sp.tile([128, 4], f32, name='o4b', bufs=3)
            nc.vector.tensor_scalar_mul(ob[:], ops[:], so[:, m:m + 1])
            nc.sync.dma_start(out=ar_in[m * 128:(m + 1) * 128, :], in_=ob[:])

    if TLSIM:
        nc.sync.dma_start(out=ar_out[:], in_=ar_in[:])
    else:
        nc.gpsimd.collective_compute('AllReduce', mybir.AluOpType.add,
                                     replica_groups=RG, ins=[ar_in.opt()],
                                     outs=[ar_out.opt()])

    # residual add (4 tokens)
    x4 = []
    with tc.tile_pool(name='l4r', bufs=3) as sp:
        for k in range(KC):
            rt = sp.tile([128, 4], f32, name='ar4l')
            nc.sync.dma_start(out=rt[:], in_=ar_out[k * 128:(k + 1) * 128, :])
            t = l4p.tile([128, 4], f32, name=f'x4_{k}')
            nc.vector.tensor_add(t[:], lastx[k][:], rt[:])
            x4.append(t)

    # norm2 + tiny MLP
    xn4 = [l4p.tile([128, 4], bf16, name=f'xn4_{k}') for k in range(KC)]
    _fm_norm(nc, tc, ctx, x4, C[f'gm{l}'][:], 4, xn4, 'm4')
    sg, sud, sd = C[f'sg{l}'], C[f'sud{l}'], C[f'sd{l}']
    with ExitStack() as ph:
        wsp = ph.enter_context(tc.tile_pool(name='wm3', bufs=1))
        wg8 = wsp.tile([128, KC * DFFS], fp8, name='wg3t')
        nc.sync.dma_start(out=wg8[:], in_=I[f'wg{l}'][:])
        wu8 = wsp.tile([128, KC * DFFS], fp8, name='wu3t')
        nc.sync.dma_start(out=wu8[:], in_=I[f'wu{l}'][:])
        wd8 = wsp.tile([128, MFF * D], fp8, name='wd3t')
        nc.sync.dma_start(out=wd8[:], in_=I[f'wd{l}'][:])
        with tc.tile_pool(name='m4s', bufs=2) as sp, \
             tc.tile_pool(name='m4p', bufs=2, space='PSUM') as pp:
            hm = []
            for mf in range(MFF):
                gps = pp.tile([128, 4], f32, name='g4ps', bufs=2)
                for k in range(KC):
                    nc.tensor.matmul(
                        gps[:], wg8[:, k * DFFS + mf * 128: k * DFFS + (mf + 1) * 128],
                        xn4[k][:], start=(k == 0), stop=(k == KC - 1))
                ups = pp.tile([128, 4], f32, name='u4ps', bufs=2)
                for k in range(KC):
                    nc.tensor.matmul(
                        ups[:], wu8[:, k * DFFS + mf * 128: k * DFFS + (mf + 1) * 128],
                        xn4[k][:], start=(k == 0), stop=(k == KC - 1))
                gsb = sp.tile([128, 4], bf16, name='g4sb', bufs=3)
                nc.scalar.activation(gsb[:], gps[:], AF.Silu, scale=sg[:, mf:mf + 1])
                ht = sp.tile([128, 4], bf16, name=f'h4_{mf}')
                nc.vector.scalar_tensor_tensor(
                    out=ht[:], in0=ups[:], scalar=sud[:, mf:mf + 1],
                    in1=gsb[:], op0=MUL, op1=MUL)
                hm.append(ht)
            for mo in range(KC):
                dps = pp.tile([128, 4], f32, name='d4ps', bufs=2)
                for k in range(MFF):
                    nc.tensor.matmul(
                        dps[:], wd8[:, k * D + mo * 128: k * D + (mo + 1) * 128],
                        hm[k][:], start=(k == 0), stop=(k == MFF - 1))
                db = sp.tile([128, 4], f32, name='d4b', bufs=3)
                nc.vector.tensor_scalar_mul(db[:], dps[:], sd[:, mo:mo + 1])
                nc.sync.dma_start(out=ar2_in[mo * 128:(mo + 1) * 128, :], in_=db[:])

    if TLSIM:
        nc.sync.dma_start(out=ar2_out[:], in_=ar2_in[:])
    else:
        nc.gpsimd.collective_compute('AllReduce', mybir.AluOpType.add,
                                     replica_groups=RG, ins=[ar2_in.opt()],
                                     outs=[ar2_out.opt()])

    # final residual + final norm + LM head
    with tc.tile_pool(name='fhs', bufs=2) as sp, \
         tc.tile_pool(name='fhp', bufs=2, space='PSUM') as pp:
        xf = []
        for k in range(KC):
            rt = sp.tile([128, 4], f32, name='ar4l2', bufs=3)
            nc.sync.dma_start(out=rt[:], in_=ar2_out[k * 128:(k + 1) * 128, :])
            t = l4p.tile([128, 4], f32, name=f'xf_{k}')
            nc.vector.tensor_add(t[:], x4[k][:], rt[:])
            xf.append(t)
        xfn = [l4p.tile([128, 4], bf16, name=f'xfn_{k}') for k in range(KC)]
        _fm_norm(nc, tc, ctx, xf, C['gf'][:], 4, xfn, 'f4')
        nch = (VS + 511) // 512
        for n in range(nch):
            cw = min(512, VS - n * 512)
            hps = pp.tile([4, 512], f32, name='hps', bufs=2)
            et = sp.tile([128, KC * 512], bf16, name='embt', bufs=3)
            nc.sync.dma_start(
                out=et[:].rearrange("p (k c) -> p k c", k=KC)[:, :, 0:cw],
                in_=_r3(I['embT'][:, n * 512:n * 512 + cw]))
            for k in range(KC):
                nc.tensor.matmul(hps[:, 0:cw], xfn[k][:],
                                 et[:, k * 512:k * 512 + cw],
                                 start=(k == 0), stop=(k == KC - 1))
            lsb = sp.tile([4, 512], f32, name='lsb', bufs=3)
            nc.scalar.copy(lsb[:, 0:cw], hps[:, 0:cw])
            nc.sync.dma_start(out=logits_out[:, n * 512:n * 512 + cw],
                              in_=lsb[:, 0:cw])


# ----------------------------------------------------------------------------
# host-side prep
# ----------------------------------------------------------------------------

def _chunk_pack(a, nchunks):
    """[(nchunks*128), cols] -> [128, nchunks*cols] fp8 (chunk-major in free dim)."""
    cols = a.shape[1]
    return np.ascontiguousarray(
        a.reshape(nchunks, 128, cols).transpose(1, 0, 2).reshape(128, nchunks * cols)
        .astype(ml_dtypes.float8_e4m3))


def _prep_in_maps(token_ids, embed, gamma_attn, gamma_mlp, gamma_final,
                  wq, sq, wk, sk, wv, sv, wo, so, wg, sg, wu, su, wd, sd):
    half = HD // 2
    inv = ROPE_THETA ** (-np.arange(half, dtype=np.float32) * 2.0 / HD)
    ang = np.arange(S, dtype=np.float32)[:, None] * inv          # [S, 32]
    cos1 = np.cos(ang).T.astype(np.float32)                      # [32, S]
    sin1 = np.sin(ang).T.astype(np.float32)
    cos64 = np.concatenate([cos1, cos1], 0)                      # [64, S]
    sin64s = np.concatenate([-sin1, sin1], 0)
    cosq = np.tile(np.concatenate([cos64, cos64], 0), (1, B))    # [128, T]
    sinq = np.tile(np.concatenate([sin64s, sin64s], 0), (1, B))
    cosq4 = np.repeat(cosq[:, S - 1:S], 4, axis=1).copy()
    sinq4 = np.repeat(sinq[:, S - 1:S], 4, axis=1).copy()

    ii, jj = np.meshgrid(np.arange(128), np.arange(128), indexing='ij')
    trimask = np.where(jj <= ii, 0.0, NEG).astype(np.float32)

    tok = np.asarray(token_ids).reshape(T)
    x0full = np.ascontiguousarray(embed[tok].T.astype(np.float32))  # [D, T]
    embT = np.ascontiguousarray(embed.T.astype(ml_dtypes.bfloat16))  # [D, V]

    def percol(a):
        return np.ascontiguousarray(a.reshape(-1, 128).T.astype(np.float32))

    common = {
        'cosq': np.ascontiguousarray(cosq.astype(ml_dtypes.bfloat16)),
        'sinq': np.ascontiguousarray(sinq.astype(ml_dtypes.bfloat16)),
        'cosq4': cosq4, 'sinq4': sinq4, 'trimask': trimask,
        'identbf': np.eye(128, dtype=ml_dtypes.bfloat16),
        'ones_k': np.ones((128, 1), np.float32),
        'ones_m': np.ones((1, 128), np.float32),
        'onebf': np.ones((1, 1), ml_dtypes.bfloat16),
        'gf': percol(gamma_final),
    }
    in_maps = []
    for c in range(NC):
        m = dict(common)
        m['x0'] = np.ascontiguousarray(x0full[:, c * TB:(c + 1) * TB])
        m['embT'] = np.ascontiguousarray(embT[:, c * VS:(c + 1) * VS])
        for l in range(L):
            qsl = slice(c * DQ, (c + 1) * DQ)
            ksl = slice(c * HD, (c + 1) * HD)
            fsl = slice(c * DFFS, (c + 1) * DFFS)
            m[f'wqkv{l}'] = _chunk_pack(np.concatenate(
                [wq[l][qsl].T, wk[l][ksl].T, wv[l][ksl].T], axis=1), KC)
            sq_l = sq[l][qsl] * np.float32(1.0 / np.sqrt(HD))
            m[f'sqkv{l}'] = np.ascontiguousarray(np.stack(
                [sq_l[0:128], sq_l[128:256],
                 np.concatenate([sk[l][ksl], sv[l][ksl]])], axis=1).astype(np.float32))
            m[f'wo{l}'] = _chunk_pack(wo[l][:, qsl].T, 2)
            m[f'so{l}'] = percol(so[l])
            m[f'wg{l}'] = _chunk_pack(wg[l][fsl].T, KC)
            m[f'sg{l}'] = percol(sg[l][fsl])
            m[f'wu{l}'] = _chunk_pack(wu[l][fsl].T, KC)
            m[f'wd{l}'] = _chunk_pack(wd[l][:, fsl].T, MFF)
            m[f'sud{l}'] = percol(su[l][fsl])
            m[f'sd{l}'] = percol(sd[l])
            m[f'ga{l}'] = percol(gamma_attn[l])
            m[f'gm{l}'] = percol(gamma_mlp[l])
        in_maps.append(m)
    return in_maps


def _get_nc():
    if 'nc' not in _CACHE:
        _CACHE['nc'] = _build()
    return _CACHE['nc']


def kernel(**inputs) -> np.ndarray:
    inputs = {k: np.asarray(v) for k, v in inputs.items()}
    in_maps = _prep_in_maps(**inputs)
    nc = _get_nc()
    res = bass_utils.run_bass_kernel_spmd(nc, in_maps, core_ids=list(range(NC)))
    logits = np.concatenate([res.results[c]['logits'] for c in range(NC)], axis=1)
    return logits.astype(np.float32)
